# revision 29
# baseline (speedup 1.0000x reference)
"""Trainium2 Bass kernel: RMSNorm + RoPE + causal attention + output projection.

Tensor-parallel over heads: 16 heads / 8 cores = 2 heads per core.
Each core computes a full [S, D] partial output (its heads' contribution to
the 'snh,dnh->sd' projection); the all-reduce is done host-side in the gather.

v2 design (fused streaming, bf16):
  - Host prep uploads the RMSNorm'd activations already transposed (h^T
    [D, S] bf16) plus bf16 weights, a pre-transposed wo^T, and fp32 RoPE
    tables.  No PE transposes and no DRAM scratch roundtrip remain.
  - Single pass over 8 q-chunks of 512: QK projections (+RoPE) append to
    per-head K^T in SBUF, V is projected directly into natural [t, hd]
    layout (ht-tile stationary), then causal attention for the chunk runs
    against all K/V tiles so far, followed by the inline output projection
    and a bf16 DMA of the partial output rows.
  - Scores are computed transposed (S^T[t, s]); softmax denominators via a
    ones-stationary matmul accumulating in PSUM; Z/PV lag scores/exp by 2
    (software pipeline) so a late exp never stalls the in-order PE queue.
  - PSUM: one pool, four 2-bank tag rings: main (proj pp / scores sp),
    aux (rope ps / V accum / outproj op), o (PV accum), z (denominator).
"""
import os
import sys
import types

import numpy as np

SEQ, D, NH, HD = 4096, 2048, 16, 128
NCORES = 8
HPC = NH // NCORES          # heads per core
M = HPC * HD                # per-core fused head dim (256)
EPS = 1e-6
ROPE_BASE = 10000.0
SM_SCALE = 1.0 / np.sqrt(HD)
CHUNK = 512                 # q-chunk
NCHUNK = SEQ // CHUNK       # 8
NT = SEQ // 128             # 32 s-tiles
DT = D // 128               # 16 d-tiles
LAG = 2


def _inject_ntff_hook():
    """Register the axon NTFF profiling hook (missing antenv.axon_hooks)."""
    if "antenv.axon_hooks" in sys.modules:
        return
    try:
        import antenv
        from trn_agent_boot.trn_boot import _ntff_profile_via_ctypes
    except ImportError:
        return
    holder = [None]
    mod = types.ModuleType("antenv.axon_hooks")
    mod.set_axon_ntff_profile_hook = lambda h: holder.__setitem__(0, h)
    mod.get_axon_ntff_profile_hook = lambda: holder[0]
    sys.modules["antenv.axon_hooks"] = mod
    antenv.axon_hooks = mod
    try:
        mod.set_axon_ntff_profile_hook(
            _ntff_profile_via_ctypes("/opt/axon/libaxon_pjrt.so"))
    except Exception:
        pass


def _build_nc():
    import concourse.bass as bass  # noqa: F401
    import concourse.mybir as mybir
    import concourse.tile as tile
    from concourse import bacc

    FP32 = mybir.dt.float32
    BF16 = mybir.dt.bfloat16
    AF = mybir.ActivationFunctionType
    ALU = mybir.AluOpType

    nc = bacc.Bacc(None, target_bir_lowering=False)

    ht_d = nc.declare_dram_parameter("ht", [128, DT, SEQ], BF16,
                                     isOutput=False)
    wq = nc.declare_dram_parameter("wq", [128, DT * M], BF16, isOutput=False)
    wk = nc.declare_dram_parameter("wk", [128, DT * M], BF16, isOutput=False)
    wv = nc.declare_dram_parameter("wv", [128, DT * M], BF16, isOutput=False)
    wot_d = nc.declare_dram_parameter("wot", [128, HPC * D], BF16,
                                      isOutput=False)
    cosd = nc.declare_dram_parameter("cosd", [128, SEQ], FP32, isOutput=False)
    sind = nc.declare_dram_parameter("sind", [128, SEQ], FP32, isOutput=False)
    tri = nc.declare_dram_parameter("tri", [128, 128], BF16, isOutput=False)
    ones = nc.declare_dram_parameter("ones", [128, 128], BF16, isOutput=False)
    out = nc.declare_dram_parameter("out", [SEQ, D], BF16, isOutput=True)

    with tile.TileContext(nc) as tc:
        with tc.tile_pool(name="consts", bufs=1) as consts, \
             tc.tile_pool(name="pht", bufs=2) as pht, \
             tc.tile_pool(name="pqt", bufs=4) as pqt, \
             tc.tile_pool(name="ppc", bufs=2) as ppc, \
             tc.tile_pool(name="ppt", bufs=6) as ppt, \
             tc.tile_pool(name="prz", bufs=2) as prz, \
             tc.tile_pool(name="pat", bufs=4) as pat, \
             tc.tile_pool(name="post", bufs=3) as post, \
             tc.tile_pool(name="pcs", bufs=2) as pcs, \
             tc.tile_pool(name="psum", bufs=2, space="PSUM") as psum:
            kt_sb = [consts.tile([128, SEQ], BF16, name=f"kt{h}")
                     for h in range(HPC)]
            # V natural, packed per t-tile: vn[p, jt*M + m] = V[jt*128+p, m]
            vn_sb = consts.tile([128, NT * M], BF16)

            def load_ht(c):
                cs = slice(c * CHUNK, (c + 1) * CHUNK)
                ht = pht.tile([128, DT, CHUNK], BF16, name="ht")
                for dt8 in range(2):
                    nc.sync.dma_start(
                        out=ht[:, dt8 * 8:(dt8 + 1) * 8, :],
                        in_=ht_d[:, dt8 * 8:(dt8 + 1) * 8, cs])
                return ht

            def load_cs(c):
                cs = slice(c * CHUNK, (c + 1) * CHUNK)
                sin_t = pcs.tile([128, CHUNK], FP32, name="sin_t", tag="sin")
                nc.sync.dma_start(out=sin_t[:], in_=sind[:, cs])
                cos_t = pcs.tile([128, CHUNK], FP32, name="cos_t", tag="cos")
                nc.sync.dma_start(out=cos_t[:], in_=cosd[:, cs])
                return cos_t, sin_t

            # startup: one sync-queue FIFO in priority order — ring
            # bandwidth serves the first projections' pieces (ht, wq,
            # rope slices) before the bulk loads behind them.
            w_sbs = {k: consts.tile([128, DT, M], BF16, name=f"w{k}_sb")
                     for k in ("q", "k", "v")}
            cs0 = slice(0, CHUNK)
            ht = pht.tile([128, DT, CHUNK], BF16, name="ht")
            for q4 in range(4):
                dts = slice(q4 * 4, (q4 + 1) * 4)
                nc.sync.dma_start(out=ht[:, dts, :], in_=ht_d[:, dts, cs0])
                nc.sync.dma_start(
                    out=w_sbs["q"][:, dts, :].rearrange("p t m -> p (t m)"),
                    in_=wq[:, q4 * 4 * M:(q4 + 1) * 4 * M])
                if q4 == 1:
                    cs_cur = load_cs(0)
            nc.sync.dma_start(
                out=w_sbs["k"][:].rearrange("p t m -> p (t m)"), in_=wk[:])
            nc.sync.dma_start(
                out=w_sbs["v"][:].rearrange("p t m -> p (t m)"), in_=wv[:])
            tri_sb = consts.tile([128, 128], BF16)
            nc.sync.dma_start(out=tri_sb[:], in_=tri[:])
            ones_sb = consts.tile([128, 128], BF16)
            nc.sync.dma_start(out=ones_sb[:], in_=ones[:])
            wot_sb = consts.tile([128, HPC * D], BF16)
            nc.sync.dma_start(out=wot_sb[:], in_=wot_d[:])

            # HAM warm-up: keep the PE busy on junk matmuls while the
            # startup DMAs land, so real matmuls start at the warm clock.
            warm = consts.tile([128, 128], BF16)
            nc.vector.memset(warm[:], 0.0)
            for i in range(100):
                wp = psum.tile([128, CHUNK], FP32, name="wp", tag="main")
                nc.tensor.matmul(wp[:, 0:128], warm[:], warm[:],
                                 start=True, stop=True)

            def outproj(ats, c, spread=False):
                for st in range(4):
                    ost = post.tile([128, D], BF16, name="ost")
                    for dc in range(4):
                        # the final chunk's projection has nothing after it
                        # to hide evac latency: spread over two tag rings so
                        # 4 banks ping-pong instead of 2.
                        tag = "main" if spread and dc % 2 else "aux"
                        op = psum.tile([128, CHUNK], FP32, name="op",
                                       tag=tag)
                        nc.tensor.matmul(
                            op[:],
                            ats[0][:, st * 128:(st + 1) * 128],
                            wot_sb[:, dc * CHUNK:(dc + 1) * CHUNK],
                            start=True, stop=False)
                        nc.tensor.matmul(
                            op[:],
                            ats[1][:, st * 128:(st + 1) * 128],
                            wot_sb[:, D + dc * CHUNK:D + (dc + 1) * CHUNK],
                            start=False, stop=True)
                        if dc % 2 == 0:
                            nc.scalar.activation(
                                ost[:, dc * CHUNK:(dc + 1) * CHUNK],
                                op[:], AF.Copy)
                        else:
                            nc.vector.tensor_copy(
                                ost[:, dc * CHUNK:(dc + 1) * CHUNK], op[:])
                    row = (c * 4 + st) * 128
                    eng = nc.sync if st % 2 == 0 else nc.scalar
                    eng.dma_start(out=out[row:row + 128, :], in_=ost[:])

            ats_prev = None
            for c in range(NCHUNK):
                cs = slice(c * CHUNK, (c + 1) * CHUNK)
                cos_t, sin_t = cs_cur

                # ---- QK projections + RoPE
                qts = []
                for head in range(HPC):
                    for kind in ("q", "k"):
                        w_sb = w_sbs[kind]
                        pp = psum.tile([128, CHUNK], FP32, name="pp",
                                       tag="main")
                        for dt in range(DT):
                            nc.tensor.matmul(
                                pp[:],
                                w_sb[:, dt, head * HD:(head + 1) * HD],
                                ht[:, dt, :],
                                start=(dt == 0), stop=(dt == DT - 1))
                        ps = psum.tile([128, CHUNK], FP32, name="ps",
                                       tag="aux")
                        nc.vector.tensor_tensor(
                            out=ps[:], in0=pp[:], in1=sin_t[:],
                            op=ALU.mult)
                        pc = ppc.tile([128, CHUNK], FP32, name="pc")
                        nc.vector.tensor_tensor(
                            out=pc[:], in0=pp[:], in1=cos_t[:],
                            op=ALU.mult)
                        if kind == "q":
                            dst = pqt.tile([128, CHUNK], BF16,
                                           name=f"qt{head}")
                            qts.append(dst)
                            d0, d1 = dst[0:64, :], dst[64:128, :]
                        else:
                            d0 = kt_sb[head][0:64, cs]
                            d1 = kt_sb[head][64:128, cs]
                        nc.vector.tensor_tensor(
                            out=d0, in0=pc[0:64, :], in1=ps[64:128, :],
                            op=ALU.subtract)
                        nc.vector.tensor_tensor(
                            out=d1, in0=pc[64:128, :], in1=ps[0:64, :],
                            op=ALU.add)

                # ---- V projection directly into natural [t, m] layout
                for g in range(2):
                    pv = psum.tile([128, CHUNK], FP32, name="pv", tag="aux")
                    for half in range(2):
                        st = g * 2 + half
                        for dt in range(DT):
                            nc.tensor.matmul(
                                pv[:, half * M:(half + 1) * M],
                                ht[:, dt, st * 128:(st + 1) * 128],
                                w_sbs["v"][:, dt, :],
                                start=(dt == 0), stop=(dt == DT - 1))
                    t0 = (c * 4 + g * 2) * M
                    nc.vector.tensor_copy(vn_sb[:, t0:t0 + 2 * M], pv[:])

                # prefetch next chunk's h^T + rope slices while computing
                if c + 1 < NCHUNK:
                    ht = load_ht(c + 1)
                    cs_cur = load_cs(c + 1)

                # ---- causal attention per head; the previous chunk's output
                # projection is emitted between the heads: its matmuls cover
                # head 0's recip+normalize latency and give the scalar engine
                # room to drain the evacuation copies before head 1's exps.
                ats = []
                for head in range(HPC):
                    if head == 1 and ats_prev is not None:
                        outproj(ats_prev, c - 1)
                    o_acc = psum.tile([128, CHUNK], FP32, name="o_acc",
                                      tag="o")
                    z_acc = psum.tile([128, CHUNK], FP32, name="z_acc",
                                      tag="z")
                    jmax = 4 * c + 3
                    pts = {}
                    for jj in range(jmax + 1 + LAG):
                        if jj <= jmax:
                            j = jj
                            r = j - 4 * c
                            off = max(r, 0) * 128
                            sp = psum.tile([128, CHUNK], FP32, name="sp",
                                           tag="main")
                            nc.tensor.matmul(
                                sp[:, off:],
                                kt_sb[head][:, j * 128:(j + 1) * 128],
                                qts[head][:, off:],
                                start=True, stop=True)
                            pt = ppt.tile([128, CHUNK], BF16, name="pt")
                            nc.scalar.activation(pt[:, off:], sp[:, off:],
                                                 AF.Exp,
                                                 scale=float(SM_SCALE))
                            if r >= 0:
                                nc.vector.tensor_tensor(
                                    out=pt[:, off:off + 128],
                                    in0=pt[:, off:off + 128],
                                    in1=tri_sb[:], op=ALU.mult)
                            pts[j] = (pt, off)
                        if jj >= LAG:
                            j = jj - LAG
                            pt, off = pts.pop(j)
                            nc.tensor.matmul(
                                z_acc[:, off:], ones_sb[:], pt[:, off:],
                                start=(j == 0), stop=(j == jmax),
                                skip_group_check=True)
                            nc.tensor.matmul(
                                o_acc[:, off:],
                                vn_sb[:, j * M + head * HD:
                                      j * M + (head + 1) * HD],
                                pt[:, off:],
                                start=(j == 0), stop=(j == jmax),
                                skip_group_check=True)
                    rz = prz.tile([128, CHUNK], FP32, name="rz")
                    nc.vector.reciprocal_approx_fast(out=rz[:], in_=z_acc[:])
                    at = pat.tile([128, CHUNK], BF16, name=f"at{head}")
                    nc.vector.tensor_tensor(out=at[:], in0=o_acc[:],
                                            in1=rz[:], op=ALU.mult)
                    ats.append(at)
                ats_prev = ats
            outproj(ats_prev, NCHUNK - 1, spread=True)
    nc.finalize()
    return nc


def _host_prep(xs, norm_w, wq, wk, wv, wo):
    """Fold RMSNorm into h^T upload + weights; build rope tables; bf16."""
    import ml_dtypes
    BF = ml_dtypes.bfloat16

    x64 = xs.astype(np.float64)
    istd = 1.0 / np.sqrt((x64 * x64).mean(axis=1) + EPS)    # [S]
    ht = (x64 * istd[:, None]).T.astype(BF)                 # [D, S]
    # partition-major pack: ht_pm[p, dt, s] = ht[dt*128+p, s]
    ht_pm = np.ascontiguousarray(
        ht.reshape(DT, 128, SEQ).transpose(1, 0, 2))

    def pack_w(w):  # [D, M] -> [128, DT*M] partition-major
        return np.ascontiguousarray(
            w.reshape(DT, 128, M).transpose(1, 0, 2).reshape(128, DT * M))

    nw = norm_w.astype(np.float32)[:, None, None]
    perm = np.concatenate([np.arange(0, HD, 2), np.arange(1, HD, 2)])
    wq_p = (wq * nw)[:, :, perm]
    wk_p = (wk * nw)[:, :, perm]

    inv_freq = 1.0 / (ROPE_BASE ** (np.arange(0, HD, 2, dtype=np.float32) / HD))
    pos = np.arange(SEQ, dtype=np.float32)
    ang = pos[:, None] * inv_freq[None, :]          # [S, 64]
    cos_t = np.cos(ang).T.astype(np.float32)        # [64, S]
    sin_t = np.sin(ang).T.astype(np.float32)
    cosd = np.ascontiguousarray(np.concatenate([cos_t, cos_t], 0))
    sind = np.ascontiguousarray(np.concatenate([sin_t, sin_t], 0))

    tri = np.triu(np.ones((128, 128), dtype=np.float32)).astype(BF)
    onesm = np.ones((128, 128), dtype=BF)

    common = {
        "ht": ht_pm,
        "cosd": cosd,
        "sind": sind,
        "tri": np.ascontiguousarray(tri),
        "ones": onesm,
    }
    in_maps = []
    for core in range(NCORES):
        sl = slice(core * HPC, (core + 1) * HPC)
        wot = np.transpose(wo[:, sl, :], (2, 1, 0)).reshape(128, HPC * D)
        in_maps.append({
            **common,
            "wq": pack_w(wq_p[:, sl, :].reshape(D, M).astype(BF)),
            "wk": pack_w(wk_p[:, sl, :].reshape(D, M).astype(BF)),
            "wv": pack_w((wv * nw)[:, sl, :].reshape(D, M).astype(BF)),
            "wot": np.ascontiguousarray(wot.astype(BF)),
        })
    return in_maps


def kernel(xs, norm_w, wq, wk, wv, wo):
    trace = bool(int(os.environ.get("KERNEL_TRACE", "0")))
    if trace:
        _inject_ntff_hook()
    from concourse.bass_utils import run_bass_kernel_spmd

    nc = _build_nc()
    in_maps = _host_prep(np.asarray(xs), np.asarray(norm_w), np.asarray(wq),
                         np.asarray(wk), np.asarray(wv), np.asarray(wo))
    try:
        res = run_bass_kernel_spmd(nc, in_maps, core_ids=list(range(NCORES)),
                                   trace=trace)
    except Exception:
        # transient device wedge (NRT_EXEC_UNIT_UNRECOVERABLE) recovers on
        # a fresh attempt; rebuild and retry once
        import time
        time.sleep(15)
        nc = _build_nc()
        res = run_bass_kernel_spmd(nc, in_maps, core_ids=list(range(NCORES)),
                                   trace=trace)
    if trace and res.exec_time_ns is not None:
        print(f"HW exec time: {res.exec_time_ns} ns")
    acc = np.zeros((SEQ, D), dtype=np.float64)
    for r in res.results:
        acc += r["out"].astype(np.float64)
    return acc.astype(np.float32)


if __name__ == "__main__":
    rng = np.random.default_rng(0)
    scale = 1.0 / np.sqrt(D)
    inputs = {
        "xs": rng.standard_normal((SEQ, D), dtype=np.float32),
        "norm_w": np.ones((D,), np.float32),
        "wq": rng.standard_normal((D, NH, HD), dtype=np.float32) * scale,
        "wk": rng.standard_normal((D, NH, HD), dtype=np.float32) * scale,
        "wv": rng.standard_normal((D, NH, HD), dtype=np.float32) * scale,
        "wo": rng.standard_normal((D, NH, HD), dtype=np.float32) * scale,
    }
    out = kernel(**inputs)
    print(out.shape, out.dtype, float(np.abs(out).max()))


# revision 30
# speedup vs baseline: 1.0040x; 1.0040x over previous
"""Trainium2 Bass kernel: RMSNorm + RoPE + causal attention + output projection.

Tensor-parallel over heads: 16 heads / 8 cores = 2 heads per core.
Each core computes a full [S, D] partial output (its heads' contribution to
the 'snh,dnh->sd' projection); the all-reduce is done host-side in the gather.

Fused streaming design (bf16, ~436 us vs 674 us baseline):
  - Host prep uploads the RMSNorm'd activations already transposed and
    partition-major packed (h^T as [128, DT, S] bf16) plus bf16 weights,
    a pre-transposed wo^T, and fp32 RoPE tables.  No PE transposes, no
    DRAM scratch roundtrip, and every DMA is a cheap contiguous pattern
    (strided DIRECT2D issue cost was multi-us per descriptor program).
  - Single pass over 8 q-chunks of 512: QK projections (+RoPE) append to
    per-head K^T in SBUF, V is projected directly into natural [t, hd]
    layout (ht-tile stationary), then causal attention for the chunk runs
    against all K/V tiles so far.  The previous chunk's output projection
    is emitted between the two heads' attention so its matmuls cover the
    softmax recip+normalize latency, and the next chunk's h^T/rope-slice
    DMAs prefetch under the current chunk's compute.
  - Scores are computed transposed (S^T[t, s]); softmax denominators via a
    ones-stationary matmul accumulating in PSUM; Z/PV lag scores/exp by 2
    (software pipeline) so a late exp never stalls the in-order PE queue;
    reciprocal_approx_fast instead of the 3.4us DVE reciprocal.
  - PSUM: one pool, four 2-bank tag rings: main (proj pp / scores sp),
    aux (rope ps / V accum / outproj op), o (PV accum), z (denominator).
    The final chunk's outproj spreads over main+aux (4 banks).
  - Startup: priority-ordered sync-queue DMA FIFO + 100 junk matmuls to
    hold the PE's HAM clock warm while the first loads land.
"""
import os
import sys
import types

import numpy as np

SEQ, D, NH, HD = 4096, 2048, 16, 128
NCORES = 8
HPC = NH // NCORES          # heads per core
M = HPC * HD                # per-core fused head dim (256)
EPS = 1e-6
ROPE_BASE = 10000.0
SM_SCALE = 1.0 / np.sqrt(HD)
CHUNK = 512                 # q-chunk
NCHUNK = SEQ // CHUNK       # 8
NT = SEQ // 128             # 32 s-tiles
DT = D // 128               # 16 d-tiles
LAG = 2


def _inject_ntff_hook():
    """Register the axon NTFF profiling hook (missing antenv.axon_hooks)."""
    if "antenv.axon_hooks" in sys.modules:
        return
    try:
        import antenv
        from trn_agent_boot.trn_boot import _ntff_profile_via_ctypes
    except ImportError:
        return
    holder = [None]
    mod = types.ModuleType("antenv.axon_hooks")
    mod.set_axon_ntff_profile_hook = lambda h: holder.__setitem__(0, h)
    mod.get_axon_ntff_profile_hook = lambda: holder[0]
    sys.modules["antenv.axon_hooks"] = mod
    antenv.axon_hooks = mod
    try:
        mod.set_axon_ntff_profile_hook(
            _ntff_profile_via_ctypes("/opt/axon/libaxon_pjrt.so"))
    except Exception:
        pass


def _build_nc():
    import concourse.bass as bass  # noqa: F401
    import concourse.mybir as mybir
    import concourse.tile as tile
    from concourse import bacc

    FP32 = mybir.dt.float32
    BF16 = mybir.dt.bfloat16
    AF = mybir.ActivationFunctionType
    ALU = mybir.AluOpType

    nc = bacc.Bacc(None, target_bir_lowering=False)

    ht_d = nc.declare_dram_parameter("ht", [128, DT, SEQ], BF16,
                                     isOutput=False)
    wq = nc.declare_dram_parameter("wq", [128, DT * M], BF16, isOutput=False)
    wk = nc.declare_dram_parameter("wk", [128, DT * M], BF16, isOutput=False)
    wv = nc.declare_dram_parameter("wv", [128, DT * M], BF16, isOutput=False)
    wot_d = nc.declare_dram_parameter("wot", [128, HPC * D], BF16,
                                      isOutput=False)
    cosd = nc.declare_dram_parameter("cosd", [128, SEQ], FP32, isOutput=False)
    sind = nc.declare_dram_parameter("sind", [128, SEQ], FP32, isOutput=False)
    tri = nc.declare_dram_parameter("tri", [128, 128], BF16, isOutput=False)
    ones = nc.declare_dram_parameter("ones", [128, 128], BF16, isOutput=False)
    out = nc.declare_dram_parameter("out", [SEQ, D], BF16, isOutput=True)

    with tile.TileContext(nc) as tc:
        with tc.tile_pool(name="consts", bufs=1) as consts, \
             tc.tile_pool(name="pht", bufs=2) as pht, \
             tc.tile_pool(name="pqt", bufs=4) as pqt, \
             tc.tile_pool(name="ppc", bufs=2) as ppc, \
             tc.tile_pool(name="ppt", bufs=6) as ppt, \
             tc.tile_pool(name="prz", bufs=2) as prz, \
             tc.tile_pool(name="pat", bufs=4) as pat, \
             tc.tile_pool(name="post", bufs=3) as post, \
             tc.tile_pool(name="pcs", bufs=2) as pcs, \
             tc.tile_pool(name="psum", bufs=2, space="PSUM") as psum:
            kt_sb = [consts.tile([128, SEQ], BF16, name=f"kt{h}")
                     for h in range(HPC)]
            # V natural, packed per t-tile: vn[p, jt*M + m] = V[jt*128+p, m]
            vn_sb = consts.tile([128, NT * M], BF16)

            def load_ht(c):
                cs = slice(c * CHUNK, (c + 1) * CHUNK)
                ht = pht.tile([128, DT, CHUNK], BF16, name="ht")
                for dt8 in range(2):
                    nc.sync.dma_start(
                        out=ht[:, dt8 * 8:(dt8 + 1) * 8, :],
                        in_=ht_d[:, dt8 * 8:(dt8 + 1) * 8, cs])
                return ht

            def load_cs(c):
                cs = slice(c * CHUNK, (c + 1) * CHUNK)
                sin_t = pcs.tile([128, CHUNK], FP32, name="sin_t", tag="sin")
                nc.sync.dma_start(out=sin_t[:], in_=sind[:, cs])
                cos_t = pcs.tile([128, CHUNK], FP32, name="cos_t", tag="cos")
                nc.sync.dma_start(out=cos_t[:], in_=cosd[:, cs])
                return cos_t, sin_t

            # startup: one sync-queue FIFO in priority order — ring
            # bandwidth serves the first projections' pieces (ht, wq,
            # rope slices) before the bulk loads behind them.
            w_sbs = {k: consts.tile([128, DT, M], BF16, name=f"w{k}_sb")
                     for k in ("q", "k", "v")}
            cs0 = slice(0, CHUNK)
            ht = pht.tile([128, DT, CHUNK], BF16, name="ht")
            for q4 in range(4):
                dts = slice(q4 * 4, (q4 + 1) * 4)
                nc.sync.dma_start(out=ht[:, dts, :], in_=ht_d[:, dts, cs0])
                nc.sync.dma_start(
                    out=w_sbs["q"][:, dts, :].rearrange("p t m -> p (t m)"),
                    in_=wq[:, q4 * 4 * M:(q4 + 1) * 4 * M])
                if q4 == 1:
                    cs_cur = load_cs(0)
            nc.sync.dma_start(
                out=w_sbs["k"][:].rearrange("p t m -> p (t m)"), in_=wk[:])
            nc.sync.dma_start(
                out=w_sbs["v"][:].rearrange("p t m -> p (t m)"), in_=wv[:])
            tri_sb = consts.tile([128, 128], BF16)
            nc.sync.dma_start(out=tri_sb[:], in_=tri[:])
            ones_sb = consts.tile([128, 128], BF16)
            nc.sync.dma_start(out=ones_sb[:], in_=ones[:])
            wot_sb = consts.tile([128, HPC * D], BF16)
            nc.sync.dma_start(out=wot_sb[:], in_=wot_d[:])

            # HAM warm-up: keep the PE busy on junk matmuls while the
            # startup DMAs land, so real matmuls start at the warm clock.
            warm = consts.tile([128, 128], BF16)
            nc.vector.memset(warm[:], 0.0)
            for i in range(100):
                wp = psum.tile([128, CHUNK], FP32, name="wp", tag="main")
                nc.tensor.matmul(wp[:, 0:128], warm[:], warm[:],
                                 start=True, stop=True)

            def outproj(ats, c, spread=False):
                for st in range(4):
                    ost = post.tile([128, D], BF16, name="ost")
                    for dc in range(4):
                        # the final chunk's projection has nothing after it
                        # to hide evac latency: spread over two tag rings so
                        # 4 banks ping-pong instead of 2.
                        tag = "main" if spread and dc % 2 else "aux"
                        op = psum.tile([128, CHUNK], FP32, name="op",
                                       tag=tag)
                        nc.tensor.matmul(
                            op[:],
                            ats[0][:, st * 128:(st + 1) * 128],
                            wot_sb[:, dc * CHUNK:(dc + 1) * CHUNK],
                            start=True, stop=False)
                        nc.tensor.matmul(
                            op[:],
                            ats[1][:, st * 128:(st + 1) * 128],
                            wot_sb[:, D + dc * CHUNK:D + (dc + 1) * CHUNK],
                            start=False, stop=True)
                        if dc % 2 == 0:
                            nc.scalar.activation(
                                ost[:, dc * CHUNK:(dc + 1) * CHUNK],
                                op[:], AF.Copy)
                        else:
                            nc.vector.tensor_copy(
                                ost[:, dc * CHUNK:(dc + 1) * CHUNK], op[:])
                    row = (c * 4 + st) * 128
                    eng = nc.sync if st % 2 == 0 else nc.scalar
                    eng.dma_start(out=out[row:row + 128, :], in_=ost[:])

            ats_prev = None
            for c in range(NCHUNK):
                cs = slice(c * CHUNK, (c + 1) * CHUNK)
                cos_t, sin_t = cs_cur

                # ---- QK projections + RoPE
                qts = []
                for head in range(HPC):
                    for kind in ("q", "k"):
                        w_sb = w_sbs[kind]
                        pp = psum.tile([128, CHUNK], FP32, name="pp",
                                       tag="main")
                        for dt in range(DT):
                            nc.tensor.matmul(
                                pp[:],
                                w_sb[:, dt, head * HD:(head + 1) * HD],
                                ht[:, dt, :],
                                start=(dt == 0), stop=(dt == DT - 1))
                        ps = psum.tile([128, CHUNK], FP32, name="ps",
                                       tag="aux")
                        nc.vector.tensor_tensor(
                            out=ps[:], in0=pp[:], in1=sin_t[:],
                            op=ALU.mult)
                        pc = ppc.tile([128, CHUNK], FP32, name="pc")
                        nc.vector.tensor_tensor(
                            out=pc[:], in0=pp[:], in1=cos_t[:],
                            op=ALU.mult)
                        if kind == "q":
                            dst = pqt.tile([128, CHUNK], BF16,
                                           name=f"qt{head}")
                            qts.append(dst)
                            d0, d1 = dst[0:64, :], dst[64:128, :]
                        else:
                            d0 = kt_sb[head][0:64, cs]
                            d1 = kt_sb[head][64:128, cs]
                        nc.vector.tensor_tensor(
                            out=d0, in0=pc[0:64, :], in1=ps[64:128, :],
                            op=ALU.subtract)
                        nc.vector.tensor_tensor(
                            out=d1, in0=pc[64:128, :], in1=ps[0:64, :],
                            op=ALU.add)

                # ---- V projection directly into natural [t, m] layout
                for g in range(2):
                    pv = psum.tile([128, CHUNK], FP32, name="pv", tag="aux")
                    for half in range(2):
                        st = g * 2 + half
                        for dt in range(DT):
                            nc.tensor.matmul(
                                pv[:, half * M:(half + 1) * M],
                                ht[:, dt, st * 128:(st + 1) * 128],
                                w_sbs["v"][:, dt, :],
                                start=(dt == 0), stop=(dt == DT - 1))
                    t0 = (c * 4 + g * 2) * M
                    nc.vector.tensor_copy(vn_sb[:, t0:t0 + 2 * M], pv[:])

                # prefetch next chunk's h^T + rope slices while computing
                if c + 1 < NCHUNK:
                    ht = load_ht(c + 1)
                    cs_cur = load_cs(c + 1)

                # ---- causal attention per head; the previous chunk's output
                # projection is emitted between the heads: its matmuls cover
                # head 0's recip+normalize latency and give the scalar engine
                # room to drain the evacuation copies before head 1's exps.
                ats = []
                for head in range(HPC):
                    if head == 1 and ats_prev is not None:
                        outproj(ats_prev, c - 1)
                    o_acc = psum.tile([128, CHUNK], FP32, name="o_acc",
                                      tag="o")
                    z_acc = psum.tile([128, CHUNK], FP32, name="z_acc",
                                      tag="z")
                    jmax = 4 * c + 3
                    pts = {}
                    for jj in range(jmax + 1 + LAG):
                        if jj <= jmax:
                            j = jj
                            r = j - 4 * c
                            off = max(r, 0) * 128
                            sp = psum.tile([128, CHUNK], FP32, name="sp",
                                           tag="main")
                            nc.tensor.matmul(
                                sp[:, off:],
                                kt_sb[head][:, j * 128:(j + 1) * 128],
                                qts[head][:, off:],
                                start=True, stop=True)
                            pt = ppt.tile([128, CHUNK], BF16, name="pt")
                            nc.scalar.activation(pt[:, off:], sp[:, off:],
                                                 AF.Exp,
                                                 scale=float(SM_SCALE))
                            if r >= 0:
                                nc.vector.tensor_tensor(
                                    out=pt[:, off:off + 128],
                                    in0=pt[:, off:off + 128],
                                    in1=tri_sb[:], op=ALU.mult)
                            pts[j] = (pt, off)
                        if jj >= LAG:
                            j = jj - LAG
                            pt, off = pts.pop(j)
                            nc.tensor.matmul(
                                z_acc[:, off:], ones_sb[:], pt[:, off:],
                                start=(j == 0), stop=(j == jmax),
                                skip_group_check=True)
                            nc.tensor.matmul(
                                o_acc[:, off:],
                                vn_sb[:, j * M + head * HD:
                                      j * M + (head + 1) * HD],
                                pt[:, off:],
                                start=(j == 0), stop=(j == jmax),
                                skip_group_check=True)
                    rz = prz.tile([128, CHUNK], FP32, name="rz")
                    nc.vector.reciprocal_approx_fast(out=rz[:], in_=z_acc[:])
                    at = pat.tile([128, CHUNK], BF16, name=f"at{head}")
                    nc.vector.tensor_tensor(out=at[:], in0=o_acc[:],
                                            in1=rz[:], op=ALU.mult)
                    ats.append(at)
                ats_prev = ats
            outproj(ats_prev, NCHUNK - 1, spread=True)
    nc.finalize()
    return nc


def _host_prep(xs, norm_w, wq, wk, wv, wo):
    """Fold RMSNorm into h^T upload + weights; build rope tables; bf16."""
    import ml_dtypes
    BF = ml_dtypes.bfloat16

    x64 = xs.astype(np.float64)
    istd = 1.0 / np.sqrt((x64 * x64).mean(axis=1) + EPS)    # [S]
    ht = (x64 * istd[:, None]).T.astype(BF)                 # [D, S]
    # partition-major pack: ht_pm[p, dt, s] = ht[dt*128+p, s]
    ht_pm = np.ascontiguousarray(
        ht.reshape(DT, 128, SEQ).transpose(1, 0, 2))

    def pack_w(w):  # [D, M] -> [128, DT*M] partition-major
        return np.ascontiguousarray(
            w.reshape(DT, 128, M).transpose(1, 0, 2).reshape(128, DT * M))

    nw = norm_w.astype(np.float32)[:, None, None]
    perm = np.concatenate([np.arange(0, HD, 2), np.arange(1, HD, 2)])
    wq_p = (wq * nw)[:, :, perm]
    wk_p = (wk * nw)[:, :, perm]

    inv_freq = 1.0 / (ROPE_BASE ** (np.arange(0, HD, 2, dtype=np.float32) / HD))
    pos = np.arange(SEQ, dtype=np.float32)
    ang = pos[:, None] * inv_freq[None, :]          # [S, 64]
    cos_t = np.cos(ang).T.astype(np.float32)        # [64, S]
    sin_t = np.sin(ang).T.astype(np.float32)
    cosd = np.ascontiguousarray(np.concatenate([cos_t, cos_t], 0))
    sind = np.ascontiguousarray(np.concatenate([sin_t, sin_t], 0))

    tri = np.triu(np.ones((128, 128), dtype=np.float32)).astype(BF)
    onesm = np.ones((128, 128), dtype=BF)

    common = {
        "ht": ht_pm,
        "cosd": cosd,
        "sind": sind,
        "tri": np.ascontiguousarray(tri),
        "ones": onesm,
    }
    in_maps = []
    for core in range(NCORES):
        sl = slice(core * HPC, (core + 1) * HPC)
        wot = np.transpose(wo[:, sl, :], (2, 1, 0)).reshape(128, HPC * D)
        in_maps.append({
            **common,
            "wq": pack_w(wq_p[:, sl, :].reshape(D, M).astype(BF)),
            "wk": pack_w(wk_p[:, sl, :].reshape(D, M).astype(BF)),
            "wv": pack_w((wv * nw)[:, sl, :].reshape(D, M).astype(BF)),
            "wot": np.ascontiguousarray(wot.astype(BF)),
        })
    return in_maps


def kernel(xs, norm_w, wq, wk, wv, wo):
    trace = bool(int(os.environ.get("KERNEL_TRACE", "0")))
    if trace:
        _inject_ntff_hook()
    from concourse.bass_utils import run_bass_kernel_spmd

    nc = _build_nc()
    in_maps = _host_prep(np.asarray(xs), np.asarray(norm_w), np.asarray(wq),
                         np.asarray(wk), np.asarray(wv), np.asarray(wo))
    try:
        res = run_bass_kernel_spmd(nc, in_maps, core_ids=list(range(NCORES)),
                                   trace=trace)
    except Exception:
        # transient device wedge (NRT_EXEC_UNIT_UNRECOVERABLE) recovers on
        # a fresh attempt; rebuild and retry once
        import time
        time.sleep(15)
        nc = _build_nc()
        res = run_bass_kernel_spmd(nc, in_maps, core_ids=list(range(NCORES)),
                                   trace=trace)
    if trace and res.exec_time_ns is not None:
        print(f"HW exec time: {res.exec_time_ns} ns")
    acc = np.zeros((SEQ, D), dtype=np.float64)
    for r in res.results:
        acc += r["out"].astype(np.float64)
    return acc.astype(np.float32)


if __name__ == "__main__":
    rng = np.random.default_rng(0)
    scale = 1.0 / np.sqrt(D)
    inputs = {
        "xs": rng.standard_normal((SEQ, D), dtype=np.float32),
        "norm_w": np.ones((D,), np.float32),
        "wq": rng.standard_normal((D, NH, HD), dtype=np.float32) * scale,
        "wk": rng.standard_normal((D, NH, HD), dtype=np.float32) * scale,
        "wv": rng.standard_normal((D, NH, HD), dtype=np.float32) * scale,
        "wo": rng.standard_normal((D, NH, HD), dtype=np.float32) * scale,
    }
    out = kernel(**inputs)
    print(out.shape, out.dtype, float(np.abs(out).max()))


# revision 32
# speedup vs baseline: 1.0336x; 1.0295x over previous
"""Trainium2 Bass kernel: RMSNorm + RoPE + causal attention + output projection.

Tensor-parallel over heads: 16 heads / 8 cores = 2 heads per core.
Each core computes a full [S, D] partial output (its heads' contribution to
the 'snh,dnh->sd' projection); the all-reduce is done host-side in the gather.

Fused streaming design (bf16, ~436 us vs 674 us baseline):
  - Host prep uploads the RMSNorm'd activations already transposed and
    partition-major packed (h^T as [128, DT, S] bf16) plus bf16 weights,
    a pre-transposed wo^T, and fp32 RoPE tables.  No PE transposes, no
    DRAM scratch roundtrip, and every DMA is a cheap contiguous pattern
    (strided DIRECT2D issue cost was multi-us per descriptor program).
  - Single pass over 8 q-chunks of 512: QK projections (+RoPE) append to
    per-head K^T in SBUF, V is projected directly into natural [t, hd]
    layout (ht-tile stationary), then causal attention for the chunk runs
    against all K/V tiles so far.  The previous chunk's output projection
    is emitted between the two heads' attention so its matmuls cover the
    softmax recip+normalize latency, and the next chunk's h^T/rope-slice
    DMAs prefetch under the current chunk's compute.
  - Scores are computed transposed (S^T[t, s]); softmax denominators via a
    ones-stationary matmul accumulating in PSUM; Z/PV lag scores/exp by 2
    (software pipeline) so a late exp never stalls the in-order PE queue;
    reciprocal_approx_fast instead of the 3.4us DVE reciprocal.
  - PSUM: one pool, four 2-bank tag rings: main (proj pp / scores sp),
    aux (rope ps / V accum / outproj op), o (PV accum), z (denominator).
    The final chunk's outproj spreads over main+aux (4 banks).
  - Startup: priority-ordered sync-queue DMA FIFO + 100 junk matmuls to
    hold the PE's HAM clock warm while the first loads land.
"""
import os
import sys
import types

import numpy as np

SEQ, D, NH, HD = 4096, 2048, 16, 128
NCORES = 8
HPC = NH // NCORES          # heads per core
M = HPC * HD                # per-core fused head dim (256)
EPS = 1e-6
ROPE_BASE = 10000.0
SM_SCALE = 1.0 / np.sqrt(HD)
CHUNK = 512                 # q-chunk
NCHUNK = SEQ // CHUNK       # 8
NT = SEQ // 128             # 32 s-tiles
DT = D // 128               # 16 d-tiles
LAG = 2


def _inject_ntff_hook():
    """Register the axon NTFF profiling hook (missing antenv.axon_hooks)."""
    if "antenv.axon_hooks" in sys.modules:
        return
    try:
        import antenv
        from trn_agent_boot.trn_boot import _ntff_profile_via_ctypes
    except ImportError:
        return
    holder = [None]
    mod = types.ModuleType("antenv.axon_hooks")
    mod.set_axon_ntff_profile_hook = lambda h: holder.__setitem__(0, h)
    mod.get_axon_ntff_profile_hook = lambda: holder[0]
    sys.modules["antenv.axon_hooks"] = mod
    antenv.axon_hooks = mod
    try:
        mod.set_axon_ntff_profile_hook(
            _ntff_profile_via_ctypes("/opt/axon/libaxon_pjrt.so"))
    except Exception:
        pass


def _build_nc():
    import concourse.bass as bass  # noqa: F401
    import concourse.mybir as mybir
    import concourse.tile as tile
    from concourse import bacc

    FP32 = mybir.dt.float32
    BF16 = mybir.dt.bfloat16
    AF = mybir.ActivationFunctionType
    ALU = mybir.AluOpType

    nc = bacc.Bacc(None, target_bir_lowering=False)

    ht_d = nc.declare_dram_parameter("ht", [128, DT, SEQ], BF16,
                                     isOutput=False)
    wq = nc.declare_dram_parameter("wq", [128, DT * M], BF16, isOutput=False)
    wk = nc.declare_dram_parameter("wk", [128, DT * M], BF16, isOutput=False)
    wv = nc.declare_dram_parameter("wv", [128, DT * M], BF16, isOutput=False)
    wot_d = nc.declare_dram_parameter("wot", [128, HPC * D], BF16,
                                      isOutput=False)
    cosd = nc.declare_dram_parameter("cosd", [128, SEQ], FP32, isOutput=False)
    sind = nc.declare_dram_parameter("sind", [128, SEQ], FP32, isOutput=False)
    tri = nc.declare_dram_parameter("tri", [128, 128], BF16, isOutput=False)
    ones = nc.declare_dram_parameter("ones", [128, 128], BF16, isOutput=False)
    out = nc.declare_dram_parameter("out", [SEQ, D], BF16, isOutput=True)

    with tile.TileContext(nc) as tc:
        with tc.tile_pool(name="consts", bufs=1) as consts, \
             tc.tile_pool(name="pht", bufs=2) as pht, \
             tc.tile_pool(name="pqt", bufs=4) as pqt, \
             tc.tile_pool(name="ppc", bufs=2) as ppc, \
             tc.tile_pool(name="ppt", bufs=6) as ppt, \
             tc.tile_pool(name="prz", bufs=2) as prz, \
             tc.tile_pool(name="pat", bufs=4) as pat, \
             tc.tile_pool(name="post", bufs=3) as post, \
             tc.tile_pool(name="pcs", bufs=2) as pcs, \
             tc.tile_pool(name="pzt", bufs=8) as pzt, \
             tc.tile_pool(name="psum", bufs=2, space="PSUM") as psum:
            kt_sb = [consts.tile([128, SEQ], BF16, name=f"kt{h}")
                     for h in range(HPC)]
            # V natural, packed per t-tile: vn[p, jt*M + m] = V[jt*128+p, m]
            vn_sb = consts.tile([128, NT * M], BF16)

            def load_ht(c):
                cs = slice(c * CHUNK, (c + 1) * CHUNK)
                ht = pht.tile([128, DT, CHUNK], BF16, name="ht")
                for dt8 in range(2):
                    nc.sync.dma_start(
                        out=ht[:, dt8 * 8:(dt8 + 1) * 8, :],
                        in_=ht_d[:, dt8 * 8:(dt8 + 1) * 8, cs])
                return ht

            def load_cs(c):
                cs = slice(c * CHUNK, (c + 1) * CHUNK)
                sin_t = pcs.tile([128, CHUNK], FP32, name="sin_t", tag="sin")
                nc.sync.dma_start(out=sin_t[:], in_=sind[:, cs])
                cos_t = pcs.tile([128, CHUNK], FP32, name="cos_t", tag="cos")
                nc.sync.dma_start(out=cos_t[:], in_=cosd[:, cs])
                return cos_t, sin_t

            # startup: one sync-queue FIFO in priority order — ring
            # bandwidth serves the first projections' pieces (ht, wq,
            # rope slices) before the bulk loads behind them.
            w_sbs = {k: consts.tile([128, DT, M], BF16, name=f"w{k}_sb")
                     for k in ("q", "k", "v")}
            cs0 = slice(0, CHUNK)
            ht = pht.tile([128, DT, CHUNK], BF16, name="ht")
            for q4 in range(4):
                dts = slice(q4 * 4, (q4 + 1) * 4)
                nc.sync.dma_start(out=ht[:, dts, :], in_=ht_d[:, dts, cs0])
                nc.sync.dma_start(
                    out=w_sbs["q"][:, dts, :].rearrange("p t m -> p (t m)"),
                    in_=wq[:, q4 * 4 * M:(q4 + 1) * 4 * M])
                if q4 == 1:
                    cs_cur = load_cs(0)
            nc.sync.dma_start(
                out=w_sbs["k"][:].rearrange("p t m -> p (t m)"), in_=wk[:])
            nc.sync.dma_start(
                out=w_sbs["v"][:].rearrange("p t m -> p (t m)"), in_=wv[:])
            tri_sb = consts.tile([128, 128], BF16)
            nc.sync.dma_start(out=tri_sb[:], in_=tri[:])
            ones_sb = consts.tile([128, 128], BF16)
            nc.sync.dma_start(out=ones_sb[:], in_=ones[:])
            wot_sb = consts.tile([128, HPC * D], BF16)
            nc.sync.dma_start(out=wot_sb[:], in_=wot_d[:])

            # HAM warm-up: keep the PE busy on junk matmuls while the
            # startup DMAs land, so real matmuls start at the warm clock.
            warm = consts.tile([128, 128], BF16)
            nc.vector.memset(warm[:], 0.0)
            for i in range(100):
                wp = psum.tile([128, CHUNK], FP32, name="wp", tag="main")
                nc.tensor.matmul(wp[:, 0:128], warm[:], warm[:],
                                 start=True, stop=True)

            def outproj(ats, c, spread=False):
                for st in range(4):
                    ost = post.tile([128, D], BF16, name="ost")
                    for dc in range(4):
                        # the final chunk's projection has nothing after it
                        # to hide evac latency: spread over two tag rings so
                        # 4 banks ping-pong instead of 2.
                        tag = "main" if spread and dc % 2 else "aux"
                        op = psum.tile([128, CHUNK], FP32, name="op",
                                       tag=tag)
                        nc.tensor.matmul(
                            op[:],
                            ats[0][:, st * 128:(st + 1) * 128],
                            wot_sb[:, dc * CHUNK:(dc + 1) * CHUNK],
                            start=True, stop=False)
                        nc.tensor.matmul(
                            op[:],
                            ats[1][:, st * 128:(st + 1) * 128],
                            wot_sb[:, D + dc * CHUNK:D + (dc + 1) * CHUNK],
                            start=False, stop=True)
                        if dc % 2 == 0:
                            nc.scalar.activation(
                                ost[:, dc * CHUNK:(dc + 1) * CHUNK],
                                op[:], AF.Copy)
                        else:
                            nc.vector.tensor_copy(
                                ost[:, dc * CHUNK:(dc + 1) * CHUNK], op[:])
                    row = (c * 4 + st) * 128
                    eng = nc.sync if st % 2 == 0 else nc.scalar
                    eng.dma_start(out=out[row:row + 128, :], in_=ost[:])

            ats_prev = None
            for c in range(NCHUNK):
                cs = slice(c * CHUNK, (c + 1) * CHUNK)
                cos_t, sin_t = cs_cur

                # ---- QK projections + RoPE
                qts = []
                for head in range(HPC):
                    for kind in ("q", "k"):
                        w_sb = w_sbs[kind]
                        pp = psum.tile([128, CHUNK], FP32, name="pp",
                                       tag="main")
                        for dt in range(DT):
                            nc.tensor.matmul(
                                pp[:],
                                w_sb[:, dt, head * HD:(head + 1) * HD],
                                ht[:, dt, :],
                                start=(dt == 0), stop=(dt == DT - 1))
                        ps = psum.tile([128, CHUNK], FP32, name="ps",
                                       tag="aux")
                        nc.vector.tensor_tensor(
                            out=ps[:], in0=pp[:], in1=sin_t[:],
                            op=ALU.mult)
                        pc = ppc.tile([128, CHUNK], FP32, name="pc")
                        nc.vector.tensor_tensor(
                            out=pc[:], in0=pp[:], in1=cos_t[:],
                            op=ALU.mult)
                        if kind == "q":
                            dst = pqt.tile([128, CHUNK], BF16,
                                           name=f"qt{head}")
                            qts.append(dst)
                            d0, d1 = dst[0:64, :], dst[64:128, :]
                        else:
                            d0 = kt_sb[head][0:64, cs]
                            d1 = kt_sb[head][64:128, cs]
                        nc.vector.tensor_tensor(
                            out=d0, in0=pc[0:64, :], in1=ps[64:128, :],
                            op=ALU.subtract)
                        nc.vector.tensor_tensor(
                            out=d1, in0=pc[64:128, :], in1=ps[0:64, :],
                            op=ALU.add)

                # ---- V projection directly into natural [t, m] layout
                for g in range(2):
                    pv = psum.tile([128, CHUNK], FP32, name="pv", tag="aux")
                    for half in range(2):
                        st = g * 2 + half
                        for dt in range(DT):
                            nc.tensor.matmul(
                                pv[:, half * M:(half + 1) * M],
                                ht[:, dt, st * 128:(st + 1) * 128],
                                w_sbs["v"][:, dt, :],
                                start=(dt == 0), stop=(dt == DT - 1))
                    t0 = (c * 4 + g * 2) * M
                    nc.vector.tensor_copy(vn_sb[:, t0:t0 + 2 * M], pv[:])

                # prefetch next chunk's h^T + rope slices while computing
                if c + 1 < NCHUNK:
                    ht = load_ht(c + 1)
                    cs_cur = load_cs(c + 1)

                # ---- causal attention per head; the previous chunk's output
                # projection is emitted between the heads: its matmuls cover
                # head 0's recip+normalize latency and give the scalar engine
                # room to drain the evacuation copies before head 1's exps.
                ats = []
                for head in range(HPC):
                    if head == 1 and ats_prev is not None:
                        outproj(ats_prev, c - 1)
                    o_acc = psum.tile([128, CHUNK], FP32, name="o_acc",
                                      tag="o")
                    z_acc = psum.tile([128, CHUNK], FP32, name="z_acc",
                                      tag="z")
                    jmax = 4 * c + 3
                    pts = {}
                    # softmax denominator: full t-tiles accumulate on the
                    # DVE as a bf16 pairwise tree (the PE paces attention,
                    # exp is the co-bottleneck; DVE has slack); only the 4
                    # diagonal tiles + one final tree reduction hit the PE.
                    zstack = {}

                    def tree_push(t):
                        lvl = 0
                        while lvl in zstack:
                            prev = zstack.pop(lvl)
                            nt = pzt.tile([128, CHUNK], BF16, name="zt")
                            nc.vector.tensor_tensor(out=nt[:], in0=prev[:],
                                                    in1=t[:], op=ALU.add)
                            t = nt
                            lvl += 1
                        zstack[lvl] = t

                    for jj in range(jmax + 1 + LAG):
                        if jj <= jmax:
                            j = jj
                            r = j - 4 * c
                            off = max(r, 0) * 128
                            sp = psum.tile([128, CHUNK], FP32, name="sp",
                                           tag="main")
                            nc.tensor.matmul(
                                sp[:, off:],
                                kt_sb[head][:, j * 128:(j + 1) * 128],
                                qts[head][:, off:],
                                start=True, stop=True)
                            pt = ppt.tile([128, CHUNK], BF16, name="pt")
                            nc.scalar.activation(pt[:, off:], sp[:, off:],
                                                 AF.Exp,
                                                 scale=float(SM_SCALE))
                            if r >= 0:
                                nc.vector.tensor_tensor(
                                    out=pt[:, off:off + 128],
                                    in0=pt[:, off:off + 128],
                                    in1=tri_sb[:], op=ALU.mult)
                            pts[j] = (pt, off)
                        if jj >= LAG:
                            j = jj - LAG
                            pt, off = pts.pop(j)
                            if j < 4 * c:
                                tree_push(pt)
                            else:
                                nc.tensor.matmul(
                                    z_acc[:, off:], ones_sb[:], pt[:, off:],
                                    start=(j == 4 * c),
                                    stop=(j == jmax and not zstack
                                          and c == 0),
                                    skip_group_check=True)
                            nc.tensor.matmul(
                                o_acc[:, off:],
                                vn_sb[:, j * M + head * HD:
                                      j * M + (head + 1) * HD],
                                pt[:, off:],
                                start=(j == 0), stop=(j == jmax),
                                skip_group_check=True)
                    if c > 0:
                        # collapse the tree and reduce over partitions with
                        # one ones-matmul accumulated into the z bank
                        parts = [zstack[l] for l in sorted(zstack)]
                        zt = parts[0]
                        for nxt in parts[1:]:
                            nt = pzt.tile([128, CHUNK], BF16, name="zt")
                            nc.vector.tensor_tensor(out=nt[:], in0=zt[:],
                                                    in1=nxt[:], op=ALU.add)
                            zt = nt
                        nc.tensor.matmul(
                            z_acc[:], ones_sb[:], zt[:],
                            start=False, stop=True, skip_group_check=True)
                    rz = prz.tile([128, CHUNK], FP32, name="rz")
                    nc.vector.reciprocal_approx_fast(out=rz[:], in_=z_acc[:])
                    at = pat.tile([128, CHUNK], BF16, name=f"at{head}")
                    nc.vector.tensor_tensor(out=at[:], in0=o_acc[:],
                                            in1=rz[:], op=ALU.mult)
                    ats.append(at)
                ats_prev = ats
            outproj(ats_prev, NCHUNK - 1, spread=True)
    nc.finalize()
    return nc


def _host_prep(xs, norm_w, wq, wk, wv, wo):
    """Fold RMSNorm into h^T upload + weights; build rope tables; bf16."""
    import ml_dtypes
    BF = ml_dtypes.bfloat16

    x64 = xs.astype(np.float64)
    istd = 1.0 / np.sqrt((x64 * x64).mean(axis=1) + EPS)    # [S]
    ht = (x64 * istd[:, None]).T.astype(BF)                 # [D, S]
    # partition-major pack: ht_pm[p, dt, s] = ht[dt*128+p, s]
    ht_pm = np.ascontiguousarray(
        ht.reshape(DT, 128, SEQ).transpose(1, 0, 2))

    def pack_w(w):  # [D, M] -> [128, DT*M] partition-major
        return np.ascontiguousarray(
            w.reshape(DT, 128, M).transpose(1, 0, 2).reshape(128, DT * M))

    nw = norm_w.astype(np.float32)[:, None, None]
    perm = np.concatenate([np.arange(0, HD, 2), np.arange(1, HD, 2)])
    wq_p = (wq * nw)[:, :, perm]
    wk_p = (wk * nw)[:, :, perm]

    inv_freq = 1.0 / (ROPE_BASE ** (np.arange(0, HD, 2, dtype=np.float32) / HD))
    pos = np.arange(SEQ, dtype=np.float32)
    ang = pos[:, None] * inv_freq[None, :]          # [S, 64]
    cos_t = np.cos(ang).T.astype(np.float32)        # [64, S]
    sin_t = np.sin(ang).T.astype(np.float32)
    cosd = np.ascontiguousarray(np.concatenate([cos_t, cos_t], 0))
    sind = np.ascontiguousarray(np.concatenate([sin_t, sin_t], 0))

    tri = np.triu(np.ones((128, 128), dtype=np.float32)).astype(BF)
    onesm = np.ones((128, 128), dtype=BF)

    common = {
        "ht": ht_pm,
        "cosd": cosd,
        "sind": sind,
        "tri": np.ascontiguousarray(tri),
        "ones": onesm,
    }
    in_maps = []
    for core in range(NCORES):
        sl = slice(core * HPC, (core + 1) * HPC)
        wot = np.transpose(wo[:, sl, :], (2, 1, 0)).reshape(128, HPC * D)
        in_maps.append({
            **common,
            "wq": pack_w(wq_p[:, sl, :].reshape(D, M).astype(BF)),
            "wk": pack_w(wk_p[:, sl, :].reshape(D, M).astype(BF)),
            "wv": pack_w((wv * nw)[:, sl, :].reshape(D, M).astype(BF)),
            "wot": np.ascontiguousarray(wot.astype(BF)),
        })
    return in_maps


def kernel(xs, norm_w, wq, wk, wv, wo):
    trace = bool(int(os.environ.get("KERNEL_TRACE", "0")))
    if trace:
        _inject_ntff_hook()
    from concourse.bass_utils import run_bass_kernel_spmd

    nc = _build_nc()
    in_maps = _host_prep(np.asarray(xs), np.asarray(norm_w), np.asarray(wq),
                         np.asarray(wk), np.asarray(wv), np.asarray(wo))
    try:
        res = run_bass_kernel_spmd(nc, in_maps, core_ids=list(range(NCORES)),
                                   trace=trace)
    except Exception:
        # transient device wedge (NRT_EXEC_UNIT_UNRECOVERABLE) recovers on
        # a fresh attempt; rebuild and retry once
        import time
        time.sleep(15)
        nc = _build_nc()
        res = run_bass_kernel_spmd(nc, in_maps, core_ids=list(range(NCORES)),
                                   trace=trace)
    if trace and res.exec_time_ns is not None:
        print(f"HW exec time: {res.exec_time_ns} ns")
    acc = np.zeros((SEQ, D), dtype=np.float64)
    for r in res.results:
        acc += r["out"].astype(np.float64)
    return acc.astype(np.float32)


if __name__ == "__main__":
    rng = np.random.default_rng(0)
    scale = 1.0 / np.sqrt(D)
    inputs = {
        "xs": rng.standard_normal((SEQ, D), dtype=np.float32),
        "norm_w": np.ones((D,), np.float32),
        "wq": rng.standard_normal((D, NH, HD), dtype=np.float32) * scale,
        "wk": rng.standard_normal((D, NH, HD), dtype=np.float32) * scale,
        "wv": rng.standard_normal((D, NH, HD), dtype=np.float32) * scale,
        "wo": rng.standard_normal((D, NH, HD), dtype=np.float32) * scale,
    }
    out = kernel(**inputs)
    print(out.shape, out.dtype, float(np.abs(out).max()))


# revision 34
# speedup vs baseline: 1.0763x; 1.0413x over previous
"""Trainium2 Bass kernel: RMSNorm + RoPE + causal attention + output projection.

Tensor-parallel over heads: 16 heads / 8 cores = 2 heads per core.
Each core computes a full [S, D] partial output (its heads' contribution to
the 'snh,dnh->sd' projection); the all-reduce is done host-side in the gather.

Fused streaming design (bf16, ~436 us vs 674 us baseline):
  - Host prep uploads the RMSNorm'd activations already transposed and
    partition-major packed (h^T as [128, DT, S] bf16) plus bf16 weights,
    a pre-transposed wo^T, and fp32 RoPE tables.  No PE transposes, no
    DRAM scratch roundtrip, and every DMA is a cheap contiguous pattern
    (strided DIRECT2D issue cost was multi-us per descriptor program).
  - Single pass over 8 q-chunks of 512: QK projections (+RoPE) append to
    per-head K^T in SBUF, V is projected directly into natural [t, hd]
    layout (ht-tile stationary), then causal attention for the chunk runs
    against all K/V tiles so far.  The previous chunk's output projection
    is emitted between the two heads' attention so its matmuls cover the
    softmax recip+normalize latency, and the next chunk's h^T/rope-slice
    DMAs prefetch under the current chunk's compute.
  - Scores are computed transposed (S^T[t, s]); softmax denominators via a
    ones-stationary matmul accumulating in PSUM; Z/PV lag scores/exp by 2
    (software pipeline) so a late exp never stalls the in-order PE queue;
    reciprocal_approx_fast instead of the 3.4us DVE reciprocal.
  - PSUM: one pool, four 2-bank tag rings: main (proj pp / scores sp),
    aux (rope ps / V accum / outproj op), o (PV accum), z (denominator).
    The final chunk's outproj spreads over main+aux (4 banks).
  - Startup: priority-ordered sync-queue DMA FIFO + 100 junk matmuls to
    hold the PE's HAM clock warm while the first loads land.
"""
import os
import sys
import types

import numpy as np

SEQ, D, NH, HD = 4096, 2048, 16, 128
NCORES = 8
HPC = NH // NCORES          # heads per core
M = HPC * HD                # per-core fused head dim (256)
EPS = 1e-6
ROPE_BASE = 10000.0
SM_SCALE = 1.0 / np.sqrt(HD)
CHUNK = 512                 # q-chunk
NCHUNK = SEQ // CHUNK       # 8
NT = SEQ // 128             # 32 s-tiles
DT = D // 128               # 16 d-tiles
LAG = 2


def _inject_ntff_hook():
    """Register the axon NTFF profiling hook (missing antenv.axon_hooks)."""
    if "antenv.axon_hooks" in sys.modules:
        return
    try:
        import antenv
        from trn_agent_boot.trn_boot import _ntff_profile_via_ctypes
    except ImportError:
        return
    holder = [None]
    mod = types.ModuleType("antenv.axon_hooks")
    mod.set_axon_ntff_profile_hook = lambda h: holder.__setitem__(0, h)
    mod.get_axon_ntff_profile_hook = lambda: holder[0]
    sys.modules["antenv.axon_hooks"] = mod
    antenv.axon_hooks = mod
    try:
        mod.set_axon_ntff_profile_hook(
            _ntff_profile_via_ctypes("/opt/axon/libaxon_pjrt.so"))
    except Exception:
        pass


def _build_nc():
    import concourse.bass as bass  # noqa: F401
    import concourse.mybir as mybir
    import concourse.tile as tile
    from concourse import bacc

    FP32 = mybir.dt.float32
    BF16 = mybir.dt.bfloat16
    AF = mybir.ActivationFunctionType
    ALU = mybir.AluOpType

    nc = bacc.Bacc(None, target_bir_lowering=False)

    ht_d = nc.declare_dram_parameter("ht", [128, DT, SEQ], BF16,
                                     isOutput=False)
    wq = nc.declare_dram_parameter("wq", [128, DT * M], BF16, isOutput=False)
    wk = nc.declare_dram_parameter("wk", [128, DT * M], BF16, isOutput=False)
    wv = nc.declare_dram_parameter("wv", [128, DT * M], BF16, isOutput=False)
    wot_d = nc.declare_dram_parameter("wot", [128, HPC * D], BF16,
                                      isOutput=False)
    cosd = nc.declare_dram_parameter("cosd", [128, SEQ], FP32, isOutput=False)
    sind = nc.declare_dram_parameter("sind", [128, SEQ], FP32, isOutput=False)
    tri = nc.declare_dram_parameter("tri", [128, 128], BF16, isOutput=False)
    ones = nc.declare_dram_parameter("ones", [128, 128], BF16, isOutput=False)
    out = nc.declare_dram_parameter("out", [SEQ, D], BF16, isOutput=True)

    with tile.TileContext(nc) as tc:
        with tc.tile_pool(name="consts", bufs=1) as consts, \
             tc.tile_pool(name="pht", bufs=2) as pht, \
             tc.tile_pool(name="pqt", bufs=4) as pqt, \
             tc.tile_pool(name="ppc", bufs=2) as ppc, \
             tc.tile_pool(name="ppt", bufs=6) as ppt, \
             tc.tile_pool(name="prz", bufs=2) as prz, \
             tc.tile_pool(name="pat", bufs=4) as pat, \
             tc.tile_pool(name="post", bufs=3) as post, \
             tc.tile_pool(name="pcs", bufs=2) as pcs, \
             tc.tile_pool(name="pzt", bufs=8) as pzt, \
             tc.tile_pool(name="psum", bufs=2, space="PSUM") as psum:
            kt_sb = [consts.tile([128, SEQ], BF16, name=f"kt{h}")
                     for h in range(HPC)]
            # V natural, packed per t-tile: vn[p, jt*M + m] = V[jt*128+p, m]
            vn_sb = consts.tile([128, NT * M], BF16)

            def load_ht(c):
                cs = slice(c * CHUNK, (c + 1) * CHUNK)
                ht = pht.tile([128, DT, CHUNK], BF16, name="ht")
                for dt8 in range(2):
                    nc.sync.dma_start(
                        out=ht[:, dt8 * 8:(dt8 + 1) * 8, :],
                        in_=ht_d[:, dt8 * 8:(dt8 + 1) * 8, cs])
                return ht

            def load_cs(c):
                cs = slice(c * CHUNK, (c + 1) * CHUNK)
                sin_t = pcs.tile([128, CHUNK], FP32, name="sin_t", tag="sin")
                nc.sync.dma_start(out=sin_t[:], in_=sind[:, cs])
                cos_t = pcs.tile([128, CHUNK], FP32, name="cos_t", tag="cos")
                nc.sync.dma_start(out=cos_t[:], in_=cosd[:, cs])
                return cos_t, sin_t

            # startup: one sync-queue FIFO in priority order — ring
            # bandwidth serves the first projections' pieces (ht, wq,
            # rope slices) before the bulk loads behind them.
            w_sbs = {k: consts.tile([128, DT, M], BF16, name=f"w{k}_sb")
                     for k in ("q", "k", "v")}
            cs0 = slice(0, CHUNK)
            ht = pht.tile([128, DT, CHUNK], BF16, name="ht")
            for q4 in range(4):
                dts = slice(q4 * 4, (q4 + 1) * 4)
                nc.sync.dma_start(out=ht[:, dts, :], in_=ht_d[:, dts, cs0])
                nc.sync.dma_start(
                    out=w_sbs["q"][:, dts, :].rearrange("p t m -> p (t m)"),
                    in_=wq[:, q4 * 4 * M:(q4 + 1) * 4 * M])
                if q4 == 1:
                    cs_cur = load_cs(0)
            nc.sync.dma_start(
                out=w_sbs["k"][:].rearrange("p t m -> p (t m)"), in_=wk[:])
            nc.sync.dma_start(
                out=w_sbs["v"][:].rearrange("p t m -> p (t m)"), in_=wv[:])
            tri_sb = consts.tile([128, 128], BF16)
            nc.sync.dma_start(out=tri_sb[:], in_=tri[:])
            ones_sb = consts.tile([128, 128], BF16)
            nc.sync.dma_start(out=ones_sb[:], in_=ones[:])
            wot_sb = consts.tile([128, HPC * D], BF16)
            nc.sync.dma_start(out=wot_sb[:], in_=wot_d[:])

            # HAM warm-up: keep the PE busy on junk matmuls while the
            # startup DMAs land, so real matmuls start at the warm clock.
            warm = consts.tile([128, 128], BF16)
            nc.vector.memset(warm[:], 0.0)
            for i in range(100):
                wp = psum.tile([128, CHUNK], FP32, name="wp", tag="main")
                nc.tensor.matmul(wp[:, 0:128], warm[:], warm[:],
                                 start=True, stop=True)

            def outproj(ats, c, spread=False):
                for st in range(4):
                    ost = post.tile([128, D], BF16, name="ost")
                    for dc in range(4):
                        # the final chunk's projection has nothing after it
                        # to hide evac latency: spread over two tag rings so
                        # 4 banks ping-pong instead of 2.
                        tag = "main" if spread and dc % 2 else "aux"
                        op = psum.tile([128, CHUNK], FP32, name="op",
                                       tag=tag)
                        nc.tensor.matmul(
                            op[:],
                            ats[0][:, st * 128:(st + 1) * 128],
                            wot_sb[:, dc * CHUNK:(dc + 1) * CHUNK],
                            start=True, stop=False)
                        nc.tensor.matmul(
                            op[:],
                            ats[1][:, st * 128:(st + 1) * 128],
                            wot_sb[:, D + dc * CHUNK:D + (dc + 1) * CHUNK],
                            start=False, stop=True)
                        if dc % 2 == 0:
                            nc.scalar.activation(
                                ost[:, dc * CHUNK:(dc + 1) * CHUNK],
                                op[:], AF.Copy)
                        else:
                            nc.vector.tensor_copy(
                                ost[:, dc * CHUNK:(dc + 1) * CHUNK], op[:])
                    row = (c * 4 + st) * 128
                    eng = nc.sync if st % 2 == 0 else nc.scalar
                    eng.dma_start(out=out[row:row + 128, :], in_=ost[:])

            ats_prev = None
            for c in range(NCHUNK):
                cs = slice(c * CHUNK, (c + 1) * CHUNK)
                cos_t, sin_t = cs_cur

                # ---- QK projections + RoPE
                qts = []
                for head in range(HPC):
                    for kind in ("q", "k"):
                        w_sb = w_sbs[kind]
                        pp = psum.tile([128, CHUNK], FP32, name="pp",
                                       tag="main")
                        for dt in range(DT):
                            nc.tensor.matmul(
                                pp[:],
                                w_sb[:, dt, head * HD:(head + 1) * HD],
                                ht[:, dt, :],
                                start=(dt == 0), stop=(dt == DT - 1))
                        ps = psum.tile([128, CHUNK], FP32, name="ps",
                                       tag="aux")
                        nc.vector.tensor_tensor(
                            out=ps[:], in0=pp[:], in1=sin_t[:],
                            op=ALU.mult)
                        pc = ppc.tile([128, CHUNK], FP32, name="pc")
                        nc.vector.tensor_tensor(
                            out=pc[:], in0=pp[:], in1=cos_t[:],
                            op=ALU.mult)
                        if kind == "q":
                            dst = pqt.tile([128, CHUNK], BF16,
                                           name=f"qt{head}")
                            qts.append(dst)
                            d0, d1 = dst[0:64, :], dst[64:128, :]
                        else:
                            d0 = kt_sb[head][0:64, cs]
                            d1 = kt_sb[head][64:128, cs]
                        nc.vector.tensor_tensor(
                            out=d0, in0=pc[0:64, :], in1=ps[64:128, :],
                            op=ALU.subtract)
                        nc.vector.tensor_tensor(
                            out=d1, in0=pc[64:128, :], in1=ps[0:64, :],
                            op=ALU.add)

                # ---- V projection directly into natural [t, m] layout
                for g in range(2):
                    pv = psum.tile([128, CHUNK], FP32, name="pv", tag="aux")
                    for half in range(2):
                        st = g * 2 + half
                        for dt in range(DT):
                            nc.tensor.matmul(
                                pv[:, half * M:(half + 1) * M],
                                ht[:, dt, st * 128:(st + 1) * 128],
                                w_sbs["v"][:, dt, :],
                                start=(dt == 0), stop=(dt == DT - 1))
                    t0 = (c * 4 + g * 2) * M
                    nc.vector.tensor_copy(vn_sb[:, t0:t0 + 2 * M], pv[:])

                # prefetch next chunk's h^T + rope slices while computing
                if c + 1 < NCHUNK:
                    ht = load_ht(c + 1)
                    cs_cur = load_cs(c + 1)

                # ---- causal attention per head; the previous chunk's output
                # projection is emitted between the heads: its matmuls cover
                # head 0's recip+normalize latency and give the scalar engine
                # room to drain the evacuation copies before head 1's exps.
                ats = []
                for head in range(HPC):
                    if head == 1 and ats_prev is not None:
                        outproj(ats_prev, c - 1)
                    o_acc = psum.tile([128, CHUNK], FP32, name="o_acc",
                                      tag="o", bufs=1)
                    z_acc = psum.tile([128, CHUNK], FP32, name="z_acc",
                                      tag="z", bufs=1)
                    jmax = 4 * c + 3
                    # Attention inner loop at PAIR granularity: two j-tiles'
                    # scores land in one 2-bank [128, 1024] PSUM tile and a
                    # single wide exp covers both (halves ACT's per-
                    # instruction overhead — exp paces attention now).
                    # Softmax denominator: full pairs fold on the DVE as a
                    # bf16 pairwise tree; only the 4 diagonal tiles + one
                    # final tree reduction hit the PE.
                    zstack = {}

                    def tree_push(t, lvl):
                        while lvl in zstack:
                            prev = zstack.pop(lvl)
                            nt = pzt.tile([128, CHUNK], BF16, name="zt")
                            nc.vector.tensor_tensor(out=nt[:], in0=prev[:],
                                                    in1=t[:], op=ALU.add)
                            t = nt
                            lvl += 1
                        zstack[lvl] = t

                    npair = (jmax + 1) // 2
                    PLAG = 2
                    pairs = {}
                    for pi in range(npair + PLAG):
                        if pi < npair:
                            spp = psum.tile([128, 2 * CHUNK], FP32,
                                            name="spp", tag="main")
                            ptp = ppt.tile([128, 2 * CHUNK], BF16,
                                           name="ptp")
                            for h2 in range(2):
                                j = 2 * pi + h2
                                off = h2 * CHUNK + max(j - 4 * c, 0) * 128
                                nc.tensor.matmul(
                                    spp[:, off:(h2 + 1) * CHUNK],
                                    kt_sb[head][:, j * 128:(j + 1) * 128],
                                    qts[head][:, off - h2 * CHUNK:],
                                    start=True, stop=True)
                            nc.scalar.activation(ptp[:], spp[:], AF.Exp,
                                                 scale=float(SM_SCALE))
                            for h2 in range(2):
                                j = 2 * pi + h2
                                r = j - 4 * c
                                if r >= 0:
                                    off = h2 * CHUNK + r * 128
                                    nc.vector.tensor_tensor(
                                        out=ptp[:, off:off + 128],
                                        in0=ptp[:, off:off + 128],
                                        in1=tri_sb[:], op=ALU.mult)
                            pairs[pi] = ptp
                        if pi >= PLAG:
                            i = pi - PLAG
                            ptp = pairs.pop(i)
                            for h2 in range(2):
                                j = 2 * i + h2
                                off = h2 * CHUNK + max(j - 4 * c, 0) * 128
                                nc.tensor.matmul(
                                    o_acc[:, off - h2 * CHUNK:],
                                    vn_sb[:, j * M + head * HD:
                                          j * M + (head + 1) * HD],
                                    ptp[:, off:(h2 + 1) * CHUNK],
                                    start=(j == 0), stop=(j == jmax),
                                    skip_group_check=True)
                            if 2 * i + 1 < 4 * c:
                                # full pair: fold its two halves, push at
                                # level 1 (each tree node covers 2 tiles)
                                nt = pzt.tile([128, CHUNK], BF16, name="zt")
                                nc.vector.tensor_tensor(
                                    out=nt[:], in0=ptp[:, 0:CHUNK],
                                    in1=ptp[:, CHUNK:], op=ALU.add)
                                tree_push(nt, 1)
                            else:
                                for h2 in range(2):
                                    j = 2 * i + h2
                                    off = h2 * CHUNK + \
                                        max(j - 4 * c, 0) * 128
                                    nc.tensor.matmul(
                                        z_acc[:, off - h2 * CHUNK:],
                                        ones_sb[:],
                                        ptp[:, off:(h2 + 1) * CHUNK],
                                        start=(j == 4 * c),
                                        stop=(j == jmax and c == 0),
                                        skip_group_check=True)
                    if c > 0:
                        # collapse the tree and reduce over partitions with
                        # one ones-matmul accumulated into the z bank
                        parts = [zstack[l] for l in sorted(zstack)]
                        zt = parts[0]
                        for nxt in parts[1:]:
                            nt = pzt.tile([128, CHUNK], BF16, name="zt")
                            nc.vector.tensor_tensor(out=nt[:], in0=zt[:],
                                                    in1=nxt[:], op=ALU.add)
                            zt = nt
                        nc.tensor.matmul(
                            z_acc[:], ones_sb[:], zt[:],
                            start=False, stop=True, skip_group_check=True)
                    rz = prz.tile([128, CHUNK], FP32, name="rz")
                    nc.vector.reciprocal_approx_fast(out=rz[:], in_=z_acc[:])
                    at = pat.tile([128, CHUNK], BF16, name=f"at{head}")
                    nc.vector.tensor_tensor(out=at[:], in0=o_acc[:],
                                            in1=rz[:], op=ALU.mult)
                    ats.append(at)
                ats_prev = ats
            outproj(ats_prev, NCHUNK - 1, spread=True)
    nc.finalize()
    return nc


def _host_prep(xs, norm_w, wq, wk, wv, wo):
    """Fold RMSNorm into h^T upload + weights; build rope tables; bf16."""
    import ml_dtypes
    BF = ml_dtypes.bfloat16

    x64 = xs.astype(np.float64)
    istd = 1.0 / np.sqrt((x64 * x64).mean(axis=1) + EPS)    # [S]
    ht = (x64 * istd[:, None]).T.astype(BF)                 # [D, S]
    # partition-major pack: ht_pm[p, dt, s] = ht[dt*128+p, s]
    ht_pm = np.ascontiguousarray(
        ht.reshape(DT, 128, SEQ).transpose(1, 0, 2))

    def pack_w(w):  # [D, M] -> [128, DT*M] partition-major
        return np.ascontiguousarray(
            w.reshape(DT, 128, M).transpose(1, 0, 2).reshape(128, DT * M))

    nw = norm_w.astype(np.float32)[:, None, None]
    perm = np.concatenate([np.arange(0, HD, 2), np.arange(1, HD, 2)])
    wq_p = (wq * nw)[:, :, perm]
    wk_p = (wk * nw)[:, :, perm]

    inv_freq = 1.0 / (ROPE_BASE ** (np.arange(0, HD, 2, dtype=np.float32) / HD))
    pos = np.arange(SEQ, dtype=np.float32)
    ang = pos[:, None] * inv_freq[None, :]          # [S, 64]
    cos_t = np.cos(ang).T.astype(np.float32)        # [64, S]
    sin_t = np.sin(ang).T.astype(np.float32)
    cosd = np.ascontiguousarray(np.concatenate([cos_t, cos_t], 0))
    sind = np.ascontiguousarray(np.concatenate([sin_t, sin_t], 0))

    tri = np.triu(np.ones((128, 128), dtype=np.float32)).astype(BF)
    onesm = np.ones((128, 128), dtype=BF)

    common = {
        "ht": ht_pm,
        "cosd": cosd,
        "sind": sind,
        "tri": np.ascontiguousarray(tri),
        "ones": onesm,
    }
    in_maps = []
    for core in range(NCORES):
        sl = slice(core * HPC, (core + 1) * HPC)
        wot = np.transpose(wo[:, sl, :], (2, 1, 0)).reshape(128, HPC * D)
        in_maps.append({
            **common,
            "wq": pack_w(wq_p[:, sl, :].reshape(D, M).astype(BF)),
            "wk": pack_w(wk_p[:, sl, :].reshape(D, M).astype(BF)),
            "wv": pack_w((wv * nw)[:, sl, :].reshape(D, M).astype(BF)),
            "wot": np.ascontiguousarray(wot.astype(BF)),
        })
    return in_maps


def kernel(xs, norm_w, wq, wk, wv, wo):
    trace = bool(int(os.environ.get("KERNEL_TRACE", "0")))
    if trace:
        _inject_ntff_hook()
    from concourse.bass_utils import run_bass_kernel_spmd

    nc = _build_nc()
    in_maps = _host_prep(np.asarray(xs), np.asarray(norm_w), np.asarray(wq),
                         np.asarray(wk), np.asarray(wv), np.asarray(wo))
    try:
        res = run_bass_kernel_spmd(nc, in_maps, core_ids=list(range(NCORES)),
                                   trace=trace)
    except Exception:
        # transient device wedge (NRT_EXEC_UNIT_UNRECOVERABLE) recovers on
        # a fresh attempt; rebuild and retry once
        import time
        time.sleep(15)
        nc = _build_nc()
        res = run_bass_kernel_spmd(nc, in_maps, core_ids=list(range(NCORES)),
                                   trace=trace)
    if trace and res.exec_time_ns is not None:
        print(f"HW exec time: {res.exec_time_ns} ns")
    acc = np.zeros((SEQ, D), dtype=np.float64)
    for r in res.results:
        acc += r["out"].astype(np.float64)
    return acc.astype(np.float32)


if __name__ == "__main__":
    rng = np.random.default_rng(0)
    scale = 1.0 / np.sqrt(D)
    inputs = {
        "xs": rng.standard_normal((SEQ, D), dtype=np.float32),
        "norm_w": np.ones((D,), np.float32),
        "wq": rng.standard_normal((D, NH, HD), dtype=np.float32) * scale,
        "wk": rng.standard_normal((D, NH, HD), dtype=np.float32) * scale,
        "wv": rng.standard_normal((D, NH, HD), dtype=np.float32) * scale,
        "wo": rng.standard_normal((D, NH, HD), dtype=np.float32) * scale,
    }
    out = kernel(**inputs)
    print(out.shape, out.dtype, float(np.abs(out).max()))


# revision 35
# speedup vs baseline: 1.0777x; 1.0013x over previous
"""Trainium2 Bass kernel: RMSNorm + RoPE + causal attention + output projection.

Tensor-parallel over heads: 16 heads / 8 cores = 2 heads per core.
Each core computes a full [S, D] partial output (its heads' contribution to
the 'snh,dnh->sd' projection); the all-reduce is done host-side in the gather.

Fused streaming design (bf16, ~436 us vs 674 us baseline):
  - Host prep uploads the RMSNorm'd activations already transposed and
    partition-major packed (h^T as [128, DT, S] bf16) plus bf16 weights,
    a pre-transposed wo^T, and fp32 RoPE tables.  No PE transposes, no
    DRAM scratch roundtrip, and every DMA is a cheap contiguous pattern
    (strided DIRECT2D issue cost was multi-us per descriptor program).
  - Single pass over 8 q-chunks of 512: QK projections (+RoPE) append to
    per-head K^T in SBUF, V is projected directly into natural [t, hd]
    layout (ht-tile stationary), then causal attention for the chunk runs
    against all K/V tiles so far.  The previous chunk's output projection
    is emitted between the two heads' attention so its matmuls cover the
    softmax recip+normalize latency, and the next chunk's h^T/rope-slice
    DMAs prefetch under the current chunk's compute.
  - Scores are computed transposed (S^T[t, s]) at PAIR granularity: two
    j-tiles' scores land in one 2-bank [128, 1024] PSUM tile and a single
    wide exp covers both, halving ACT per-instruction overhead (exp is
    the attention pacer at ~507 ns/j; PE runs ~430 ns/j underneath).
    Softmax denominators: full pairs fold on the DVE as a bf16 pairwise
    tree (~436 ns/add, inside the exp shadow); only the 4 diagonal tiles
    + one final tree reduction hit the PE as ones-matmuls.  PV/Z lag
    scores/exp by 2 pairs so a late exp never stalls the in-order PE
    queue; reciprocal_approx_fast instead of the 3.4us DVE reciprocal.
  - PSUM (16 KiB fully allocated): main 2x2-bank slots (proj pp / score
    pairs / V accum), aux 2 banks (rope ps / outproj op), o 1 bank (PV
    accum), z 1 bank (denominator).  The final chunk's outproj spreads
    over main+aux.
  - Startup: priority-ordered sync-queue DMA FIFO + 100 junk matmuls to
    hold the PE's HAM clock warm while the first loads land.
"""
import os
import sys
import types

import numpy as np

SEQ, D, NH, HD = 4096, 2048, 16, 128
NCORES = 8
HPC = NH // NCORES          # heads per core
M = HPC * HD                # per-core fused head dim (256)
EPS = 1e-6
ROPE_BASE = 10000.0
SM_SCALE = 1.0 / np.sqrt(HD)
CHUNK = 512                 # q-chunk
NCHUNK = SEQ // CHUNK       # 8
NT = SEQ // 128             # 32 s-tiles
DT = D // 128               # 16 d-tiles
LAG = 2


def _inject_ntff_hook():
    """Register the axon NTFF profiling hook (missing antenv.axon_hooks)."""
    if "antenv.axon_hooks" in sys.modules:
        return
    try:
        import antenv
        from trn_agent_boot.trn_boot import _ntff_profile_via_ctypes
    except ImportError:
        return
    holder = [None]
    mod = types.ModuleType("antenv.axon_hooks")
    mod.set_axon_ntff_profile_hook = lambda h: holder.__setitem__(0, h)
    mod.get_axon_ntff_profile_hook = lambda: holder[0]
    sys.modules["antenv.axon_hooks"] = mod
    antenv.axon_hooks = mod
    try:
        mod.set_axon_ntff_profile_hook(
            _ntff_profile_via_ctypes("/opt/axon/libaxon_pjrt.so"))
    except Exception:
        pass


def _build_nc():
    import concourse.bass as bass  # noqa: F401
    import concourse.mybir as mybir
    import concourse.tile as tile
    from concourse import bacc

    FP32 = mybir.dt.float32
    BF16 = mybir.dt.bfloat16
    AF = mybir.ActivationFunctionType
    ALU = mybir.AluOpType

    nc = bacc.Bacc(None, target_bir_lowering=False)

    ht_d = nc.declare_dram_parameter("ht", [128, DT, SEQ], BF16,
                                     isOutput=False)
    wq = nc.declare_dram_parameter("wq", [128, DT * M], BF16, isOutput=False)
    wk = nc.declare_dram_parameter("wk", [128, DT * M], BF16, isOutput=False)
    wv = nc.declare_dram_parameter("wv", [128, DT * M], BF16, isOutput=False)
    wot_d = nc.declare_dram_parameter("wot", [128, HPC * D], BF16,
                                      isOutput=False)
    cosd = nc.declare_dram_parameter("cosd", [128, SEQ], FP32, isOutput=False)
    sind = nc.declare_dram_parameter("sind", [128, SEQ], FP32, isOutput=False)
    tri = nc.declare_dram_parameter("tri", [128, 128], BF16, isOutput=False)
    ones = nc.declare_dram_parameter("ones", [128, 128], BF16, isOutput=False)
    out = nc.declare_dram_parameter("out", [SEQ, D], BF16, isOutput=True)

    with tile.TileContext(nc) as tc:
        with tc.tile_pool(name="consts", bufs=1) as consts, \
             tc.tile_pool(name="pht", bufs=2) as pht, \
             tc.tile_pool(name="pqt", bufs=4) as pqt, \
             tc.tile_pool(name="ppc", bufs=2) as ppc, \
             tc.tile_pool(name="ppt", bufs=6) as ppt, \
             tc.tile_pool(name="prz", bufs=2) as prz, \
             tc.tile_pool(name="pat", bufs=4) as pat, \
             tc.tile_pool(name="post", bufs=3) as post, \
             tc.tile_pool(name="pcs", bufs=2) as pcs, \
             tc.tile_pool(name="pzt", bufs=8) as pzt, \
             tc.tile_pool(name="psum", bufs=2, space="PSUM") as psum:
            kt_sb = [consts.tile([128, SEQ], BF16, name=f"kt{h}")
                     for h in range(HPC)]
            # V natural, packed per t-tile: vn[p, jt*M + m] = V[jt*128+p, m]
            vn_sb = consts.tile([128, NT * M], BF16)

            def load_ht(c):
                cs = slice(c * CHUNK, (c + 1) * CHUNK)
                ht = pht.tile([128, DT, CHUNK], BF16, name="ht")
                for dt8 in range(2):
                    nc.sync.dma_start(
                        out=ht[:, dt8 * 8:(dt8 + 1) * 8, :],
                        in_=ht_d[:, dt8 * 8:(dt8 + 1) * 8, cs])
                return ht

            def load_cs(c):
                cs = slice(c * CHUNK, (c + 1) * CHUNK)
                sin_t = pcs.tile([128, CHUNK], FP32, name="sin_t", tag="sin")
                nc.sync.dma_start(out=sin_t[:], in_=sind[:, cs])
                cos_t = pcs.tile([128, CHUNK], FP32, name="cos_t", tag="cos")
                nc.sync.dma_start(out=cos_t[:], in_=cosd[:, cs])
                return cos_t, sin_t

            # startup: one sync-queue FIFO in priority order — ring
            # bandwidth serves the first projections' pieces (ht, wq,
            # rope slices) before the bulk loads behind them.
            w_sbs = {k: consts.tile([128, DT, M], BF16, name=f"w{k}_sb")
                     for k in ("q", "k", "v")}
            cs0 = slice(0, CHUNK)
            ht = pht.tile([128, DT, CHUNK], BF16, name="ht")
            for q4 in range(4):
                dts = slice(q4 * 4, (q4 + 1) * 4)
                nc.sync.dma_start(out=ht[:, dts, :], in_=ht_d[:, dts, cs0])
                nc.sync.dma_start(
                    out=w_sbs["q"][:, dts, :].rearrange("p t m -> p (t m)"),
                    in_=wq[:, q4 * 4 * M:(q4 + 1) * 4 * M])
                if q4 == 1:
                    cs_cur = load_cs(0)
            nc.sync.dma_start(
                out=w_sbs["k"][:].rearrange("p t m -> p (t m)"), in_=wk[:])
            nc.sync.dma_start(
                out=w_sbs["v"][:].rearrange("p t m -> p (t m)"), in_=wv[:])
            tri_sb = consts.tile([128, 128], BF16)
            nc.sync.dma_start(out=tri_sb[:], in_=tri[:])
            ones_sb = consts.tile([128, 128], BF16)
            nc.sync.dma_start(out=ones_sb[:], in_=ones[:])
            wot_sb = consts.tile([128, HPC * D], BF16)
            nc.sync.dma_start(out=wot_sb[:], in_=wot_d[:])

            # HAM warm-up: keep the PE busy on junk matmuls while the
            # startup DMAs land, so real matmuls start at the warm clock.
            warm = consts.tile([128, 128], BF16)
            nc.vector.memset(warm[:], 0.0)
            for i in range(100):
                wp = psum.tile([128, CHUNK], FP32, name="wp", tag="main")
                nc.tensor.matmul(wp[:, 0:128], warm[:], warm[:],
                                 start=True, stop=True)

            def outproj(ats, c, spread=False):
                for st in range(4):
                    ost = post.tile([128, D], BF16, name="ost")
                    for dc in range(4):
                        # the final chunk's projection has nothing after it
                        # to hide evac latency: spread over two tag rings so
                        # 4 banks ping-pong instead of 2.
                        tag = "main" if spread and dc % 2 else "aux"
                        op = psum.tile([128, CHUNK], FP32, name="op",
                                       tag=tag)
                        nc.tensor.matmul(
                            op[:],
                            ats[0][:, st * 128:(st + 1) * 128],
                            wot_sb[:, dc * CHUNK:(dc + 1) * CHUNK],
                            start=True, stop=False)
                        nc.tensor.matmul(
                            op[:],
                            ats[1][:, st * 128:(st + 1) * 128],
                            wot_sb[:, D + dc * CHUNK:D + (dc + 1) * CHUNK],
                            start=False, stop=True)
                        if dc % 2 == 0:
                            nc.scalar.activation(
                                ost[:, dc * CHUNK:(dc + 1) * CHUNK],
                                op[:], AF.Copy)
                        else:
                            nc.vector.tensor_copy(
                                ost[:, dc * CHUNK:(dc + 1) * CHUNK], op[:])
                    row = (c * 4 + st) * 128
                    eng = nc.sync if st % 2 == 0 else nc.scalar
                    eng.dma_start(out=out[row:row + 128, :], in_=ost[:])

            ats_prev = None
            for c in range(NCHUNK):
                cs = slice(c * CHUNK, (c + 1) * CHUNK)
                cos_t, sin_t = cs_cur

                # ---- QK projections + RoPE
                qts = []
                for head in range(HPC):
                    for kind in ("q", "k"):
                        w_sb = w_sbs[kind]
                        pp = psum.tile([128, CHUNK], FP32, name="pp",
                                       tag="main")
                        for dt in range(DT):
                            nc.tensor.matmul(
                                pp[:],
                                w_sb[:, dt, head * HD:(head + 1) * HD],
                                ht[:, dt, :],
                                start=(dt == 0), stop=(dt == DT - 1))
                        ps = psum.tile([128, CHUNK], FP32, name="ps",
                                       tag="aux")
                        nc.vector.tensor_tensor(
                            out=ps[:], in0=pp[:], in1=sin_t[:],
                            op=ALU.mult)
                        pc = ppc.tile([128, CHUNK], FP32, name="pc")
                        nc.vector.tensor_tensor(
                            out=pc[:], in0=pp[:], in1=cos_t[:],
                            op=ALU.mult)
                        if kind == "q":
                            dst = pqt.tile([128, CHUNK], BF16,
                                           name=f"qt{head}")
                            qts.append(dst)
                            d0, d1 = dst[0:64, :], dst[64:128, :]
                        else:
                            d0 = kt_sb[head][0:64, cs]
                            d1 = kt_sb[head][64:128, cs]
                        nc.vector.tensor_tensor(
                            out=d0, in0=pc[0:64, :], in1=ps[64:128, :],
                            op=ALU.subtract)
                        nc.vector.tensor_tensor(
                            out=d1, in0=pc[64:128, :], in1=ps[0:64, :],
                            op=ALU.add)

                # ---- V projection directly into natural [t, m] layout
                for g in range(2):
                    pv = psum.tile([128, CHUNK], FP32, name="pv", tag="aux")
                    for half in range(2):
                        st = g * 2 + half
                        for dt in range(DT):
                            nc.tensor.matmul(
                                pv[:, half * M:(half + 1) * M],
                                ht[:, dt, st * 128:(st + 1) * 128],
                                w_sbs["v"][:, dt, :],
                                start=(dt == 0), stop=(dt == DT - 1))
                    t0 = (c * 4 + g * 2) * M
                    nc.vector.tensor_copy(vn_sb[:, t0:t0 + 2 * M], pv[:])

                # prefetch next chunk's h^T + rope slices while computing
                if c + 1 < NCHUNK:
                    ht = load_ht(c + 1)
                    cs_cur = load_cs(c + 1)

                # ---- causal attention per head; the previous chunk's output
                # projection is emitted between the heads: its matmuls cover
                # head 0's recip+normalize latency and give the scalar engine
                # room to drain the evacuation copies before head 1's exps.
                ats = []
                for head in range(HPC):
                    if head == 1 and ats_prev is not None:
                        outproj(ats_prev, c - 1)
                    o_acc = psum.tile([128, CHUNK], FP32, name="o_acc",
                                      tag="o", bufs=1)
                    z_acc = psum.tile([128, CHUNK], FP32, name="z_acc",
                                      tag="z", bufs=1)
                    jmax = 4 * c + 3
                    # Attention inner loop at PAIR granularity: two j-tiles'
                    # scores land in one 2-bank [128, 1024] PSUM tile and a
                    # single wide exp covers both (halves ACT's per-
                    # instruction overhead — exp paces attention now).
                    # Softmax denominator: full pairs fold on the DVE as a
                    # bf16 pairwise tree; only the 4 diagonal tiles + one
                    # final tree reduction hit the PE.
                    zstack = {}

                    def tree_push(t, lvl):
                        while lvl in zstack:
                            prev = zstack.pop(lvl)
                            nt = pzt.tile([128, CHUNK], BF16, name="zt")
                            nc.vector.tensor_tensor(out=nt[:], in0=prev[:],
                                                    in1=t[:], op=ALU.add)
                            t = nt
                            lvl += 1
                        zstack[lvl] = t

                    npair = (jmax + 1) // 2
                    PLAG = 2
                    pairs = {}
                    for pi in range(npair + PLAG):
                        if pi < npair:
                            spp = psum.tile([128, 2 * CHUNK], FP32,
                                            name="spp", tag="main")
                            ptp = ppt.tile([128, 2 * CHUNK], BF16,
                                           name="ptp")
                            for h2 in range(2):
                                j = 2 * pi + h2
                                off = h2 * CHUNK + max(j - 4 * c, 0) * 128
                                nc.tensor.matmul(
                                    spp[:, off:(h2 + 1) * CHUNK],
                                    kt_sb[head][:, j * 128:(j + 1) * 128],
                                    qts[head][:, off - h2 * CHUNK:],
                                    start=True, stop=True)
                            nc.scalar.activation(ptp[:], spp[:], AF.Exp,
                                                 scale=float(SM_SCALE))
                            for h2 in range(2):
                                j = 2 * pi + h2
                                r = j - 4 * c
                                if r >= 0:
                                    off = h2 * CHUNK + r * 128
                                    nc.vector.tensor_tensor(
                                        out=ptp[:, off:off + 128],
                                        in0=ptp[:, off:off + 128],
                                        in1=tri_sb[:], op=ALU.mult)
                            pairs[pi] = ptp
                        if pi >= PLAG:
                            i = pi - PLAG
                            ptp = pairs.pop(i)
                            for h2 in range(2):
                                j = 2 * i + h2
                                off = h2 * CHUNK + max(j - 4 * c, 0) * 128
                                nc.tensor.matmul(
                                    o_acc[:, off - h2 * CHUNK:],
                                    vn_sb[:, j * M + head * HD:
                                          j * M + (head + 1) * HD],
                                    ptp[:, off:(h2 + 1) * CHUNK],
                                    start=(j == 0), stop=(j == jmax),
                                    skip_group_check=True)
                            if 2 * i + 1 < 4 * c:
                                # full pair: fold its two halves, push at
                                # level 1 (each tree node covers 2 tiles)
                                nt = pzt.tile([128, CHUNK], BF16, name="zt")
                                nc.vector.tensor_tensor(
                                    out=nt[:], in0=ptp[:, 0:CHUNK],
                                    in1=ptp[:, CHUNK:], op=ALU.add)
                                tree_push(nt, 1)
                            else:
                                for h2 in range(2):
                                    j = 2 * i + h2
                                    off = h2 * CHUNK + \
                                        max(j - 4 * c, 0) * 128
                                    nc.tensor.matmul(
                                        z_acc[:, off - h2 * CHUNK:],
                                        ones_sb[:],
                                        ptp[:, off:(h2 + 1) * CHUNK],
                                        start=(j == 4 * c),
                                        stop=(j == jmax and c == 0),
                                        skip_group_check=True)
                    if c > 0:
                        # collapse the tree and reduce over partitions with
                        # one ones-matmul accumulated into the z bank
                        parts = [zstack[l] for l in sorted(zstack)]
                        zt = parts[0]
                        for nxt in parts[1:]:
                            nt = pzt.tile([128, CHUNK], BF16, name="zt")
                            nc.vector.tensor_tensor(out=nt[:], in0=zt[:],
                                                    in1=nxt[:], op=ALU.add)
                            zt = nt
                        nc.tensor.matmul(
                            z_acc[:], ones_sb[:], zt[:],
                            start=False, stop=True, skip_group_check=True)
                    rz = prz.tile([128, CHUNK], FP32, name="rz")
                    nc.vector.reciprocal_approx_fast(out=rz[:], in_=z_acc[:])
                    at = pat.tile([128, CHUNK], BF16, name=f"at{head}")
                    nc.vector.tensor_tensor(out=at[:], in0=o_acc[:],
                                            in1=rz[:], op=ALU.mult)
                    ats.append(at)
                ats_prev = ats
            outproj(ats_prev, NCHUNK - 1, spread=True)
    nc.finalize()
    return nc


def _host_prep(xs, norm_w, wq, wk, wv, wo):
    """Fold RMSNorm into h^T upload + weights; build rope tables; bf16."""
    import ml_dtypes
    BF = ml_dtypes.bfloat16

    x64 = xs.astype(np.float64)
    istd = 1.0 / np.sqrt((x64 * x64).mean(axis=1) + EPS)    # [S]
    ht = (x64 * istd[:, None]).T.astype(BF)                 # [D, S]
    # partition-major pack: ht_pm[p, dt, s] = ht[dt*128+p, s]
    ht_pm = np.ascontiguousarray(
        ht.reshape(DT, 128, SEQ).transpose(1, 0, 2))

    def pack_w(w):  # [D, M] -> [128, DT*M] partition-major
        return np.ascontiguousarray(
            w.reshape(DT, 128, M).transpose(1, 0, 2).reshape(128, DT * M))

    nw = norm_w.astype(np.float32)[:, None, None]
    perm = np.concatenate([np.arange(0, HD, 2), np.arange(1, HD, 2)])
    wq_p = (wq * nw)[:, :, perm]
    wk_p = (wk * nw)[:, :, perm]

    inv_freq = 1.0 / (ROPE_BASE ** (np.arange(0, HD, 2, dtype=np.float32) / HD))
    pos = np.arange(SEQ, dtype=np.float32)
    ang = pos[:, None] * inv_freq[None, :]          # [S, 64]
    cos_t = np.cos(ang).T.astype(np.float32)        # [64, S]
    sin_t = np.sin(ang).T.astype(np.float32)
    cosd = np.ascontiguousarray(np.concatenate([cos_t, cos_t], 0))
    sind = np.ascontiguousarray(np.concatenate([sin_t, sin_t], 0))

    tri = np.triu(np.ones((128, 128), dtype=np.float32)).astype(BF)
    onesm = np.ones((128, 128), dtype=BF)

    common = {
        "ht": ht_pm,
        "cosd": cosd,
        "sind": sind,
        "tri": np.ascontiguousarray(tri),
        "ones": onesm,
    }
    in_maps = []
    for core in range(NCORES):
        sl = slice(core * HPC, (core + 1) * HPC)
        wot = np.transpose(wo[:, sl, :], (2, 1, 0)).reshape(128, HPC * D)
        in_maps.append({
            **common,
            "wq": pack_w(wq_p[:, sl, :].reshape(D, M).astype(BF)),
            "wk": pack_w(wk_p[:, sl, :].reshape(D, M).astype(BF)),
            "wv": pack_w((wv * nw)[:, sl, :].reshape(D, M).astype(BF)),
            "wot": np.ascontiguousarray(wot.astype(BF)),
        })
    return in_maps


def kernel(xs, norm_w, wq, wk, wv, wo):
    trace = bool(int(os.environ.get("KERNEL_TRACE", "0")))
    if trace:
        _inject_ntff_hook()
    from concourse.bass_utils import run_bass_kernel_spmd

    nc = _build_nc()
    in_maps = _host_prep(np.asarray(xs), np.asarray(norm_w), np.asarray(wq),
                         np.asarray(wk), np.asarray(wv), np.asarray(wo))
    try:
        res = run_bass_kernel_spmd(nc, in_maps, core_ids=list(range(NCORES)),
                                   trace=trace)
    except Exception:
        # transient device wedge (NRT_EXEC_UNIT_UNRECOVERABLE) recovers on
        # a fresh attempt; rebuild and retry once
        import time
        time.sleep(15)
        nc = _build_nc()
        res = run_bass_kernel_spmd(nc, in_maps, core_ids=list(range(NCORES)),
                                   trace=trace)
    if trace and res.exec_time_ns is not None:
        print(f"HW exec time: {res.exec_time_ns} ns")
    acc = np.zeros((SEQ, D), dtype=np.float64)
    for r in res.results:
        acc += r["out"].astype(np.float64)
    return acc.astype(np.float32)


if __name__ == "__main__":
    rng = np.random.default_rng(0)
    scale = 1.0 / np.sqrt(D)
    inputs = {
        "xs": rng.standard_normal((SEQ, D), dtype=np.float32),
        "norm_w": np.ones((D,), np.float32),
        "wq": rng.standard_normal((D, NH, HD), dtype=np.float32) * scale,
        "wk": rng.standard_normal((D, NH, HD), dtype=np.float32) * scale,
        "wv": rng.standard_normal((D, NH, HD), dtype=np.float32) * scale,
        "wo": rng.standard_normal((D, NH, HD), dtype=np.float32) * scale,
    }
    out = kernel(**inputs)
    print(out.shape, out.dtype, float(np.abs(out).max()))


# revision 36
# speedup vs baseline: 1.0797x; 1.0018x over previous
"""Trainium2 Bass kernel: RMSNorm + RoPE + causal attention + output projection.

Tensor-parallel over heads: 16 heads / 8 cores = 2 heads per core.
Each core computes a full [S, D] partial output (its heads' contribution to
the 'snh,dnh->sd' projection); the all-reduce is done host-side in the gather.

Fused streaming design (bf16, ~436 us vs 674 us baseline):
  - Host prep uploads the RMSNorm'd activations already transposed and
    partition-major packed (h^T as [128, DT, S] bf16) plus bf16 weights,
    a pre-transposed wo^T, and fp32 RoPE tables.  No PE transposes, no
    DRAM scratch roundtrip, and every DMA is a cheap contiguous pattern
    (strided DIRECT2D issue cost was multi-us per descriptor program).
  - Single pass over 8 q-chunks of 512: QK projections (+RoPE) append to
    per-head K^T in SBUF, V is projected directly into natural [t, hd]
    layout (ht-tile stationary), then causal attention for the chunk runs
    against all K/V tiles so far.  The previous chunk's output projection
    is emitted between the two heads' attention so its matmuls cover the
    softmax recip+normalize latency, and the next chunk's h^T/rope-slice
    DMAs prefetch under the current chunk's compute.
  - Scores are computed transposed (S^T[t, s]) at PAIR granularity: two
    j-tiles' scores land in one 2-bank [128, 1024] PSUM tile and a single
    wide exp covers both, halving ACT per-instruction overhead (exp is
    the attention pacer at ~507 ns/j; PE runs ~430 ns/j underneath).
    Softmax denominators: full pairs fold on the DVE as a bf16 pairwise
    tree (~436 ns/add, inside the exp shadow); only the 4 diagonal tiles
    + one final tree reduction hit the PE as ones-matmuls.  PV/Z lag
    scores/exp by 2 pairs so a late exp never stalls the in-order PE
    queue; reciprocal_approx_fast instead of the 3.4us DVE reciprocal.
  - PSUM (16 KiB fully allocated): main 2x2-bank slots (proj pp / score
    pairs / V accum), aux 2 banks (rope ps / outproj op), o 1 bank (PV
    accum), z 1 bank (denominator).  The final chunk's outproj spreads
    over main+aux.
  - Startup: priority-ordered sync-queue DMA FIFO + 100 junk matmuls to
    hold the PE's HAM clock warm while the first loads land.
"""
import os
import sys
import types

import numpy as np

SEQ, D, NH, HD = 4096, 2048, 16, 128
NCORES = 8
HPC = NH // NCORES          # heads per core
M = HPC * HD                # per-core fused head dim (256)
EPS = 1e-6
ROPE_BASE = 10000.0
SM_SCALE = 1.0 / np.sqrt(HD)
CHUNK = 512                 # q-chunk
NCHUNK = SEQ // CHUNK       # 8
NT = SEQ // 128             # 32 s-tiles
DT = D // 128               # 16 d-tiles
LAG = 2


def _inject_ntff_hook():
    """Register the axon NTFF profiling hook (missing antenv.axon_hooks)."""
    if "antenv.axon_hooks" in sys.modules:
        return
    try:
        import antenv
        from trn_agent_boot.trn_boot import _ntff_profile_via_ctypes
    except ImportError:
        return
    holder = [None]
    mod = types.ModuleType("antenv.axon_hooks")
    mod.set_axon_ntff_profile_hook = lambda h: holder.__setitem__(0, h)
    mod.get_axon_ntff_profile_hook = lambda: holder[0]
    sys.modules["antenv.axon_hooks"] = mod
    antenv.axon_hooks = mod
    try:
        mod.set_axon_ntff_profile_hook(
            _ntff_profile_via_ctypes("/opt/axon/libaxon_pjrt.so"))
    except Exception:
        pass


def _build_nc():
    import concourse.bass as bass  # noqa: F401
    import concourse.mybir as mybir
    import concourse.tile as tile
    from concourse import bacc

    FP32 = mybir.dt.float32
    BF16 = mybir.dt.bfloat16
    AF = mybir.ActivationFunctionType
    ALU = mybir.AluOpType

    nc = bacc.Bacc(None, target_bir_lowering=False)

    ht_d = nc.declare_dram_parameter("ht", [128, DT, SEQ], BF16,
                                     isOutput=False)
    wq = nc.declare_dram_parameter("wq", [128, DT * M], BF16, isOutput=False)
    wk = nc.declare_dram_parameter("wk", [128, DT * M], BF16, isOutput=False)
    wv = nc.declare_dram_parameter("wv", [128, DT * M], BF16, isOutput=False)
    wot_d = nc.declare_dram_parameter("wot", [128, HPC * D], BF16,
                                      isOutput=False)
    cosd = nc.declare_dram_parameter("cosd", [128, SEQ], FP32, isOutput=False)
    sind = nc.declare_dram_parameter("sind", [128, SEQ], FP32, isOutput=False)
    tri = nc.declare_dram_parameter("tri", [128, 128], BF16, isOutput=False)
    ones = nc.declare_dram_parameter("ones", [128, 128], BF16, isOutput=False)
    out = nc.declare_dram_parameter("out", [SEQ, D], BF16, isOutput=True)

    with tile.TileContext(nc) as tc:
        with tc.tile_pool(name="consts", bufs=1) as consts, \
             tc.tile_pool(name="pht", bufs=2) as pht, \
             tc.tile_pool(name="pqt", bufs=4) as pqt, \
             tc.tile_pool(name="ppc", bufs=2) as ppc, \
             tc.tile_pool(name="ppt", bufs=6) as ppt, \
             tc.tile_pool(name="prz", bufs=2) as prz, \
             tc.tile_pool(name="pat", bufs=4) as pat, \
             tc.tile_pool(name="post", bufs=3) as post, \
             tc.tile_pool(name="pcs", bufs=2) as pcs, \
             tc.tile_pool(name="pzt", bufs=8) as pzt, \
             tc.tile_pool(name="psum", bufs=2, space="PSUM") as psum:
            kt_sb = [consts.tile([128, SEQ], BF16, name=f"kt{h}")
                     for h in range(HPC)]
            # V natural, packed per t-tile: vn[p, jt*M + m] = V[jt*128+p, m]
            vn_sb = consts.tile([128, NT * M], BF16)

            def load_ht(c):
                cs = slice(c * CHUNK, (c + 1) * CHUNK)
                ht = pht.tile([128, DT, CHUNK], BF16, name="ht")
                for dt8 in range(2):
                    nc.sync.dma_start(
                        out=ht[:, dt8 * 8:(dt8 + 1) * 8, :],
                        in_=ht_d[:, dt8 * 8:(dt8 + 1) * 8, cs])
                return ht

            def load_cs(c):
                cs = slice(c * CHUNK, (c + 1) * CHUNK)
                sin_t = pcs.tile([128, CHUNK], FP32, name="sin_t", tag="sin")
                nc.sync.dma_start(out=sin_t[:], in_=sind[:, cs])
                cos_t = pcs.tile([128, CHUNK], FP32, name="cos_t", tag="cos")
                nc.sync.dma_start(out=cos_t[:], in_=cosd[:, cs])
                return cos_t, sin_t

            # startup: one sync-queue FIFO in priority order — ring
            # bandwidth serves the first projections' pieces (ht, wq,
            # rope slices) before the bulk loads behind them.
            w_sbs = {k: consts.tile([128, DT, M], BF16, name=f"w{k}_sb")
                     for k in ("q", "k", "v")}
            cs0 = slice(0, CHUNK)
            ht = pht.tile([128, DT, CHUNK], BF16, name="ht")
            for q4 in range(4):
                dts = slice(q4 * 4, (q4 + 1) * 4)
                nc.sync.dma_start(out=ht[:, dts, :], in_=ht_d[:, dts, cs0])
                nc.sync.dma_start(
                    out=w_sbs["q"][:, dts, :].rearrange("p t m -> p (t m)"),
                    in_=wq[:, q4 * 4 * M:(q4 + 1) * 4 * M])
                if q4 == 1:
                    cs_cur = load_cs(0)
            nc.sync.dma_start(
                out=w_sbs["k"][:].rearrange("p t m -> p (t m)"), in_=wk[:])
            nc.sync.dma_start(
                out=w_sbs["v"][:].rearrange("p t m -> p (t m)"), in_=wv[:])
            tri_sb = consts.tile([128, 128], BF16)
            nc.sync.dma_start(out=tri_sb[:], in_=tri[:])
            ones_sb = consts.tile([128, 128], BF16)
            nc.sync.dma_start(out=ones_sb[:], in_=ones[:])
            wot_sb = consts.tile([128, HPC * D], BF16)
            nc.sync.dma_start(out=wot_sb[:], in_=wot_d[:])

            # HAM warm-up: keep the PE busy on junk matmuls while the
            # startup DMAs land, so real matmuls start at the warm clock.
            warm = consts.tile([128, 128], BF16)
            nc.vector.memset(warm[:], 0.0)
            for i in range(100):
                wp = psum.tile([128, CHUNK], FP32, name="wp", tag="main")
                nc.tensor.matmul(wp[:, 0:128], warm[:], warm[:],
                                 start=True, stop=True)

            def outproj(ats, c, spread=False):
                for st in range(4):
                    ost = post.tile([128, D], BF16, name="ost")
                    for dc in range(4):
                        # the final chunk's projection has nothing after it
                        # to hide evac latency: spread over two tag rings so
                        # 4 banks ping-pong instead of 2.
                        tag = "main" if spread and dc % 2 else "aux"
                        op = psum.tile([128, CHUNK], FP32, name="op",
                                       tag=tag)
                        nc.tensor.matmul(
                            op[:],
                            ats[0][:, st * 128:(st + 1) * 128],
                            wot_sb[:, dc * CHUNK:(dc + 1) * CHUNK],
                            start=True, stop=False)
                        nc.tensor.matmul(
                            op[:],
                            ats[1][:, st * 128:(st + 1) * 128],
                            wot_sb[:, D + dc * CHUNK:D + (dc + 1) * CHUNK],
                            start=False, stop=True)
                        if dc % 2 == 0:
                            nc.scalar.activation(
                                ost[:, dc * CHUNK:(dc + 1) * CHUNK],
                                op[:], AF.Copy)
                        else:
                            nc.vector.tensor_copy(
                                ost[:, dc * CHUNK:(dc + 1) * CHUNK], op[:])
                    row = (c * 4 + st) * 128
                    eng = nc.sync if st % 2 == 0 else nc.scalar
                    eng.dma_start(out=out[row:row + 128, :], in_=ost[:])

            ats_prev = None
            for c in range(NCHUNK):
                cs = slice(c * CHUNK, (c + 1) * CHUNK)
                cos_t, sin_t = cs_cur

                # ---- QK projections + RoPE
                qts = []
                for head in range(HPC):
                    for kind in ("q", "k"):
                        w_sb = w_sbs[kind]
                        pp = psum.tile([128, CHUNK], FP32, name="pp",
                                       tag="main")
                        for dt in range(DT):
                            nc.tensor.matmul(
                                pp[:],
                                w_sb[:, dt, head * HD:(head + 1) * HD],
                                ht[:, dt, :],
                                start=(dt == 0), stop=(dt == DT - 1))
                        ps = psum.tile([128, CHUNK], FP32, name="ps",
                                       tag="aux")
                        nc.vector.tensor_tensor(
                            out=ps[:], in0=pp[:], in1=sin_t[:],
                            op=ALU.mult)
                        pc = ppc.tile([128, CHUNK], FP32, name="pc")
                        nc.vector.tensor_tensor(
                            out=pc[:], in0=pp[:], in1=cos_t[:],
                            op=ALU.mult)
                        if kind == "q":
                            dst = pqt.tile([128, CHUNK], BF16,
                                           name=f"qt{head}")
                            qts.append(dst)
                            d0, d1 = dst[0:64, :], dst[64:128, :]
                        else:
                            d0 = kt_sb[head][0:64, cs]
                            d1 = kt_sb[head][64:128, cs]
                        nc.vector.tensor_tensor(
                            out=d0, in0=pc[0:64, :], in1=ps[64:128, :],
                            op=ALU.subtract)
                        nc.vector.tensor_tensor(
                            out=d1, in0=pc[64:128, :], in1=ps[0:64, :],
                            op=ALU.add)

                # ---- V projection directly into natural [t, m] layout
                for g in range(2):
                    pv = psum.tile([128, CHUNK], FP32, name="pv", tag="aux")
                    for half in range(2):
                        st = g * 2 + half
                        for dt in range(DT):
                            nc.tensor.matmul(
                                pv[:, half * M:(half + 1) * M],
                                ht[:, dt, st * 128:(st + 1) * 128],
                                w_sbs["v"][:, dt, :],
                                start=(dt == 0), stop=(dt == DT - 1))
                    t0 = (c * 4 + g * 2) * M
                    nc.vector.tensor_copy(vn_sb[:, t0:t0 + 2 * M], pv[:])

                # prefetch next chunk's h^T + rope slices while computing
                if c + 1 < NCHUNK:
                    ht = load_ht(c + 1)
                    cs_cur = load_cs(c + 1)

                # ---- causal attention per head; the previous chunk's output
                # projection is emitted between the heads: its matmuls cover
                # head 0's recip+normalize latency and give the scalar engine
                # room to drain the evacuation copies before head 1's exps.
                ats = []
                for head in range(HPC):
                    if head == 1 and ats_prev is not None:
                        outproj(ats_prev, c - 1)
                    o_acc = psum.tile([128, CHUNK], FP32, name="o_acc",
                                      tag="o", bufs=1)
                    z_acc = psum.tile([128, CHUNK], FP32, name="z_acc",
                                      tag="z", bufs=1)
                    jmax = 4 * c + 3
                    # Attention inner loop at PAIR granularity: two j-tiles'
                    # scores land in one 2-bank [128, 1024] PSUM tile and a
                    # single wide exp covers both (halves ACT's per-
                    # instruction overhead — exp paces attention now).
                    # Softmax denominator: full pairs fold on the DVE as a
                    # bf16 pairwise tree; only the 4 diagonal tiles + one
                    # final tree reduction hit the PE.
                    zstack = {}

                    def tree_push(t, lvl):
                        while lvl in zstack:
                            prev = zstack.pop(lvl)
                            nt = pzt.tile([128, CHUNK], BF16, name="zt")
                            nc.vector.tensor_tensor(out=nt[:], in0=prev[:],
                                                    in1=t[:], op=ALU.add)
                            t = nt
                            lvl += 1
                        zstack[lvl] = t

                    npair = (jmax + 1) // 2
                    PLAG = 2
                    pairs = {}
                    for pi in range(npair + PLAG):
                        if pi < npair:
                            spp = psum.tile([128, 2 * CHUNK], FP32,
                                            name="spp", tag="main")
                            ptp = ppt.tile([128, 2 * CHUNK], BF16,
                                           name="ptp")
                            for h2 in range(2):
                                j = 2 * pi + h2
                                off = h2 * CHUNK + max(j - 4 * c, 0) * 128
                                nc.tensor.matmul(
                                    spp[:, off:(h2 + 1) * CHUNK],
                                    kt_sb[head][:, j * 128:(j + 1) * 128],
                                    qts[head][:, off - h2 * CHUNK:],
                                    start=True, stop=True)
                            lo = max(2 * pi - 4 * c, 0) * 128
                            nc.scalar.activation(ptp[:, lo:], spp[:, lo:],
                                                 AF.Exp,
                                                 scale=float(SM_SCALE))
                            for h2 in range(2):
                                j = 2 * pi + h2
                                r = j - 4 * c
                                if r >= 0:
                                    off = h2 * CHUNK + r * 128
                                    nc.vector.tensor_tensor(
                                        out=ptp[:, off:off + 128],
                                        in0=ptp[:, off:off + 128],
                                        in1=tri_sb[:], op=ALU.mult)
                            pairs[pi] = ptp
                        if pi >= PLAG:
                            i = pi - PLAG
                            ptp = pairs.pop(i)
                            for h2 in range(2):
                                j = 2 * i + h2
                                off = h2 * CHUNK + max(j - 4 * c, 0) * 128
                                nc.tensor.matmul(
                                    o_acc[:, off - h2 * CHUNK:],
                                    vn_sb[:, j * M + head * HD:
                                          j * M + (head + 1) * HD],
                                    ptp[:, off:(h2 + 1) * CHUNK],
                                    start=(j == 0), stop=(j == jmax),
                                    skip_group_check=True)
                            if 2 * i + 1 < 4 * c:
                                # full pair: fold its two halves, push at
                                # level 1 (each tree node covers 2 tiles)
                                nt = pzt.tile([128, CHUNK], BF16, name="zt")
                                nc.vector.tensor_tensor(
                                    out=nt[:], in0=ptp[:, 0:CHUNK],
                                    in1=ptp[:, CHUNK:], op=ALU.add)
                                tree_push(nt, 1)
                            else:
                                for h2 in range(2):
                                    j = 2 * i + h2
                                    off = h2 * CHUNK + \
                                        max(j - 4 * c, 0) * 128
                                    nc.tensor.matmul(
                                        z_acc[:, off - h2 * CHUNK:],
                                        ones_sb[:],
                                        ptp[:, off:(h2 + 1) * CHUNK],
                                        start=(j == 4 * c),
                                        stop=(j == jmax and c == 0),
                                        skip_group_check=True)
                    if c > 0:
                        # collapse the tree and reduce over partitions with
                        # one ones-matmul accumulated into the z bank
                        parts = [zstack[l] for l in sorted(zstack)]
                        zt = parts[0]
                        for nxt in parts[1:]:
                            nt = pzt.tile([128, CHUNK], BF16, name="zt")
                            nc.vector.tensor_tensor(out=nt[:], in0=zt[:],
                                                    in1=nxt[:], op=ALU.add)
                            zt = nt
                        nc.tensor.matmul(
                            z_acc[:], ones_sb[:], zt[:],
                            start=False, stop=True, skip_group_check=True)
                    rz = prz.tile([128, CHUNK], FP32, name="rz")
                    nc.vector.reciprocal_approx_fast(out=rz[:], in_=z_acc[:])
                    at = pat.tile([128, CHUNK], BF16, name=f"at{head}")
                    nc.vector.tensor_tensor(out=at[:], in0=o_acc[:],
                                            in1=rz[:], op=ALU.mult)
                    ats.append(at)
                ats_prev = ats
            outproj(ats_prev, NCHUNK - 1, spread=True)
    nc.finalize()
    return nc


def _host_prep(xs, norm_w, wq, wk, wv, wo):
    """Fold RMSNorm into h^T upload + weights; build rope tables; bf16."""
    import ml_dtypes
    BF = ml_dtypes.bfloat16

    x64 = xs.astype(np.float64)
    istd = 1.0 / np.sqrt((x64 * x64).mean(axis=1) + EPS)    # [S]
    ht = (x64 * istd[:, None]).T.astype(BF)                 # [D, S]
    # partition-major pack: ht_pm[p, dt, s] = ht[dt*128+p, s]
    ht_pm = np.ascontiguousarray(
        ht.reshape(DT, 128, SEQ).transpose(1, 0, 2))

    def pack_w(w):  # [D, M] -> [128, DT*M] partition-major
        return np.ascontiguousarray(
            w.reshape(DT, 128, M).transpose(1, 0, 2).reshape(128, DT * M))

    nw = norm_w.astype(np.float32)[:, None, None]
    perm = np.concatenate([np.arange(0, HD, 2), np.arange(1, HD, 2)])
    wq_p = (wq * nw)[:, :, perm]
    wk_p = (wk * nw)[:, :, perm]

    inv_freq = 1.0 / (ROPE_BASE ** (np.arange(0, HD, 2, dtype=np.float32) / HD))
    pos = np.arange(SEQ, dtype=np.float32)
    ang = pos[:, None] * inv_freq[None, :]          # [S, 64]
    cos_t = np.cos(ang).T.astype(np.float32)        # [64, S]
    sin_t = np.sin(ang).T.astype(np.float32)
    cosd = np.ascontiguousarray(np.concatenate([cos_t, cos_t], 0))
    sind = np.ascontiguousarray(np.concatenate([sin_t, sin_t], 0))

    tri = np.triu(np.ones((128, 128), dtype=np.float32)).astype(BF)
    onesm = np.ones((128, 128), dtype=BF)

    common = {
        "ht": ht_pm,
        "cosd": cosd,
        "sind": sind,
        "tri": np.ascontiguousarray(tri),
        "ones": onesm,
    }
    in_maps = []
    for core in range(NCORES):
        sl = slice(core * HPC, (core + 1) * HPC)
        wot = np.transpose(wo[:, sl, :], (2, 1, 0)).reshape(128, HPC * D)
        in_maps.append({
            **common,
            "wq": pack_w(wq_p[:, sl, :].reshape(D, M).astype(BF)),
            "wk": pack_w(wk_p[:, sl, :].reshape(D, M).astype(BF)),
            "wv": pack_w((wv * nw)[:, sl, :].reshape(D, M).astype(BF)),
            "wot": np.ascontiguousarray(wot.astype(BF)),
        })
    return in_maps


def kernel(xs, norm_w, wq, wk, wv, wo):
    trace = bool(int(os.environ.get("KERNEL_TRACE", "0")))
    if trace:
        _inject_ntff_hook()
    from concourse.bass_utils import run_bass_kernel_spmd

    nc = _build_nc()
    in_maps = _host_prep(np.asarray(xs), np.asarray(norm_w), np.asarray(wq),
                         np.asarray(wk), np.asarray(wv), np.asarray(wo))
    try:
        res = run_bass_kernel_spmd(nc, in_maps, core_ids=list(range(NCORES)),
                                   trace=trace)
    except Exception:
        # transient device wedge (NRT_EXEC_UNIT_UNRECOVERABLE) recovers on
        # a fresh attempt; rebuild and retry once
        import time
        time.sleep(15)
        nc = _build_nc()
        res = run_bass_kernel_spmd(nc, in_maps, core_ids=list(range(NCORES)),
                                   trace=trace)
    if trace and res.exec_time_ns is not None:
        print(f"HW exec time: {res.exec_time_ns} ns")
    acc = np.zeros((SEQ, D), dtype=np.float64)
    for r in res.results:
        acc += r["out"].astype(np.float64)
    return acc.astype(np.float32)


if __name__ == "__main__":
    rng = np.random.default_rng(0)
    scale = 1.0 / np.sqrt(D)
    inputs = {
        "xs": rng.standard_normal((SEQ, D), dtype=np.float32),
        "norm_w": np.ones((D,), np.float32),
        "wq": rng.standard_normal((D, NH, HD), dtype=np.float32) * scale,
        "wk": rng.standard_normal((D, NH, HD), dtype=np.float32) * scale,
        "wv": rng.standard_normal((D, NH, HD), dtype=np.float32) * scale,
        "wo": rng.standard_normal((D, NH, HD), dtype=np.float32) * scale,
    }
    out = kernel(**inputs)
    print(out.shape, out.dtype, float(np.abs(out).max()))


# revision 37
# speedup vs baseline: 1.0836x; 1.0036x over previous
"""Trainium2 Bass kernel: RMSNorm + RoPE + causal attention + output projection.

Tensor-parallel over heads: 16 heads / 8 cores = 2 heads per core.
Each core computes a full [S, D] partial output (its heads' contribution to
the 'snh,dnh->sd' projection); the all-reduce is done host-side in the gather.

Fused streaming design (bf16, ~436 us vs 674 us baseline):
  - Host prep uploads the RMSNorm'd activations already transposed and
    partition-major packed (h^T as [128, DT, S] bf16) plus bf16 weights,
    a pre-transposed wo^T, and fp32 RoPE tables.  No PE transposes, no
    DRAM scratch roundtrip, and every DMA is a cheap contiguous pattern
    (strided DIRECT2D issue cost was multi-us per descriptor program).
  - Single pass over 8 q-chunks of 512: QK projections (+RoPE) append to
    per-head K^T in SBUF, V is projected directly into natural [t, hd]
    layout (ht-tile stationary), then causal attention for the chunk runs
    against all K/V tiles so far.  The previous chunk's output projection
    is emitted between the two heads' attention so its matmuls cover the
    softmax recip+normalize latency, and the next chunk's h^T/rope-slice
    DMAs prefetch under the current chunk's compute.
  - Scores are computed transposed (S^T[t, s]) at PAIR granularity: two
    j-tiles' scores land in one 2-bank [128, 1024] PSUM tile and a single
    wide exp covers both, halving ACT per-instruction overhead (exp is
    the attention pacer at ~507 ns/j; PE runs ~430 ns/j underneath).
    Softmax denominators: full pairs fold on the DVE as a bf16 pairwise
    tree (~436 ns/add, inside the exp shadow); only the 4 diagonal tiles
    + one final tree reduction hit the PE as ones-matmuls.  PV/Z lag
    scores/exp by 2 pairs so a late exp never stalls the in-order PE
    queue; reciprocal_approx_fast instead of the 3.4us DVE reciprocal.
  - PSUM (16 KiB fully allocated): main 2x2-bank slots (proj pp / score
    pairs / V accum), aux 2 banks (rope ps / outproj op), o 1 bank (PV
    accum), z 1 bank (denominator).  The final chunk's outproj spreads
    over main+aux.
  - Startup: priority-ordered sync-queue DMA FIFO + 100 junk matmuls to
    hold the PE's HAM clock warm while the first loads land.
"""
import os
import sys
import types

import numpy as np

SEQ, D, NH, HD = 4096, 2048, 16, 128
NCORES = 8
HPC = NH // NCORES          # heads per core
M = HPC * HD                # per-core fused head dim (256)
EPS = 1e-6
ROPE_BASE = 10000.0
SM_SCALE = 1.0 / np.sqrt(HD)
CHUNK = 512                 # q-chunk
NCHUNK = SEQ // CHUNK       # 8
NT = SEQ // 128             # 32 s-tiles
DT = D // 128               # 16 d-tiles
LAG = 2


def _inject_ntff_hook():
    """Register the axon NTFF profiling hook (missing antenv.axon_hooks)."""
    if "antenv.axon_hooks" in sys.modules:
        return
    try:
        import antenv
        from trn_agent_boot.trn_boot import _ntff_profile_via_ctypes
    except ImportError:
        return
    holder = [None]
    mod = types.ModuleType("antenv.axon_hooks")
    mod.set_axon_ntff_profile_hook = lambda h: holder.__setitem__(0, h)
    mod.get_axon_ntff_profile_hook = lambda: holder[0]
    sys.modules["antenv.axon_hooks"] = mod
    antenv.axon_hooks = mod
    try:
        mod.set_axon_ntff_profile_hook(
            _ntff_profile_via_ctypes("/opt/axon/libaxon_pjrt.so"))
    except Exception:
        pass


def _build_nc():
    import concourse.bass as bass  # noqa: F401
    import concourse.mybir as mybir
    import concourse.tile as tile
    from concourse import bacc

    FP32 = mybir.dt.float32
    BF16 = mybir.dt.bfloat16
    AF = mybir.ActivationFunctionType
    ALU = mybir.AluOpType

    nc = bacc.Bacc(None, target_bir_lowering=False)

    ht_d = nc.declare_dram_parameter("ht", [128, DT, SEQ], BF16,
                                     isOutput=False)
    wq = nc.declare_dram_parameter("wq", [128, DT * M], BF16, isOutput=False)
    wk = nc.declare_dram_parameter("wk", [128, DT * M], BF16, isOutput=False)
    wv = nc.declare_dram_parameter("wv", [128, DT * M], BF16, isOutput=False)
    wot_d = nc.declare_dram_parameter("wot", [128, HPC * D], BF16,
                                      isOutput=False)
    cosd = nc.declare_dram_parameter("cosd", [128, SEQ], FP32, isOutput=False)
    sind = nc.declare_dram_parameter("sind", [128, SEQ], FP32, isOutput=False)
    tri = nc.declare_dram_parameter("tri", [128, 128], BF16, isOutput=False)
    ones = nc.declare_dram_parameter("ones", [128, 128], BF16, isOutput=False)
    out = nc.declare_dram_parameter("out", [SEQ, D], BF16, isOutput=True)

    with tile.TileContext(nc) as tc:
        with tc.tile_pool(name="consts", bufs=1) as consts, \
             tc.tile_pool(name="pht", bufs=2) as pht, \
             tc.tile_pool(name="pqt", bufs=4) as pqt, \
             tc.tile_pool(name="ppc", bufs=2) as ppc, \
             tc.tile_pool(name="ppt", bufs=6) as ppt, \
             tc.tile_pool(name="prz", bufs=2) as prz, \
             tc.tile_pool(name="pat", bufs=4) as pat, \
             tc.tile_pool(name="post", bufs=3) as post, \
             tc.tile_pool(name="pcs", bufs=2) as pcs, \
             tc.tile_pool(name="pzt", bufs=8) as pzt, \
             tc.tile_pool(name="psum", bufs=2, space="PSUM") as psum:
            kt_sb = [consts.tile([128, SEQ], BF16, name=f"kt{h}")
                     for h in range(HPC)]
            # V natural, packed per t-tile: vn[p, jt*M + m] = V[jt*128+p, m]
            vn_sb = consts.tile([128, NT * M], BF16)

            def load_ht(c):
                cs = slice(c * CHUNK, (c + 1) * CHUNK)
                ht = pht.tile([128, DT, CHUNK], BF16, name="ht")
                for dt8 in range(2):
                    nc.sync.dma_start(
                        out=ht[:, dt8 * 8:(dt8 + 1) * 8, :],
                        in_=ht_d[:, dt8 * 8:(dt8 + 1) * 8, cs])
                return ht

            def load_cs(c):
                cs = slice(c * CHUNK, (c + 1) * CHUNK)
                sin_t = pcs.tile([128, CHUNK], FP32, name="sin_t", tag="sin")
                nc.sync.dma_start(out=sin_t[:], in_=sind[:, cs])
                cos_t = pcs.tile([128, CHUNK], FP32, name="cos_t", tag="cos")
                nc.sync.dma_start(out=cos_t[:], in_=cosd[:, cs])
                return cos_t, sin_t

            # startup: one sync-queue FIFO in priority order — ring
            # bandwidth serves the first projections' pieces (ht, wq,
            # rope slices) before the bulk loads behind them.
            w_sbs = {k: consts.tile([128, DT, M], BF16, name=f"w{k}_sb")
                     for k in ("q", "k", "v")}
            cs0 = slice(0, CHUNK)
            ht = pht.tile([128, DT, CHUNK], BF16, name="ht")
            for q4 in range(4):
                dts = slice(q4 * 4, (q4 + 1) * 4)
                nc.sync.dma_start(out=ht[:, dts, :], in_=ht_d[:, dts, cs0])
                nc.sync.dma_start(
                    out=w_sbs["q"][:, dts, :].rearrange("p t m -> p (t m)"),
                    in_=wq[:, q4 * 4 * M:(q4 + 1) * 4 * M])
                if q4 == 1:
                    cs_cur = load_cs(0)
            nc.sync.dma_start(
                out=w_sbs["k"][:].rearrange("p t m -> p (t m)"), in_=wk[:])
            nc.sync.dma_start(
                out=w_sbs["v"][:].rearrange("p t m -> p (t m)"), in_=wv[:])
            tri_sb = consts.tile([128, 128], BF16)
            nc.sync.dma_start(out=tri_sb[:], in_=tri[:])
            ones_sb = consts.tile([128, 128], BF16)
            nc.sync.dma_start(out=ones_sb[:], in_=ones[:])
            wot_sb = consts.tile([128, HPC * D], BF16)
            nc.sync.dma_start(out=wot_sb[:], in_=wot_d[:])

            # HAM warm-up: keep the PE busy on junk matmuls while the
            # startup DMAs land, so real matmuls start at the warm clock.
            warm = consts.tile([128, 128], BF16)
            nc.vector.memset(warm[:], 0.0)
            for i in range(100):
                wp = psum.tile([128, CHUNK], FP32, name="wp", tag="main")
                nc.tensor.matmul(wp[:, 0:128], warm[:], warm[:],
                                 start=True, stop=True)

            def outproj(ats, c, spread=False):
                for st in range(4):
                    ost = post.tile([128, D], BF16, name="ost")
                    for dc in range(4):
                        # the final chunk's projection has nothing after it
                        # to hide evac latency: spread over two tag rings so
                        # 4 banks ping-pong instead of 2.
                        tag = "main" if spread and dc % 2 else "aux"
                        op = psum.tile([128, CHUNK], FP32, name="op",
                                       tag=tag)
                        nc.tensor.matmul(
                            op[:],
                            ats[0][:, st * 128:(st + 1) * 128],
                            wot_sb[:, dc * CHUNK:(dc + 1) * CHUNK],
                            start=True, stop=False)
                        nc.tensor.matmul(
                            op[:],
                            ats[1][:, st * 128:(st + 1) * 128],
                            wot_sb[:, D + dc * CHUNK:D + (dc + 1) * CHUNK],
                            start=False, stop=True)
                        if dc % 2 == 0:
                            nc.scalar.activation(
                                ost[:, dc * CHUNK:(dc + 1) * CHUNK],
                                op[:], AF.Copy)
                        else:
                            nc.vector.tensor_copy(
                                ost[:, dc * CHUNK:(dc + 1) * CHUNK], op[:])
                    row = (c * 4 + st) * 128
                    eng = nc.sync if st % 2 == 0 else nc.scalar
                    eng.dma_start(out=out[row:row + 128, :], in_=ost[:])

            ats_prev = None
            for c in range(NCHUNK):
                cs = slice(c * CHUNK, (c + 1) * CHUNK)
                cos_t, sin_t = cs_cur

                # ---- QK projections + RoPE
                qts = []
                for head in range(HPC):
                    for kind in ("q", "k"):
                        w_sb = w_sbs[kind]
                        pp = psum.tile([128, CHUNK], FP32, name="pp",
                                       tag="main")
                        for dt in range(DT):
                            nc.tensor.matmul(
                                pp[:],
                                w_sb[:, dt, head * HD:(head + 1) * HD],
                                ht[:, dt, :],
                                start=(dt == 0), stop=(dt == DT - 1))
                        ps = psum.tile([128, CHUNK], FP32, name="ps",
                                       tag="aux")
                        nc.vector.tensor_tensor(
                            out=ps[:], in0=pp[:], in1=sin_t[:],
                            op=ALU.mult)
                        pc = ppc.tile([128, CHUNK], FP32, name="pc")
                        nc.vector.tensor_tensor(
                            out=pc[:], in0=pp[:], in1=cos_t[:],
                            op=ALU.mult)
                        if kind == "q":
                            dst = pqt.tile([128, CHUNK], BF16,
                                           name=f"qt{head}")
                            qts.append(dst)
                            d0, d1 = dst[0:64, :], dst[64:128, :]
                        else:
                            d0 = kt_sb[head][0:64, cs]
                            d1 = kt_sb[head][64:128, cs]
                        nc.vector.tensor_tensor(
                            out=d0, in0=pc[0:64, :], in1=ps[64:128, :],
                            op=ALU.subtract)
                        nc.vector.tensor_tensor(
                            out=d1, in0=pc[64:128, :], in1=ps[0:64, :],
                            op=ALU.add)

                # ---- V projection directly into natural [t, m] layout
                for g in range(2):
                    pv = psum.tile([128, CHUNK], FP32, name="pv", tag="aux")
                    for half in range(2):
                        st = g * 2 + half
                        for dt in range(DT):
                            nc.tensor.matmul(
                                pv[:, half * M:(half + 1) * M],
                                ht[:, dt, st * 128:(st + 1) * 128],
                                w_sbs["v"][:, dt, :],
                                start=(dt == 0), stop=(dt == DT - 1))
                    t0 = (c * 4 + g * 2) * M
                    nc.vector.tensor_copy(vn_sb[:, t0:t0 + 2 * M], pv[:])

                # prefetch next chunk's h^T + rope slices while computing
                if c + 1 < NCHUNK:
                    ht = load_ht(c + 1)
                    cs_cur = load_cs(c + 1)

                # ---- causal attention, both heads merged into ONE pair
                # pipeline: head 1's scores/exps produce while head 0's tail
                # drains, so the exp stream (the attention pacer) never
                # bubbles at the head transition.  Two j-tiles' scores land
                # in one 2-bank [128, 1024] PSUM tile and a single wide exp
                # covers both (halves ACT per-instruction overhead).
                # Softmax denominator: full pairs fold on the DVE as a bf16
                # pairwise tree; only the 4 diagonal tiles + one final tree
                # reduction hit the PE.
                jmax = 4 * c + 3
                npair = (jmax + 1) // 2
                PLAG = 2
                ats = []
                st8 = [{"o": None, "z": None, "zstack": {}}
                       for _ in range(HPC)]
                pairs = {}

                def tree_push(head, t, lvl):
                    zstack = st8[head]["zstack"]
                    while lvl in zstack:
                        prev = zstack.pop(lvl)
                        nt = pzt.tile([128, CHUNK], BF16, name="zt")
                        nc.vector.tensor_tensor(out=nt[:], in0=prev[:],
                                                in1=t[:], op=ALU.add)
                        t = nt
                        lvl += 1
                    zstack[lvl] = t

                def produce(head, pi):
                    spp = psum.tile([128, 2 * CHUNK], FP32,
                                    name="spp", tag="main")
                    ptp = ppt.tile([128, 2 * CHUNK], BF16, name="ptp")
                    for h2 in range(2):
                        j = 2 * pi + h2
                        off = h2 * CHUNK + max(j - 4 * c, 0) * 128
                        nc.tensor.matmul(
                            spp[:, off:(h2 + 1) * CHUNK],
                            kt_sb[head][:, j * 128:(j + 1) * 128],
                            qts[head][:, off - h2 * CHUNK:],
                            start=True, stop=True)
                    lo = max(2 * pi - 4 * c, 0) * 128
                    nc.scalar.activation(ptp[:, lo:], spp[:, lo:], AF.Exp,
                                         scale=float(SM_SCALE))
                    for h2 in range(2):
                        j = 2 * pi + h2
                        r = j - 4 * c
                        if r >= 0:
                            off = h2 * CHUNK + r * 128
                            nc.vector.tensor_tensor(
                                out=ptp[:, off:off + 128],
                                in0=ptp[:, off:off + 128],
                                in1=tri_sb[:], op=ALU.mult)
                    pairs[(head, pi)] = ptp

                def consume(head, i):
                    s = st8[head]
                    if s["o"] is None:
                        s["o"] = psum.tile([128, CHUNK], FP32, name="o_acc",
                                           tag="o", bufs=1)
                    ptp = pairs.pop((head, i))
                    for h2 in range(2):
                        j = 2 * i + h2
                        off = h2 * CHUNK + max(j - 4 * c, 0) * 128
                        nc.tensor.matmul(
                            s["o"][:, off - h2 * CHUNK:],
                            vn_sb[:, j * M + head * HD:
                                  j * M + (head + 1) * HD],
                            ptp[:, off:(h2 + 1) * CHUNK],
                            start=(j == 0), stop=(j == jmax),
                            skip_group_check=True)
                    if 2 * i + 1 < 4 * c:
                        # full pair: fold its two halves, push at level 1
                        # (each tree node covers 2 tiles)
                        nt = pzt.tile([128, CHUNK], BF16, name="zt")
                        nc.vector.tensor_tensor(
                            out=nt[:], in0=ptp[:, 0:CHUNK],
                            in1=ptp[:, CHUNK:], op=ALU.add)
                        tree_push(head, nt, 1)
                    else:
                        if s["z"] is None:
                            s["z"] = psum.tile([128, CHUNK], FP32,
                                               name="z_acc", tag="z",
                                               bufs=1)
                        for h2 in range(2):
                            j = 2 * i + h2
                            off = h2 * CHUNK + max(j - 4 * c, 0) * 128
                            nc.tensor.matmul(
                                s["z"][:, off - h2 * CHUNK:],
                                ones_sb[:],
                                ptp[:, off:(h2 + 1) * CHUNK],
                                start=(j == 4 * c),
                                stop=(j == jmax and c == 0),
                                skip_group_check=True)

                def finish(head):
                    s = st8[head]
                    if c > 0:
                        # collapse the tree and reduce over partitions with
                        # one ones-matmul accumulated into the z bank
                        parts = [s["zstack"][l] for l in sorted(s["zstack"])]
                        zt = parts[0]
                        for nxt in parts[1:]:
                            nt = pzt.tile([128, CHUNK], BF16, name="zt")
                            nc.vector.tensor_tensor(out=nt[:], in0=zt[:],
                                                    in1=nxt[:], op=ALU.add)
                            zt = nt
                        nc.tensor.matmul(
                            s["z"][:], ones_sb[:], zt[:],
                            start=False, stop=True, skip_group_check=True)
                    rz = prz.tile([128, CHUNK], FP32, name="rz")
                    nc.vector.reciprocal_approx_fast(out=rz[:],
                                                     in_=s["z"][:])
                    at = pat.tile([128, CHUNK], BF16, name=f"at{head}")
                    nc.vector.tensor_tensor(out=at[:], in0=s["o"][:],
                                            in1=rz[:], op=ALU.mult)
                    ats.append(at)

                for gp in range(2 * npair + PLAG):
                    if gp < npair:
                        produce(0, gp)
                    elif gp < 2 * npair:
                        produce(1, gp - npair)
                    cg = gp - PLAG
                    if cg >= 0:
                        if cg < npair:
                            consume(0, cg)
                        else:
                            consume(1, cg - npair)
                    if cg == npair - 1:
                        finish(0)
                if ats_prev is not None:
                    outproj(ats_prev, c - 1)
                finish(1)
                ats_prev = ats
            outproj(ats_prev, NCHUNK - 1, spread=True)
    nc.finalize()
    return nc


def _host_prep(xs, norm_w, wq, wk, wv, wo):
    """Fold RMSNorm into h^T upload + weights; build rope tables; bf16."""
    import ml_dtypes
    BF = ml_dtypes.bfloat16

    x64 = xs.astype(np.float64)
    istd = 1.0 / np.sqrt((x64 * x64).mean(axis=1) + EPS)    # [S]
    ht = (x64 * istd[:, None]).T.astype(BF)                 # [D, S]
    # partition-major pack: ht_pm[p, dt, s] = ht[dt*128+p, s]
    ht_pm = np.ascontiguousarray(
        ht.reshape(DT, 128, SEQ).transpose(1, 0, 2))

    def pack_w(w):  # [D, M] -> [128, DT*M] partition-major
        return np.ascontiguousarray(
            w.reshape(DT, 128, M).transpose(1, 0, 2).reshape(128, DT * M))

    nw = norm_w.astype(np.float32)[:, None, None]
    perm = np.concatenate([np.arange(0, HD, 2), np.arange(1, HD, 2)])
    wq_p = (wq * nw)[:, :, perm]
    wk_p = (wk * nw)[:, :, perm]

    inv_freq = 1.0 / (ROPE_BASE ** (np.arange(0, HD, 2, dtype=np.float32) / HD))
    pos = np.arange(SEQ, dtype=np.float32)
    ang = pos[:, None] * inv_freq[None, :]          # [S, 64]
    cos_t = np.cos(ang).T.astype(np.float32)        # [64, S]
    sin_t = np.sin(ang).T.astype(np.float32)
    cosd = np.ascontiguousarray(np.concatenate([cos_t, cos_t], 0))
    sind = np.ascontiguousarray(np.concatenate([sin_t, sin_t], 0))

    tri = np.triu(np.ones((128, 128), dtype=np.float32)).astype(BF)
    onesm = np.ones((128, 128), dtype=BF)

    common = {
        "ht": ht_pm,
        "cosd": cosd,
        "sind": sind,
        "tri": np.ascontiguousarray(tri),
        "ones": onesm,
    }
    in_maps = []
    for core in range(NCORES):
        sl = slice(core * HPC, (core + 1) * HPC)
        wot = np.transpose(wo[:, sl, :], (2, 1, 0)).reshape(128, HPC * D)
        in_maps.append({
            **common,
            "wq": pack_w(wq_p[:, sl, :].reshape(D, M).astype(BF)),
            "wk": pack_w(wk_p[:, sl, :].reshape(D, M).astype(BF)),
            "wv": pack_w((wv * nw)[:, sl, :].reshape(D, M).astype(BF)),
            "wot": np.ascontiguousarray(wot.astype(BF)),
        })
    return in_maps


def kernel(xs, norm_w, wq, wk, wv, wo):
    trace = bool(int(os.environ.get("KERNEL_TRACE", "0")))
    if trace:
        _inject_ntff_hook()
    from concourse.bass_utils import run_bass_kernel_spmd

    nc = _build_nc()
    in_maps = _host_prep(np.asarray(xs), np.asarray(norm_w), np.asarray(wq),
                         np.asarray(wk), np.asarray(wv), np.asarray(wo))
    try:
        res = run_bass_kernel_spmd(nc, in_maps, core_ids=list(range(NCORES)),
                                   trace=trace)
    except Exception:
        # transient device wedge (NRT_EXEC_UNIT_UNRECOVERABLE) recovers on
        # a fresh attempt; rebuild and retry once
        import time
        time.sleep(15)
        nc = _build_nc()
        res = run_bass_kernel_spmd(nc, in_maps, core_ids=list(range(NCORES)),
                                   trace=trace)
    if trace and res.exec_time_ns is not None:
        print(f"HW exec time: {res.exec_time_ns} ns")
    acc = np.zeros((SEQ, D), dtype=np.float64)
    for r in res.results:
        acc += r["out"].astype(np.float64)
    return acc.astype(np.float32)


if __name__ == "__main__":
    rng = np.random.default_rng(0)
    scale = 1.0 / np.sqrt(D)
    inputs = {
        "xs": rng.standard_normal((SEQ, D), dtype=np.float32),
        "norm_w": np.ones((D,), np.float32),
        "wq": rng.standard_normal((D, NH, HD), dtype=np.float32) * scale,
        "wk": rng.standard_normal((D, NH, HD), dtype=np.float32) * scale,
        "wv": rng.standard_normal((D, NH, HD), dtype=np.float32) * scale,
        "wo": rng.standard_normal((D, NH, HD), dtype=np.float32) * scale,
    }
    out = kernel(**inputs)
    print(out.shape, out.dtype, float(np.abs(out).max()))


# revision 39
# speedup vs baseline: 1.0886x; 1.0046x over previous
"""Trainium2 Bass kernel: RMSNorm + RoPE + causal attention + output projection.

Tensor-parallel over heads: 16 heads / 8 cores = 2 heads per core.
Each core computes a full [S, D] partial output (its heads' contribution to
the 'snh,dnh->sd' projection); the all-reduce is done host-side in the gather.

Fused streaming design (bf16, ~436 us vs 674 us baseline):
  - Host prep uploads the RMSNorm'd activations already transposed and
    partition-major packed (h^T as [128, DT, S] bf16) plus bf16 weights,
    a pre-transposed wo^T, and fp32 RoPE tables.  No PE transposes, no
    DRAM scratch roundtrip, and every DMA is a cheap contiguous pattern
    (strided DIRECT2D issue cost was multi-us per descriptor program).
  - Single pass over 8 q-chunks of 512: QK projections (+RoPE) append to
    per-head K^T in SBUF, V is projected directly into natural [t, hd]
    layout (ht-tile stationary), then causal attention for the chunk runs
    against all K/V tiles so far.  The previous chunk's output projection
    is emitted between the two heads' attention so its matmuls cover the
    softmax recip+normalize latency, and the next chunk's h^T/rope-slice
    DMAs prefetch under the current chunk's compute.
  - Scores are computed transposed (S^T[t, s]) at PAIR granularity: two
    j-tiles' scores land in one 2-bank [128, 1024] PSUM tile and a single
    wide exp covers both, halving ACT per-instruction overhead (exp is
    the attention pacer at ~507 ns/j; PE runs ~430 ns/j underneath).
    Softmax denominators: full pairs fold on the DVE as a bf16 pairwise
    tree (~436 ns/add, inside the exp shadow); only the 4 diagonal tiles
    + one final tree reduction hit the PE as ones-matmuls.  PV/Z lag
    scores/exp by 2 pairs so a late exp never stalls the in-order PE
    queue; reciprocal_approx_fast instead of the 3.4us DVE reciprocal.
  - PSUM (16 KiB fully allocated): main 2x2-bank slots (proj pp / score
    pairs / V accum), aux 2 banks (rope ps / outproj op), o 1 bank (PV
    accum), z 1 bank (denominator).  The final chunk's outproj spreads
    over main+aux.
  - Startup: priority-ordered sync-queue DMA FIFO + 100 junk matmuls to
    hold the PE's HAM clock warm while the first loads land.
"""
import os
import sys
import types

import numpy as np

SEQ, D, NH, HD = 4096, 2048, 16, 128
NCORES = 8
HPC = NH // NCORES          # heads per core
M = HPC * HD                # per-core fused head dim (256)
EPS = 1e-6
ROPE_BASE = 10000.0
SM_SCALE = 1.0 / np.sqrt(HD)
CHUNK = 512                 # q-chunk
NCHUNK = SEQ // CHUNK       # 8
NT = SEQ // 128             # 32 s-tiles
DT = D // 128               # 16 d-tiles
LAG = 2


def _inject_ntff_hook():
    """Register the axon NTFF profiling hook (missing antenv.axon_hooks)."""
    if "antenv.axon_hooks" in sys.modules:
        return
    try:
        import antenv
        from trn_agent_boot.trn_boot import _ntff_profile_via_ctypes
    except ImportError:
        return
    holder = [None]
    mod = types.ModuleType("antenv.axon_hooks")
    mod.set_axon_ntff_profile_hook = lambda h: holder.__setitem__(0, h)
    mod.get_axon_ntff_profile_hook = lambda: holder[0]
    sys.modules["antenv.axon_hooks"] = mod
    antenv.axon_hooks = mod
    try:
        mod.set_axon_ntff_profile_hook(
            _ntff_profile_via_ctypes("/opt/axon/libaxon_pjrt.so"))
    except Exception:
        pass


def _build_nc():
    import concourse.bass as bass  # noqa: F401
    import concourse.mybir as mybir
    import concourse.tile as tile
    from concourse import bacc

    FP32 = mybir.dt.float32
    BF16 = mybir.dt.bfloat16
    AF = mybir.ActivationFunctionType
    ALU = mybir.AluOpType

    nc = bacc.Bacc(None, target_bir_lowering=False)

    ht_d = nc.declare_dram_parameter("ht", [128, DT, SEQ], BF16,
                                     isOutput=False)
    wq = nc.declare_dram_parameter("wq", [128, DT * M], BF16, isOutput=False)
    wk = nc.declare_dram_parameter("wk", [128, DT * M], BF16, isOutput=False)
    wv = nc.declare_dram_parameter("wv", [128, DT * M], BF16, isOutput=False)
    wot_d = nc.declare_dram_parameter("wot", [128, HPC * D], BF16,
                                      isOutput=False)
    cosd = nc.declare_dram_parameter("cosd", [128, SEQ], FP32, isOutput=False)
    sind = nc.declare_dram_parameter("sind", [128, SEQ], FP32, isOutput=False)
    tri = nc.declare_dram_parameter("tri", [128, 128], BF16, isOutput=False)
    ones = nc.declare_dram_parameter("ones", [128, 128], BF16, isOutput=False)
    out = nc.declare_dram_parameter("out", [SEQ, D], BF16, isOutput=True)

    with tile.TileContext(nc) as tc:
        with tc.tile_pool(name="consts", bufs=1) as consts, \
             tc.tile_pool(name="pht", bufs=2) as pht, \
             tc.tile_pool(name="pqt", bufs=4) as pqt, \
             tc.tile_pool(name="ppc", bufs=2) as ppc, \
             tc.tile_pool(name="ppt", bufs=6) as ppt, \
             tc.tile_pool(name="prz", bufs=2) as prz, \
             tc.tile_pool(name="pat", bufs=4) as pat, \
             tc.tile_pool(name="post", bufs=3) as post, \
             tc.tile_pool(name="pcs", bufs=2) as pcs, \
             tc.tile_pool(name="pzt", bufs=8) as pzt, \
             tc.tile_pool(name="psum", bufs=2, space="PSUM") as psum:
            kt_sb = [consts.tile([128, SEQ], BF16, name=f"kt{h}")
                     for h in range(HPC)]
            # V natural, packed per t-tile: vn[p, jt*M + m] = V[jt*128+p, m]
            vn_sb = consts.tile([128, NT * M], BF16)

            def load_ht(c):
                cs = slice(c * CHUNK, (c + 1) * CHUNK)
                ht = pht.tile([128, DT, CHUNK], BF16, name="ht")
                for dt8 in range(2):
                    nc.sync.dma_start(
                        out=ht[:, dt8 * 8:(dt8 + 1) * 8, :],
                        in_=ht_d[:, dt8 * 8:(dt8 + 1) * 8, cs])
                return ht

            def load_cs(c):
                cs = slice(c * CHUNK, (c + 1) * CHUNK)
                sin_t = pcs.tile([128, CHUNK], FP32, name="sin_t", tag="sin")
                nc.sync.dma_start(out=sin_t[:], in_=sind[:, cs])
                cos_t = pcs.tile([128, CHUNK], FP32, name="cos_t", tag="cos")
                nc.sync.dma_start(out=cos_t[:], in_=cosd[:, cs])
                return cos_t, sin_t

            # startup: one sync-queue FIFO in priority order — ring
            # bandwidth serves the first projections' pieces (ht, wq,
            # rope slices) before the bulk loads behind them.
            w_sbs = {k: consts.tile([128, DT, M], BF16, name=f"w{k}_sb")
                     for k in ("q", "k", "v")}
            cs0 = slice(0, CHUNK)
            ht = pht.tile([128, DT, CHUNK], BF16, name="ht")
            for q4 in range(4):
                dts = slice(q4 * 4, (q4 + 1) * 4)
                nc.sync.dma_start(out=ht[:, dts, :], in_=ht_d[:, dts, cs0])
                nc.sync.dma_start(
                    out=w_sbs["q"][:, dts, :].rearrange("p t m -> p (t m)"),
                    in_=wq[:, q4 * 4 * M:(q4 + 1) * 4 * M])
                if q4 == 1:
                    cs_cur = load_cs(0)
            nc.sync.dma_start(
                out=w_sbs["k"][:].rearrange("p t m -> p (t m)"), in_=wk[:])
            nc.sync.dma_start(
                out=w_sbs["v"][:].rearrange("p t m -> p (t m)"), in_=wv[:])
            tri_sb = consts.tile([128, 128], BF16)
            nc.sync.dma_start(out=tri_sb[:], in_=tri[:])
            ones_sb = consts.tile([128, 128], BF16)
            nc.sync.dma_start(out=ones_sb[:], in_=ones[:])
            wot_sb = consts.tile([128, HPC * D], BF16)
            nc.sync.dma_start(out=wot_sb[:], in_=wot_d[:])

            # HAM warm-up: keep the PE busy on junk matmuls while the
            # startup DMAs land, so real matmuls start at the warm clock.
            warm = consts.tile([128, 128], BF16)
            nc.vector.memset(warm[:], 0.0)
            for i in range(100):
                wp = psum.tile([128, CHUNK], FP32, name="wp", tag="main")
                nc.tensor.matmul(wp[:, 0:128], warm[:], warm[:],
                                 start=True, stop=True)

            def outproj(ats, c, spread=False):
                for st in range(4):
                    ost = post.tile([128, D], BF16, name="ost")
                    for dc in range(4):
                        # the final chunk's projection has nothing after it
                        # to hide evac latency: spread over two tag rings so
                        # 4 banks ping-pong instead of 2.
                        tag = "main" if spread and dc % 2 else "aux"
                        op = psum.tile([128, CHUNK], FP32, name="op",
                                       tag=tag)
                        nc.tensor.matmul(
                            op[:],
                            ats[0][:, st * 128:(st + 1) * 128],
                            wot_sb[:, dc * CHUNK:(dc + 1) * CHUNK],
                            start=True, stop=False)
                        nc.tensor.matmul(
                            op[:],
                            ats[1][:, st * 128:(st + 1) * 128],
                            wot_sb[:, D + dc * CHUNK:D + (dc + 1) * CHUNK],
                            start=False, stop=True)
                        if dc % 2 == 0:
                            nc.scalar.activation(
                                ost[:, dc * CHUNK:(dc + 1) * CHUNK],
                                op[:], AF.Copy)
                        else:
                            nc.vector.tensor_copy(
                                ost[:, dc * CHUNK:(dc + 1) * CHUNK], op[:])
                    row = (c * 4 + st) * 128
                    eng = nc.sync if st % 2 == 0 else nc.scalar
                    eng.dma_start(out=out[row:row + 128, :], in_=ost[:])

            ats_prev = None
            for c in range(NCHUNK):
                cs = slice(c * CHUNK, (c + 1) * CHUNK)
                cos_t, sin_t = cs_cur

                # ---- QK projections + RoPE
                qts = []
                for head in range(HPC):
                    for kind in ("q", "k"):
                        w_sb = w_sbs[kind]
                        pp = psum.tile([128, CHUNK], FP32, name="pp",
                                       tag="main")
                        for dt in range(DT):
                            nc.tensor.matmul(
                                pp[:],
                                w_sb[:, dt, head * HD:(head + 1) * HD],
                                ht[:, dt, :],
                                start=(dt == 0), stop=(dt == DT - 1))
                        ps = psum.tile([128, CHUNK], FP32, name="ps",
                                       tag="aux")
                        nc.vector.tensor_tensor(
                            out=ps[:], in0=pp[:], in1=sin_t[:],
                            op=ALU.mult)
                        pc = ppc.tile([128, CHUNK], FP32, name="pc")
                        nc.vector.tensor_tensor(
                            out=pc[:], in0=pp[:], in1=cos_t[:],
                            op=ALU.mult)
                        if kind == "q":
                            dst = pqt.tile([128, CHUNK], BF16,
                                           name=f"qt{head}")
                            qts.append(dst)
                            d0, d1 = dst[0:64, :], dst[64:128, :]
                        else:
                            d0 = kt_sb[head][0:64, cs]
                            d1 = kt_sb[head][64:128, cs]
                        nc.vector.tensor_tensor(
                            out=d0, in0=pc[0:64, :], in1=ps[64:128, :],
                            op=ALU.subtract)
                        nc.vector.tensor_tensor(
                            out=d1, in0=pc[64:128, :], in1=ps[0:64, :],
                            op=ALU.add)

                # ---- V projection directly into natural [t, m] layout.
                # Emitted as thunks: the leading 64-2c matmuls go here, the
                # last 2c stream one-per-pair-slot into the attention
                # pipeline below, filling the PE slack under the exp pacing
                # (the aux ring is idle during attention).
                ht_c, c_ = ht, c
                v_mms = [(g, half, dt)
                         for g in range(2)
                         for half in range(2)
                         for dt in range(DT)]
                pv_tiles = {}

                def emit_v(n):
                    for _ in range(n):
                        if not v_mms:
                            return
                        g, half, dt = v_mms.pop(0)
                        if g not in pv_tiles:
                            pv_tiles[g] = psum.tile([128, CHUNK], FP32,
                                                    name="pv", tag="aux")
                        st = g * 2 + half
                        nc.tensor.matmul(
                            pv_tiles[g][:, half * M:(half + 1) * M],
                            ht_c[:, dt, st * 128:(st + 1) * 128],
                            w_sbs["v"][:, dt, :],
                            start=(dt == 0), stop=(dt == DT - 1))
                        if half == 1 and dt == DT - 1:
                            t0 = (c_ * 4 + g * 2) * M
                            nc.vector.tensor_copy(
                                vn_sb[:, t0:t0 + 2 * M], pv_tiles[g][:])
                emit_v(64 - 2 * c)

                # prefetch next chunk's h^T + rope slices while computing
                if c + 1 < NCHUNK:
                    ht = load_ht(c + 1)
                    cs_cur = load_cs(c + 1)

                # ---- causal attention, both heads merged into ONE pair
                # pipeline: head 1's scores/exps produce while head 0's tail
                # drains, so the exp stream (the attention pacer) never
                # bubbles at the head transition.  Two j-tiles' scores land
                # in one 2-bank [128, 1024] PSUM tile and a single wide exp
                # covers both (halves ACT per-instruction overhead).
                # Softmax denominator: full pairs fold on the DVE as a bf16
                # pairwise tree; only the 4 diagonal tiles + one final tree
                # reduction hit the PE.
                jmax = 4 * c + 3
                npair = (jmax + 1) // 2
                PLAG = 2
                ats = []
                st8 = [{"o": None, "z": None, "zstack": {}}
                       for _ in range(HPC)]
                pairs = {}

                def tree_push(head, t, lvl):
                    zstack = st8[head]["zstack"]
                    while lvl in zstack:
                        prev = zstack.pop(lvl)
                        nt = pzt.tile([128, CHUNK], BF16, name="zt")
                        nc.vector.tensor_tensor(out=nt[:], in0=prev[:],
                                                in1=t[:], op=ALU.add)
                        t = nt
                        lvl += 1
                    zstack[lvl] = t

                def produce(head, pi):
                    spp = psum.tile([128, 2 * CHUNK], FP32,
                                    name="spp", tag="main")
                    ptp = ppt.tile([128, 2 * CHUNK], BF16, name="ptp")
                    for h2 in range(2):
                        j = 2 * pi + h2
                        off = h2 * CHUNK + max(j - 4 * c, 0) * 128
                        nc.tensor.matmul(
                            spp[:, off:(h2 + 1) * CHUNK],
                            kt_sb[head][:, j * 128:(j + 1) * 128],
                            qts[head][:, off - h2 * CHUNK:],
                            start=True, stop=True)
                    lo = max(2 * pi - 4 * c, 0) * 128
                    nc.scalar.activation(ptp[:, lo:], spp[:, lo:], AF.Exp,
                                         scale=float(SM_SCALE))
                    for h2 in range(2):
                        j = 2 * pi + h2
                        r = j - 4 * c
                        if r >= 0:
                            off = h2 * CHUNK + r * 128
                            nc.vector.tensor_tensor(
                                out=ptp[:, off:off + 128],
                                in0=ptp[:, off:off + 128],
                                in1=tri_sb[:], op=ALU.mult)
                    pairs[(head, pi)] = ptp

                def consume(head, i):
                    s = st8[head]
                    if s["o"] is None:
                        s["o"] = psum.tile([128, CHUNK], FP32, name="o_acc",
                                           tag="o", bufs=1)
                    ptp = pairs.pop((head, i))
                    for h2 in range(2):
                        j = 2 * i + h2
                        off = h2 * CHUNK + max(j - 4 * c, 0) * 128
                        nc.tensor.matmul(
                            s["o"][:, off - h2 * CHUNK:],
                            vn_sb[:, j * M + head * HD:
                                  j * M + (head + 1) * HD],
                            ptp[:, off:(h2 + 1) * CHUNK],
                            start=(j == 0), stop=(j == jmax),
                            skip_group_check=True)
                    if 2 * i + 1 < 4 * c:
                        # full pair: fold its two halves, push at level 1
                        # (each tree node covers 2 tiles)
                        nt = pzt.tile([128, CHUNK], BF16, name="zt")
                        nc.vector.tensor_tensor(
                            out=nt[:], in0=ptp[:, 0:CHUNK],
                            in1=ptp[:, CHUNK:], op=ALU.add)
                        tree_push(head, nt, 1)
                    else:
                        if s["z"] is None:
                            s["z"] = psum.tile([128, CHUNK], FP32,
                                               name="z_acc", tag="z",
                                               bufs=1)
                        for h2 in range(2):
                            j = 2 * i + h2
                            off = h2 * CHUNK + max(j - 4 * c, 0) * 128
                            nc.tensor.matmul(
                                s["z"][:, off - h2 * CHUNK:],
                                ones_sb[:],
                                ptp[:, off:(h2 + 1) * CHUNK],
                                start=(j == 4 * c),
                                stop=(j == jmax and c == 0),
                                skip_group_check=True)

                def finish(head):
                    s = st8[head]
                    if c > 0:
                        # collapse the tree and reduce over partitions with
                        # one ones-matmul accumulated into the z bank
                        parts = [s["zstack"][l] for l in sorted(s["zstack"])]
                        zt = parts[0]
                        for nxt in parts[1:]:
                            nt = pzt.tile([128, CHUNK], BF16, name="zt")
                            nc.vector.tensor_tensor(out=nt[:], in0=zt[:],
                                                    in1=nxt[:], op=ALU.add)
                            zt = nt
                        nc.tensor.matmul(
                            s["z"][:], ones_sb[:], zt[:],
                            start=False, stop=True, skip_group_check=True)
                    rz = prz.tile([128, CHUNK], FP32, name="rz")
                    nc.vector.reciprocal_approx_fast(out=rz[:],
                                                     in_=s["z"][:])
                    at = pat.tile([128, CHUNK], BF16, name=f"at{head}")
                    nc.vector.tensor_tensor(out=at[:], in0=s["o"][:],
                                            in1=rz[:], op=ALU.mult)
                    ats.append(at)

                for gp in range(2 * npair + PLAG):
                    if gp < npair:
                        produce(0, gp)
                    elif gp < 2 * npair:
                        produce(1, gp - npair)
                    emit_v(1)
                    cg = gp - PLAG
                    if cg >= 0:
                        if cg < npair:
                            consume(0, cg)
                        else:
                            consume(1, cg - npair)
                    if cg == npair - 1:
                        finish(0)
                if ats_prev is not None:
                    outproj(ats_prev, c - 1)
                finish(1)
                ats_prev = ats
            outproj(ats_prev, NCHUNK - 1, spread=True)
    nc.finalize()
    return nc


def _host_prep(xs, norm_w, wq, wk, wv, wo):
    """Fold RMSNorm into h^T upload + weights; build rope tables; bf16."""
    import ml_dtypes
    BF = ml_dtypes.bfloat16

    x64 = xs.astype(np.float64)
    istd = 1.0 / np.sqrt((x64 * x64).mean(axis=1) + EPS)    # [S]
    ht = (x64 * istd[:, None]).T.astype(BF)                 # [D, S]
    # partition-major pack: ht_pm[p, dt, s] = ht[dt*128+p, s]
    ht_pm = np.ascontiguousarray(
        ht.reshape(DT, 128, SEQ).transpose(1, 0, 2))

    def pack_w(w):  # [D, M] -> [128, DT*M] partition-major
        return np.ascontiguousarray(
            w.reshape(DT, 128, M).transpose(1, 0, 2).reshape(128, DT * M))

    nw = norm_w.astype(np.float32)[:, None, None]
    perm = np.concatenate([np.arange(0, HD, 2), np.arange(1, HD, 2)])
    wq_p = (wq * nw)[:, :, perm]
    wk_p = (wk * nw)[:, :, perm]

    inv_freq = 1.0 / (ROPE_BASE ** (np.arange(0, HD, 2, dtype=np.float32) / HD))
    pos = np.arange(SEQ, dtype=np.float32)
    ang = pos[:, None] * inv_freq[None, :]          # [S, 64]
    cos_t = np.cos(ang).T.astype(np.float32)        # [64, S]
    sin_t = np.sin(ang).T.astype(np.float32)
    cosd = np.ascontiguousarray(np.concatenate([cos_t, cos_t], 0))
    sind = np.ascontiguousarray(np.concatenate([sin_t, sin_t], 0))

    tri = np.triu(np.ones((128, 128), dtype=np.float32)).astype(BF)
    onesm = np.ones((128, 128), dtype=BF)

    common = {
        "ht": ht_pm,
        "cosd": cosd,
        "sind": sind,
        "tri": np.ascontiguousarray(tri),
        "ones": onesm,
    }
    in_maps = []
    for core in range(NCORES):
        sl = slice(core * HPC, (core + 1) * HPC)
        wot = np.transpose(wo[:, sl, :], (2, 1, 0)).reshape(128, HPC * D)
        in_maps.append({
            **common,
            "wq": pack_w(wq_p[:, sl, :].reshape(D, M).astype(BF)),
            "wk": pack_w(wk_p[:, sl, :].reshape(D, M).astype(BF)),
            "wv": pack_w((wv * nw)[:, sl, :].reshape(D, M).astype(BF)),
            "wot": np.ascontiguousarray(wot.astype(BF)),
        })
    return in_maps


def kernel(xs, norm_w, wq, wk, wv, wo):
    trace = bool(int(os.environ.get("KERNEL_TRACE", "0")))
    if trace:
        _inject_ntff_hook()
    from concourse.bass_utils import run_bass_kernel_spmd

    nc = _build_nc()
    in_maps = _host_prep(np.asarray(xs), np.asarray(norm_w), np.asarray(wq),
                         np.asarray(wk), np.asarray(wv), np.asarray(wo))
    try:
        res = run_bass_kernel_spmd(nc, in_maps, core_ids=list(range(NCORES)),
                                   trace=trace)
    except Exception:
        # transient device wedge (NRT_EXEC_UNIT_UNRECOVERABLE) recovers on
        # a fresh attempt; rebuild and retry once
        import time
        time.sleep(15)
        nc = _build_nc()
        res = run_bass_kernel_spmd(nc, in_maps, core_ids=list(range(NCORES)),
                                   trace=trace)
    if trace and res.exec_time_ns is not None:
        print(f"HW exec time: {res.exec_time_ns} ns")
    acc = np.zeros((SEQ, D), dtype=np.float64)
    for r in res.results:
        acc += r["out"].astype(np.float64)
    return acc.astype(np.float32)


if __name__ == "__main__":
    rng = np.random.default_rng(0)
    scale = 1.0 / np.sqrt(D)
    inputs = {
        "xs": rng.standard_normal((SEQ, D), dtype=np.float32),
        "norm_w": np.ones((D,), np.float32),
        "wq": rng.standard_normal((D, NH, HD), dtype=np.float32) * scale,
        "wk": rng.standard_normal((D, NH, HD), dtype=np.float32) * scale,
        "wv": rng.standard_normal((D, NH, HD), dtype=np.float32) * scale,
        "wo": rng.standard_normal((D, NH, HD), dtype=np.float32) * scale,
    }
    out = kernel(**inputs)
    print(out.shape, out.dtype, float(np.abs(out).max()))


# revision 40
# speedup vs baseline: 1.0975x; 1.0082x over previous
"""Trainium2 Bass kernel: RMSNorm + RoPE + causal attention + output projection.

Tensor-parallel over heads: 16 heads / 8 cores = 2 heads per core.
Each core computes a full [S, D] partial output (its heads' contribution to
the 'snh,dnh->sd' projection); the all-reduce is done host-side in the gather.

Fused streaming design (bf16, ~436 us vs 674 us baseline):
  - Host prep uploads the RMSNorm'd activations already transposed and
    partition-major packed (h^T as [128, DT, S] bf16) plus bf16 weights,
    a pre-transposed wo^T, and fp32 RoPE tables.  No PE transposes, no
    DRAM scratch roundtrip, and every DMA is a cheap contiguous pattern
    (strided DIRECT2D issue cost was multi-us per descriptor program).
  - Single pass over 8 q-chunks of 512: QK projections (+RoPE) append to
    per-head K^T in SBUF, V is projected directly into natural [t, hd]
    layout (ht-tile stationary), then causal attention for the chunk runs
    against all K/V tiles so far.  The previous chunk's output projection
    is emitted between the two heads' attention so its matmuls cover the
    softmax recip+normalize latency, and the next chunk's h^T/rope-slice
    DMAs prefetch under the current chunk's compute.
  - Scores are computed transposed (S^T[t, s]) at PAIR granularity: two
    j-tiles' scores land in one 2-bank [128, 1024] PSUM tile and a single
    wide exp covers both, halving ACT per-instruction overhead (exp is
    the attention pacer at ~507 ns/j; PE runs ~430 ns/j underneath).
    Softmax denominators: full pairs fold on the DVE as a bf16 pairwise
    tree (~436 ns/add, inside the exp shadow); only the 4 diagonal tiles
    + one final tree reduction hit the PE as ones-matmuls.  PV/Z lag
    scores/exp by 2 pairs so a late exp never stalls the in-order PE
    queue; reciprocal_approx_fast instead of the 3.4us DVE reciprocal.
  - PSUM (16 KiB fully allocated): main 2x2-bank slots (proj pp / score
    pairs / V accum), aux 2 banks (rope ps / outproj op), o 1 bank (PV
    accum), z 1 bank (denominator).  The final chunk's outproj spreads
    over main+aux.
  - Startup: priority-ordered sync-queue DMA FIFO + 100 junk matmuls to
    hold the PE's HAM clock warm while the first loads land.
"""
import os
import sys
import types

import numpy as np

SEQ, D, NH, HD = 4096, 2048, 16, 128
NCORES = 8
HPC = NH // NCORES          # heads per core
M = HPC * HD                # per-core fused head dim (256)
EPS = 1e-6
ROPE_BASE = 10000.0
SM_SCALE = 1.0 / np.sqrt(HD)
CHUNK = 512                 # q-chunk
NCHUNK = SEQ // CHUNK       # 8
NT = SEQ // 128             # 32 s-tiles
DT = D // 128               # 16 d-tiles
LAG = 2


def _inject_ntff_hook():
    """Register the axon NTFF profiling hook (missing antenv.axon_hooks)."""
    if "antenv.axon_hooks" in sys.modules:
        return
    try:
        import antenv
        from trn_agent_boot.trn_boot import _ntff_profile_via_ctypes
    except ImportError:
        return
    holder = [None]
    mod = types.ModuleType("antenv.axon_hooks")
    mod.set_axon_ntff_profile_hook = lambda h: holder.__setitem__(0, h)
    mod.get_axon_ntff_profile_hook = lambda: holder[0]
    sys.modules["antenv.axon_hooks"] = mod
    antenv.axon_hooks = mod
    try:
        mod.set_axon_ntff_profile_hook(
            _ntff_profile_via_ctypes("/opt/axon/libaxon_pjrt.so"))
    except Exception:
        pass


def _build_nc():
    import concourse.bass as bass  # noqa: F401
    import concourse.mybir as mybir
    import concourse.tile as tile
    from concourse import bacc

    FP32 = mybir.dt.float32
    BF16 = mybir.dt.bfloat16
    AF = mybir.ActivationFunctionType
    ALU = mybir.AluOpType

    nc = bacc.Bacc(None, target_bir_lowering=False)

    ht_d = nc.declare_dram_parameter("ht", [128, DT, SEQ], BF16,
                                     isOutput=False)
    wq = nc.declare_dram_parameter("wq", [128, DT * M], BF16, isOutput=False)
    wk = nc.declare_dram_parameter("wk", [128, DT * M], BF16, isOutput=False)
    wv = nc.declare_dram_parameter("wv", [128, DT * M], BF16, isOutput=False)
    wot_d = nc.declare_dram_parameter("wot", [128, HPC * D], BF16,
                                      isOutput=False)
    cosd = nc.declare_dram_parameter("cosd", [128, SEQ], FP32, isOutput=False)
    sind = nc.declare_dram_parameter("sind", [128, SEQ], FP32, isOutput=False)
    tri = nc.declare_dram_parameter("tri", [128, 128], BF16, isOutput=False)
    ones = nc.declare_dram_parameter("ones", [128, 128], BF16, isOutput=False)
    out = nc.declare_dram_parameter("out", [SEQ, D], BF16, isOutput=True)

    with tile.TileContext(nc) as tc:
        with tc.tile_pool(name="consts", bufs=1) as consts, \
             tc.tile_pool(name="pht", bufs=2) as pht, \
             tc.tile_pool(name="pqt", bufs=4) as pqt, \
             tc.tile_pool(name="ppc", bufs=2) as ppc, \
             tc.tile_pool(name="ppt", bufs=6) as ppt, \
             tc.tile_pool(name="prz", bufs=2) as prz, \
             tc.tile_pool(name="pat", bufs=4) as pat, \
             tc.tile_pool(name="post", bufs=3) as post, \
             tc.tile_pool(name="pcs", bufs=2) as pcs, \
             tc.tile_pool(name="pzt", bufs=8) as pzt, \
             tc.tile_pool(name="psum", bufs=2, space="PSUM") as psum:
            kt_sb = [consts.tile([128, SEQ], BF16, name=f"kt{h}")
                     for h in range(HPC)]
            # V natural, packed per t-tile: vn[p, jt*M + m] = V[jt*128+p, m]
            vn_sb = consts.tile([128, NT * M], BF16)

            def load_ht(c):
                cs = slice(c * CHUNK, (c + 1) * CHUNK)
                ht = pht.tile([128, DT, CHUNK], BF16, name="ht")
                for dt8 in range(2):
                    nc.sync.dma_start(
                        out=ht[:, dt8 * 8:(dt8 + 1) * 8, :],
                        in_=ht_d[:, dt8 * 8:(dt8 + 1) * 8, cs])
                return ht

            def load_cs(c):
                cs = slice(c * CHUNK, (c + 1) * CHUNK)
                sin_t = pcs.tile([128, CHUNK], FP32, name="sin_t", tag="sin")
                nc.sync.dma_start(out=sin_t[:], in_=sind[:, cs])
                cos_t = pcs.tile([128, CHUNK], FP32, name="cos_t", tag="cos")
                nc.sync.dma_start(out=cos_t[:], in_=cosd[:, cs])
                return cos_t, sin_t

            # startup: one sync-queue FIFO in priority order — ring
            # bandwidth serves the first projections' pieces (ht, wq,
            # rope slices) before the bulk loads behind them.
            w_sbs = {k: consts.tile([128, DT, M], BF16, name=f"w{k}_sb")
                     for k in ("q", "k", "v")}
            cs0 = slice(0, CHUNK)
            ht = pht.tile([128, DT, CHUNK], BF16, name="ht")
            for q4 in range(4):
                dts = slice(q4 * 4, (q4 + 1) * 4)
                nc.sync.dma_start(out=ht[:, dts, :], in_=ht_d[:, dts, cs0])
                nc.sync.dma_start(
                    out=w_sbs["q"][:, dts, :].rearrange("p t m -> p (t m)"),
                    in_=wq[:, q4 * 4 * M:(q4 + 1) * 4 * M])
                if q4 == 1:
                    cs_cur = load_cs(0)
            nc.sync.dma_start(
                out=w_sbs["k"][:].rearrange("p t m -> p (t m)"), in_=wk[:])
            nc.sync.dma_start(
                out=w_sbs["v"][:].rearrange("p t m -> p (t m)"), in_=wv[:])
            tri_sb = consts.tile([128, 128], BF16)
            nc.sync.dma_start(out=tri_sb[:], in_=tri[:])
            ones_sb = consts.tile([128, 128], BF16)
            nc.sync.dma_start(out=ones_sb[:], in_=ones[:])
            wot_sb = consts.tile([128, HPC * D], BF16)
            nc.sync.dma_start(out=wot_sb[:], in_=wot_d[:])

            # HAM warm-up: keep the PE busy on junk matmuls while the
            # startup DMAs land, so real matmuls start at the warm clock.
            warm = consts.tile([128, 128], BF16)
            nc.vector.memset(warm[:], 0.0)
            for i in range(48):
                wp = psum.tile([128, CHUNK], FP32, name="wp", tag="main")
                nc.tensor.matmul(wp[:, 0:128], warm[:], warm[:],
                                 start=True, stop=True)

            def outproj(ats, c, spread=False):
                for st in range(4):
                    ost = post.tile([128, D], BF16, name="ost")
                    for dc in range(4):
                        # the final chunk's projection has nothing after it
                        # to hide evac latency: spread over two tag rings so
                        # 4 banks ping-pong instead of 2.
                        tag = "main" if spread and dc % 2 else "aux"
                        op = psum.tile([128, CHUNK], FP32, name="op",
                                       tag=tag)
                        nc.tensor.matmul(
                            op[:],
                            ats[0][:, st * 128:(st + 1) * 128],
                            wot_sb[:, dc * CHUNK:(dc + 1) * CHUNK],
                            start=True, stop=False)
                        nc.tensor.matmul(
                            op[:],
                            ats[1][:, st * 128:(st + 1) * 128],
                            wot_sb[:, D + dc * CHUNK:D + (dc + 1) * CHUNK],
                            start=False, stop=True)
                        if dc % 2 == 0:
                            nc.scalar.activation(
                                ost[:, dc * CHUNK:(dc + 1) * CHUNK],
                                op[:], AF.Copy)
                        else:
                            nc.vector.tensor_copy(
                                ost[:, dc * CHUNK:(dc + 1) * CHUNK], op[:])
                    row = (c * 4 + st) * 128
                    eng = nc.sync if st % 2 == 0 else nc.scalar
                    eng.dma_start(out=out[row:row + 128, :], in_=ost[:])

            ats_prev = None
            for c in range(NCHUNK):
                cs = slice(c * CHUNK, (c + 1) * CHUNK)
                cos_t, sin_t = cs_cur

                # ---- QK projections + RoPE
                qts = []
                for head in range(HPC):
                    for kind in ("q", "k"):
                        w_sb = w_sbs[kind]
                        pp = psum.tile([128, CHUNK], FP32, name="pp",
                                       tag="main")
                        for dt in range(DT):
                            nc.tensor.matmul(
                                pp[:],
                                w_sb[:, dt, head * HD:(head + 1) * HD],
                                ht[:, dt, :],
                                start=(dt == 0), stop=(dt == DT - 1))
                        ps = psum.tile([128, CHUNK], FP32, name="ps",
                                       tag="aux")
                        nc.vector.tensor_tensor(
                            out=ps[:], in0=pp[:], in1=sin_t[:],
                            op=ALU.mult)
                        pc = ppc.tile([128, CHUNK], FP32, name="pc")
                        nc.vector.tensor_tensor(
                            out=pc[:], in0=pp[:], in1=cos_t[:],
                            op=ALU.mult)
                        if kind == "q":
                            dst = pqt.tile([128, CHUNK], BF16,
                                           name=f"qt{head}")
                            qts.append(dst)
                            d0, d1 = dst[0:64, :], dst[64:128, :]
                        else:
                            d0 = kt_sb[head][0:64, cs]
                            d1 = kt_sb[head][64:128, cs]
                        nc.vector.tensor_tensor(
                            out=d0, in0=pc[0:64, :], in1=ps[64:128, :],
                            op=ALU.subtract)
                        nc.vector.tensor_tensor(
                            out=d1, in0=pc[64:128, :], in1=ps[0:64, :],
                            op=ALU.add)

                # ---- V projection directly into natural [t, m] layout.
                # Emitted as thunks: the leading 64-2c matmuls go here, the
                # last 2c stream one-per-pair-slot into the attention
                # pipeline below, filling the PE slack under the exp pacing
                # (the aux ring is idle during attention).
                ht_c, c_ = ht, c
                v_mms = [(g, half, dt)
                         for g in range(2)
                         for half in range(2)
                         for dt in range(DT)]
                pv_tiles = {}

                def emit_v(n):
                    for _ in range(n):
                        if not v_mms:
                            return
                        g, half, dt = v_mms.pop(0)
                        if g not in pv_tiles:
                            pv_tiles[g] = psum.tile([128, CHUNK], FP32,
                                                    name="pv", tag="aux")
                        st = g * 2 + half
                        nc.tensor.matmul(
                            pv_tiles[g][:, half * M:(half + 1) * M],
                            ht_c[:, dt, st * 128:(st + 1) * 128],
                            w_sbs["v"][:, dt, :],
                            start=(dt == 0), stop=(dt == DT - 1))
                        if half == 1 and dt == DT - 1:
                            t0 = (c_ * 4 + g * 2) * M
                            nc.vector.tensor_copy(
                                vn_sb[:, t0:t0 + 2 * M], pv_tiles[g][:])
                emit_v(64 - 2 * c)

                # prefetch next chunk's h^T + rope slices while computing
                if c + 1 < NCHUNK:
                    ht = load_ht(c + 1)
                    cs_cur = load_cs(c + 1)

                # ---- causal attention, both heads merged into ONE pair
                # pipeline: head 1's scores/exps produce while head 0's tail
                # drains, so the exp stream (the attention pacer) never
                # bubbles at the head transition.  Two j-tiles' scores land
                # in one 2-bank [128, 1024] PSUM tile and a single wide exp
                # covers both (halves ACT per-instruction overhead).
                # Softmax denominator: full pairs fold on the DVE as a bf16
                # pairwise tree; only the 4 diagonal tiles + one final tree
                # reduction hit the PE.
                jmax = 4 * c + 3
                npair = (jmax + 1) // 2
                PLAG = 2
                ats = []
                st8 = [{"o": None, "z": None, "zstack": {}}
                       for _ in range(HPC)]
                pairs = {}

                def tree_push(head, t, lvl):
                    zstack = st8[head]["zstack"]
                    while lvl in zstack:
                        prev = zstack.pop(lvl)
                        nt = pzt.tile([128, CHUNK], BF16, name="zt")
                        nc.vector.tensor_tensor(out=nt[:], in0=prev[:],
                                                in1=t[:], op=ALU.add)
                        t = nt
                        lvl += 1
                    zstack[lvl] = t

                def produce(head, pi):
                    spp = psum.tile([128, 2 * CHUNK], FP32,
                                    name="spp", tag="main")
                    ptp = ppt.tile([128, 2 * CHUNK], BF16, name="ptp")
                    for h2 in range(2):
                        j = 2 * pi + h2
                        off = h2 * CHUNK + max(j - 4 * c, 0) * 128
                        nc.tensor.matmul(
                            spp[:, off:(h2 + 1) * CHUNK],
                            kt_sb[head][:, j * 128:(j + 1) * 128],
                            qts[head][:, off - h2 * CHUNK:],
                            start=True, stop=True)
                    lo = max(2 * pi - 4 * c, 0) * 128
                    nc.scalar.activation(ptp[:, lo:], spp[:, lo:], AF.Exp,
                                         scale=float(SM_SCALE))
                    for h2 in range(2):
                        j = 2 * pi + h2
                        r = j - 4 * c
                        if r >= 0:
                            off = h2 * CHUNK + r * 128
                            nc.vector.tensor_tensor(
                                out=ptp[:, off:off + 128],
                                in0=ptp[:, off:off + 128],
                                in1=tri_sb[:], op=ALU.mult)
                    pairs[(head, pi)] = ptp

                def consume(head, i):
                    s = st8[head]
                    if s["o"] is None:
                        s["o"] = psum.tile([128, CHUNK], FP32, name="o_acc",
                                           tag="o", bufs=1)
                    ptp = pairs.pop((head, i))
                    for h2 in range(2):
                        j = 2 * i + h2
                        off = h2 * CHUNK + max(j - 4 * c, 0) * 128
                        nc.tensor.matmul(
                            s["o"][:, off - h2 * CHUNK:],
                            vn_sb[:, j * M + head * HD:
                                  j * M + (head + 1) * HD],
                            ptp[:, off:(h2 + 1) * CHUNK],
                            start=(j == 0), stop=(j == jmax),
                            skip_group_check=True)
                    if 2 * i + 1 < 4 * c:
                        # full pair: fold its two halves, push at level 1
                        # (each tree node covers 2 tiles)
                        nt = pzt.tile([128, CHUNK], BF16, name="zt")
                        nc.vector.tensor_tensor(
                            out=nt[:], in0=ptp[:, 0:CHUNK],
                            in1=ptp[:, CHUNK:], op=ALU.add)
                        tree_push(head, nt, 1)
                    else:
                        if s["z"] is None:
                            s["z"] = psum.tile([128, CHUNK], FP32,
                                               name="z_acc", tag="z",
                                               bufs=1)
                        for h2 in range(2):
                            j = 2 * i + h2
                            off = h2 * CHUNK + max(j - 4 * c, 0) * 128
                            nc.tensor.matmul(
                                s["z"][:, off - h2 * CHUNK:],
                                ones_sb[:],
                                ptp[:, off:(h2 + 1) * CHUNK],
                                start=(j == 4 * c),
                                stop=(j == jmax and c == 0),
                                skip_group_check=True)

                def finish(head):
                    s = st8[head]
                    if c > 0:
                        # collapse the tree and reduce over partitions with
                        # one ones-matmul accumulated into the z bank
                        parts = [s["zstack"][l] for l in sorted(s["zstack"])]
                        zt = parts[0]
                        for nxt in parts[1:]:
                            nt = pzt.tile([128, CHUNK], BF16, name="zt")
                            nc.vector.tensor_tensor(out=nt[:], in0=zt[:],
                                                    in1=nxt[:], op=ALU.add)
                            zt = nt
                        nc.tensor.matmul(
                            s["z"][:], ones_sb[:], zt[:],
                            start=False, stop=True, skip_group_check=True)
                    rz = prz.tile([128, CHUNK], FP32, name="rz")
                    nc.vector.reciprocal_approx_fast(out=rz[:],
                                                     in_=s["z"][:])
                    at = pat.tile([128, CHUNK], BF16, name=f"at{head}")
                    nc.vector.tensor_tensor(out=at[:], in0=s["o"][:],
                                            in1=rz[:], op=ALU.mult)
                    ats.append(at)

                for gp in range(2 * npair + PLAG):
                    if gp < npair:
                        produce(0, gp)
                    elif gp < 2 * npair:
                        produce(1, gp - npair)
                    emit_v(1)
                    cg = gp - PLAG
                    if cg >= 0:
                        if cg < npair:
                            consume(0, cg)
                        else:
                            consume(1, cg - npair)
                    if cg == npair - 1:
                        finish(0)
                if ats_prev is not None:
                    outproj(ats_prev, c - 1)
                finish(1)
                ats_prev = ats
            outproj(ats_prev, NCHUNK - 1, spread=True)
    nc.finalize()
    return nc


def _host_prep(xs, norm_w, wq, wk, wv, wo):
    """Fold RMSNorm into h^T upload + weights; build rope tables; bf16."""
    import ml_dtypes
    BF = ml_dtypes.bfloat16

    x64 = xs.astype(np.float64)
    istd = 1.0 / np.sqrt((x64 * x64).mean(axis=1) + EPS)    # [S]
    ht = (x64 * istd[:, None]).T.astype(BF)                 # [D, S]
    # partition-major pack: ht_pm[p, dt, s] = ht[dt*128+p, s]
    ht_pm = np.ascontiguousarray(
        ht.reshape(DT, 128, SEQ).transpose(1, 0, 2))

    def pack_w(w):  # [D, M] -> [128, DT*M] partition-major
        return np.ascontiguousarray(
            w.reshape(DT, 128, M).transpose(1, 0, 2).reshape(128, DT * M))

    nw = norm_w.astype(np.float32)[:, None, None]
    perm = np.concatenate([np.arange(0, HD, 2), np.arange(1, HD, 2)])
    wq_p = (wq * nw)[:, :, perm]
    wk_p = (wk * nw)[:, :, perm]

    inv_freq = 1.0 / (ROPE_BASE ** (np.arange(0, HD, 2, dtype=np.float32) / HD))
    pos = np.arange(SEQ, dtype=np.float32)
    ang = pos[:, None] * inv_freq[None, :]          # [S, 64]
    cos_t = np.cos(ang).T.astype(np.float32)        # [64, S]
    sin_t = np.sin(ang).T.astype(np.float32)
    cosd = np.ascontiguousarray(np.concatenate([cos_t, cos_t], 0))
    sind = np.ascontiguousarray(np.concatenate([sin_t, sin_t], 0))

    tri = np.triu(np.ones((128, 128), dtype=np.float32)).astype(BF)
    onesm = np.ones((128, 128), dtype=BF)

    common = {
        "ht": ht_pm,
        "cosd": cosd,
        "sind": sind,
        "tri": np.ascontiguousarray(tri),
        "ones": onesm,
    }
    in_maps = []
    for core in range(NCORES):
        sl = slice(core * HPC, (core + 1) * HPC)
        wot = np.transpose(wo[:, sl, :], (2, 1, 0)).reshape(128, HPC * D)
        in_maps.append({
            **common,
            "wq": pack_w(wq_p[:, sl, :].reshape(D, M).astype(BF)),
            "wk": pack_w(wk_p[:, sl, :].reshape(D, M).astype(BF)),
            "wv": pack_w((wv * nw)[:, sl, :].reshape(D, M).astype(BF)),
            "wot": np.ascontiguousarray(wot.astype(BF)),
        })
    return in_maps


def kernel(xs, norm_w, wq, wk, wv, wo):
    trace = bool(int(os.environ.get("KERNEL_TRACE", "0")))
    if trace:
        _inject_ntff_hook()
    from concourse.bass_utils import run_bass_kernel_spmd

    nc = _build_nc()
    in_maps = _host_prep(np.asarray(xs), np.asarray(norm_w), np.asarray(wq),
                         np.asarray(wk), np.asarray(wv), np.asarray(wo))
    try:
        res = run_bass_kernel_spmd(nc, in_maps, core_ids=list(range(NCORES)),
                                   trace=trace)
    except Exception:
        # transient device wedge (NRT_EXEC_UNIT_UNRECOVERABLE) recovers on
        # a fresh attempt; rebuild and retry once
        import time
        time.sleep(15)
        nc = _build_nc()
        res = run_bass_kernel_spmd(nc, in_maps, core_ids=list(range(NCORES)),
                                   trace=trace)
    if trace and res.exec_time_ns is not None:
        print(f"HW exec time: {res.exec_time_ns} ns")
    acc = np.zeros((SEQ, D), dtype=np.float64)
    for r in res.results:
        acc += r["out"].astype(np.float64)
    return acc.astype(np.float32)


if __name__ == "__main__":
    rng = np.random.default_rng(0)
    scale = 1.0 / np.sqrt(D)
    inputs = {
        "xs": rng.standard_normal((SEQ, D), dtype=np.float32),
        "norm_w": np.ones((D,), np.float32),
        "wq": rng.standard_normal((D, NH, HD), dtype=np.float32) * scale,
        "wk": rng.standard_normal((D, NH, HD), dtype=np.float32) * scale,
        "wv": rng.standard_normal((D, NH, HD), dtype=np.float32) * scale,
        "wo": rng.standard_normal((D, NH, HD), dtype=np.float32) * scale,
    }
    out = kernel(**inputs)
    print(out.shape, out.dtype, float(np.abs(out).max()))


# revision 43
# speedup vs baseline: 1.0995x; 1.0019x over previous
"""Trainium2 Bass kernel: RMSNorm + RoPE + causal attention + output projection.

Tensor-parallel over heads: 16 heads / 8 cores = 2 heads per core.
Each core computes a full [S, D] partial output (its heads' contribution to
the 'snh,dnh->sd' projection); the all-reduce is done host-side in the gather.

Fused streaming design (bf16, ~398 us vs 674 us baseline):
  - Host prep uploads the RMSNorm'd activations already transposed and
    partition-major packed (h^T as [128, DT, S] bf16) plus bf16 weights,
    a pre-transposed wo^T, and fp32 RoPE tables.  No PE transposes, no
    DRAM scratch roundtrip, and every DMA is a cheap contiguous pattern
    (strided DIRECT2D issue cost was multi-us per descriptor program).
  - Single pass over 8 q-chunks of 512: QK projections (+RoPE) append to
    per-head K^T in SBUF, V is projected directly into natural [t, hd]
    layout (ht-tile stationary), then causal attention for the chunk runs
    against all K/V tiles so far, with both heads merged into one pair
    pipeline (head 1 produces scores/exps while head 0 drains) and the
    trailing V matmuls streamed one-per-pair-slot into the PE slack; the
    previous chunk's output projection runs at chunk end, and the next
    chunk's h^T/rope-slice DMAs prefetch under the current compute.
  - Scores are computed transposed (S^T[t, s]) at PAIR granularity: two
    j-tiles' scores land in one 2-bank [128, 1024] PSUM tile and a single
    wide exp covers both, halving ACT per-instruction overhead (exp is
    the attention pacer at ~507 ns/j; PE runs ~430 ns/j underneath).
    Softmax denominators: full pairs fold on the DVE as a bf16 pairwise
    tree (~436 ns/add, inside the exp shadow); only the 4 diagonal tiles
    + one final tree reduction hit the PE as ones-matmuls.  PV/Z lag
    scores/exp by 2 pairs so a late exp never stalls the in-order PE
    queue; reciprocal_approx_fast instead of the 3.4us DVE reciprocal.
  - PSUM (16 KiB fully allocated): main 2x2-bank slots (proj pp / score
    pairs / V accum), aux 2 banks (rope ps / outproj op), o 1 bank (PV
    accum), z 1 bank (denominator).  The final chunk's outproj spreads
    over main+aux.
  - Startup: priority-ordered sync-queue DMA FIFO + 48 junk matmuls to
    hold the PE's HAM clock warm while the first loads land.
"""
import os
import sys
import types

import numpy as np

SEQ, D, NH, HD = 4096, 2048, 16, 128
NCORES = 8
HPC = NH // NCORES          # heads per core
M = HPC * HD                # per-core fused head dim (256)
EPS = 1e-6
ROPE_BASE = 10000.0
SM_SCALE = 1.0 / np.sqrt(HD)
CHUNK = 512                 # q-chunk
NCHUNK = SEQ // CHUNK       # 8
NT = SEQ // 128             # 32 s-tiles
DT = D // 128               # 16 d-tiles
LAG = 2


def _inject_ntff_hook():
    """Register the axon NTFF profiling hook (missing antenv.axon_hooks)."""
    if "antenv.axon_hooks" in sys.modules:
        return
    try:
        import antenv
        from trn_agent_boot.trn_boot import _ntff_profile_via_ctypes
    except ImportError:
        return
    holder = [None]
    mod = types.ModuleType("antenv.axon_hooks")
    mod.set_axon_ntff_profile_hook = lambda h: holder.__setitem__(0, h)
    mod.get_axon_ntff_profile_hook = lambda: holder[0]
    sys.modules["antenv.axon_hooks"] = mod
    antenv.axon_hooks = mod
    try:
        mod.set_axon_ntff_profile_hook(
            _ntff_profile_via_ctypes("/opt/axon/libaxon_pjrt.so"))
    except Exception:
        pass


def _build_nc():
    import concourse.bass as bass  # noqa: F401
    import concourse.mybir as mybir
    import concourse.tile as tile
    from concourse import bacc

    FP32 = mybir.dt.float32
    BF16 = mybir.dt.bfloat16
    AF = mybir.ActivationFunctionType
    ALU = mybir.AluOpType

    nc = bacc.Bacc(None, target_bir_lowering=False)

    ht_d = nc.declare_dram_parameter("ht", [128, DT, SEQ], BF16,
                                     isOutput=False)
    wq = nc.declare_dram_parameter("wq", [128, DT * M], BF16, isOutput=False)
    wk = nc.declare_dram_parameter("wk", [128, DT * M], BF16, isOutput=False)
    wv = nc.declare_dram_parameter("wv", [128, DT * M], BF16, isOutput=False)
    wot_d = nc.declare_dram_parameter("wot", [128, HPC * D], BF16,
                                      isOutput=False)
    csd = nc.declare_dram_parameter("cossin", [128, 2, SEQ], FP32,
                                    isOutput=False)
    tri = nc.declare_dram_parameter("tri", [128, 128], BF16, isOutput=False)
    ones = nc.declare_dram_parameter("ones", [128, 128], BF16, isOutput=False)
    out = nc.declare_dram_parameter("out", [SEQ, D], BF16, isOutput=True)

    with tile.TileContext(nc) as tc:
        with tc.tile_pool(name="consts", bufs=1) as consts, \
             tc.tile_pool(name="pht", bufs=2) as pht, \
             tc.tile_pool(name="pqt", bufs=4) as pqt, \
             tc.tile_pool(name="ppc", bufs=2) as ppc, \
             tc.tile_pool(name="ppt", bufs=6) as ppt, \
             tc.tile_pool(name="prz", bufs=2) as prz, \
             tc.tile_pool(name="pat", bufs=4) as pat, \
             tc.tile_pool(name="post", bufs=3) as post, \
             tc.tile_pool(name="pcs", bufs=2) as pcs, \
             tc.tile_pool(name="pzt", bufs=8) as pzt, \
             tc.tile_pool(name="psum", bufs=2, space="PSUM") as psum:
            kt_sb = [consts.tile([128, SEQ], BF16, name=f"kt{h}")
                     for h in range(HPC)]
            # V natural, packed per t-tile: vn[p, jt*M + m] = V[jt*128+p, m]
            vn_sb = consts.tile([128, NT * M], BF16)

            def load_ht(c):
                cs = slice(c * CHUNK, (c + 1) * CHUNK)
                ht = pht.tile([128, DT, CHUNK], BF16, name="ht")
                nc.sync.dma_start(out=ht[:], in_=ht_d[:, :, cs])
                return ht

            def load_cs(c):
                cs = slice(c * CHUNK, (c + 1) * CHUNK)
                cst = pcs.tile([128, 2, CHUNK], FP32, name="cst")
                nc.sync.dma_start(out=cst[:], in_=csd[:, :, cs])
                return cst[:, 0, :], cst[:, 1, :]

            # startup: one sync-queue FIFO in priority order — ring
            # bandwidth serves the first projections' pieces (ht, wq,
            # rope slices) before the bulk loads behind them.
            w_sbs = {k: consts.tile([128, DT, M], BF16, name=f"w{k}_sb")
                     for k in ("q", "k", "v")}
            cs0 = slice(0, CHUNK)
            ht = pht.tile([128, DT, CHUNK], BF16, name="ht")
            for q4 in range(4):
                dts = slice(q4 * 4, (q4 + 1) * 4)
                nc.sync.dma_start(out=ht[:, dts, :], in_=ht_d[:, dts, cs0])
                nc.sync.dma_start(
                    out=w_sbs["q"][:, dts, :].rearrange("p t m -> p (t m)"),
                    in_=wq[:, q4 * 4 * M:(q4 + 1) * 4 * M])
                if q4 == 1:
                    cs_cur = load_cs(0)
            nc.sync.dma_start(
                out=w_sbs["k"][:].rearrange("p t m -> p (t m)"), in_=wk[:])
            nc.sync.dma_start(
                out=w_sbs["v"][:].rearrange("p t m -> p (t m)"), in_=wv[:])
            tri_sb = consts.tile([128, 128], BF16)
            nc.sync.dma_start(out=tri_sb[:], in_=tri[:])
            ones_sb = consts.tile([128, 128], BF16)
            nc.sync.dma_start(out=ones_sb[:], in_=ones[:])
            wot_sb = consts.tile([128, HPC * D], BF16)
            nc.sync.dma_start(out=wot_sb[:], in_=wot_d[:])

            # HAM warm-up: keep the PE busy on junk matmuls while the
            # startup DMAs land, so real matmuls start at the warm clock.
            warm = consts.tile([128, 128], BF16)
            nc.vector.memset(warm[:], 0.0)
            for i in range(48):
                wp = psum.tile([128, CHUNK], FP32, name="wp", tag="main")
                nc.tensor.matmul(wp[:, 0:128], warm[:], warm[:],
                                 start=True, stop=True)

            def outproj(ats, c, spread=False):
                for st in range(4):
                    ost = post.tile([128, D], BF16, name="ost")
                    for dc in range(4):
                        # the final chunk's projection has nothing after it
                        # to hide evac latency: spread over two tag rings so
                        # 4 banks ping-pong instead of 2.
                        tag = "main" if spread and dc % 2 else "aux"
                        op = psum.tile([128, CHUNK], FP32, name="op",
                                       tag=tag)
                        nc.tensor.matmul(
                            op[:],
                            ats[0][:, st * 128:(st + 1) * 128],
                            wot_sb[:, dc * CHUNK:(dc + 1) * CHUNK],
                            start=True, stop=False)
                        nc.tensor.matmul(
                            op[:],
                            ats[1][:, st * 128:(st + 1) * 128],
                            wot_sb[:, D + dc * CHUNK:D + (dc + 1) * CHUNK],
                            start=False, stop=True)
                        if dc % 2 == 0:
                            nc.scalar.activation(
                                ost[:, dc * CHUNK:(dc + 1) * CHUNK],
                                op[:], AF.Copy)
                        else:
                            nc.vector.tensor_copy(
                                ost[:, dc * CHUNK:(dc + 1) * CHUNK], op[:])
                    row = (c * 4 + st) * 128
                    eng = nc.sync if st % 2 == 0 else nc.scalar
                    eng.dma_start(out=out[row:row + 128, :], in_=ost[:])

            ats_prev = None
            for c in range(NCHUNK):
                cs = slice(c * CHUNK, (c + 1) * CHUNK)
                cos_t, sin_t = cs_cur

                # ---- QK projections + RoPE
                qts = []
                for head in range(HPC):
                    for kind in ("q", "k"):
                        w_sb = w_sbs[kind]
                        pp = psum.tile([128, CHUNK], FP32, name="pp",
                                       tag="main")
                        for dt in range(DT):
                            nc.tensor.matmul(
                                pp[:],
                                w_sb[:, dt, head * HD:(head + 1) * HD],
                                ht[:, dt, :],
                                start=(dt == 0), stop=(dt == DT - 1))
                        ps = psum.tile([128, CHUNK], FP32, name="ps",
                                       tag="aux")
                        nc.vector.tensor_tensor(
                            out=ps[:], in0=pp[:], in1=sin_t[:],
                            op=ALU.mult)
                        pc = ppc.tile([128, CHUNK], FP32, name="pc")
                        nc.vector.tensor_tensor(
                            out=pc[:], in0=pp[:], in1=cos_t[:],
                            op=ALU.mult)
                        if kind == "q":
                            dst = pqt.tile([128, CHUNK], BF16,
                                           name=f"qt{head}")
                            qts.append(dst)
                            d0, d1 = dst[0:64, :], dst[64:128, :]
                        else:
                            d0 = kt_sb[head][0:64, cs]
                            d1 = kt_sb[head][64:128, cs]
                        nc.vector.tensor_tensor(
                            out=d0, in0=pc[0:64, :], in1=ps[64:128, :],
                            op=ALU.subtract)
                        nc.vector.tensor_tensor(
                            out=d1, in0=pc[64:128, :], in1=ps[0:64, :],
                            op=ALU.add)

                # ---- V projection directly into natural [t, m] layout.
                # Emitted as thunks: the leading 64-2c matmuls go here, the
                # last 2c stream one-per-pair-slot into the attention
                # pipeline below, filling the PE slack under the exp pacing
                # (the aux ring is idle during attention).
                ht_c, c_ = ht, c
                v_mms = [(g, half, dt)
                         for g in range(2)
                         for half in range(2)
                         for dt in range(DT)]
                pv_tiles = {}

                def emit_v(n):
                    for _ in range(n):
                        if not v_mms:
                            return
                        g, half, dt = v_mms.pop(0)
                        if g not in pv_tiles:
                            pv_tiles[g] = psum.tile([128, CHUNK], FP32,
                                                    name="pv", tag="aux")
                        st = g * 2 + half
                        nc.tensor.matmul(
                            pv_tiles[g][:, half * M:(half + 1) * M],
                            ht_c[:, dt, st * 128:(st + 1) * 128],
                            w_sbs["v"][:, dt, :],
                            start=(dt == 0), stop=(dt == DT - 1))
                        if half == 1 and dt == DT - 1:
                            t0 = (c_ * 4 + g * 2) * M
                            nc.vector.tensor_copy(
                                vn_sb[:, t0:t0 + 2 * M], pv_tiles[g][:])
                emit_v(64 - 2 * c)

                # prefetch next chunk's h^T + rope slices while computing
                if c + 1 < NCHUNK:
                    ht = load_ht(c + 1)
                    cs_cur = load_cs(c + 1)

                # ---- causal attention, both heads merged into ONE pair
                # pipeline: head 1's scores/exps produce while head 0's tail
                # drains, so the exp stream (the attention pacer) never
                # bubbles at the head transition.  Two j-tiles' scores land
                # in one 2-bank [128, 1024] PSUM tile and a single wide exp
                # covers both (halves ACT per-instruction overhead).
                # Softmax denominator: full pairs fold on the DVE as a bf16
                # pairwise tree; only the 4 diagonal tiles + one final tree
                # reduction hit the PE.
                jmax = 4 * c + 3
                npair = (jmax + 1) // 2
                PLAG = 2
                ats = []
                st8 = [{"o": None, "z": None, "zstack": {}}
                       for _ in range(HPC)]
                pairs = {}

                def tree_push(head, t, lvl):
                    zstack = st8[head]["zstack"]
                    while lvl in zstack:
                        prev = zstack.pop(lvl)
                        nt = pzt.tile([128, CHUNK], BF16, name="zt")
                        nc.vector.tensor_tensor(out=nt[:], in0=prev[:],
                                                in1=t[:], op=ALU.add)
                        t = nt
                        lvl += 1
                    zstack[lvl] = t

                def produce(head, pi):
                    spp = psum.tile([128, 2 * CHUNK], FP32,
                                    name="spp", tag="main")
                    ptp = ppt.tile([128, 2 * CHUNK], BF16, name="ptp")
                    for h2 in range(2):
                        j = 2 * pi + h2
                        off = h2 * CHUNK + max(j - 4 * c, 0) * 128
                        nc.tensor.matmul(
                            spp[:, off:(h2 + 1) * CHUNK],
                            kt_sb[head][:, j * 128:(j + 1) * 128],
                            qts[head][:, off - h2 * CHUNK:],
                            start=True, stop=True)
                    lo = max(2 * pi - 4 * c, 0) * 128
                    nc.scalar.activation(ptp[:, lo:], spp[:, lo:], AF.Exp,
                                         scale=float(SM_SCALE))
                    for h2 in range(2):
                        j = 2 * pi + h2
                        r = j - 4 * c
                        if r >= 0:
                            off = h2 * CHUNK + r * 128
                            nc.vector.tensor_tensor(
                                out=ptp[:, off:off + 128],
                                in0=ptp[:, off:off + 128],
                                in1=tri_sb[:], op=ALU.mult)
                    pairs[(head, pi)] = ptp

                def consume(head, i):
                    s = st8[head]
                    if s["o"] is None:
                        s["o"] = psum.tile([128, CHUNK], FP32, name="o_acc",
                                           tag="o", bufs=1)
                    ptp = pairs.pop((head, i))
                    for h2 in range(2):
                        j = 2 * i + h2
                        off = h2 * CHUNK + max(j - 4 * c, 0) * 128
                        nc.tensor.matmul(
                            s["o"][:, off - h2 * CHUNK:],
                            vn_sb[:, j * M + head * HD:
                                  j * M + (head + 1) * HD],
                            ptp[:, off:(h2 + 1) * CHUNK],
                            start=(j == 0), stop=(j == jmax),
                            skip_group_check=True)
                    if 2 * i + 1 < 4 * c:
                        # full pair: fold its two halves, push at level 1
                        # (each tree node covers 2 tiles)
                        nt = pzt.tile([128, CHUNK], BF16, name="zt")
                        nc.vector.tensor_tensor(
                            out=nt[:], in0=ptp[:, 0:CHUNK],
                            in1=ptp[:, CHUNK:], op=ALU.add)
                        tree_push(head, nt, 1)
                    else:
                        if s["z"] is None:
                            s["z"] = psum.tile([128, CHUNK], FP32,
                                               name="z_acc", tag="z",
                                               bufs=1)
                        for h2 in range(2):
                            j = 2 * i + h2
                            off = h2 * CHUNK + max(j - 4 * c, 0) * 128
                            nc.tensor.matmul(
                                s["z"][:, off - h2 * CHUNK:],
                                ones_sb[:],
                                ptp[:, off:(h2 + 1) * CHUNK],
                                start=(j == 4 * c),
                                stop=(j == jmax and c == 0),
                                skip_group_check=True)

                def finish(head):
                    s = st8[head]
                    if c > 0:
                        # collapse the tree and reduce over partitions with
                        # one ones-matmul accumulated into the z bank
                        parts = [s["zstack"][l] for l in sorted(s["zstack"])]
                        zt = parts[0]
                        for nxt in parts[1:]:
                            nt = pzt.tile([128, CHUNK], BF16, name="zt")
                            nc.vector.tensor_tensor(out=nt[:], in0=zt[:],
                                                    in1=nxt[:], op=ALU.add)
                            zt = nt
                        nc.tensor.matmul(
                            s["z"][:], ones_sb[:], zt[:],
                            start=False, stop=True, skip_group_check=True)
                    rz = prz.tile([128, CHUNK], FP32, name="rz")
                    nc.vector.reciprocal_approx_fast(out=rz[:],
                                                     in_=s["z"][:])
                    at = pat.tile([128, CHUNK], BF16, name=f"at{head}")
                    nc.vector.tensor_tensor(out=at[:], in0=s["o"][:],
                                            in1=rz[:], op=ALU.mult)
                    ats.append(at)

                for gp in range(2 * npair + PLAG):
                    if gp < npair:
                        produce(0, gp)
                    elif gp < 2 * npair:
                        produce(1, gp - npair)
                    emit_v(1)
                    cg = gp - PLAG
                    if cg >= 0:
                        if cg < npair:
                            consume(0, cg)
                        else:
                            consume(1, cg - npair)
                    if cg == npair - 1:
                        finish(0)
                if ats_prev is not None:
                    outproj(ats_prev, c - 1)
                finish(1)
                ats_prev = ats
            outproj(ats_prev, NCHUNK - 1, spread=True)
    nc.finalize()
    return nc


def _host_prep(xs, norm_w, wq, wk, wv, wo):
    """Fold RMSNorm into h^T upload + weights; build rope tables; bf16."""
    import ml_dtypes
    BF = ml_dtypes.bfloat16

    x64 = xs.astype(np.float64)
    istd = 1.0 / np.sqrt((x64 * x64).mean(axis=1) + EPS)    # [S]
    ht = (x64 * istd[:, None]).T.astype(BF)                 # [D, S]
    # partition-major pack: ht_pm[p, dt, s] = ht[dt*128+p, s]
    ht_pm = np.ascontiguousarray(
        ht.reshape(DT, 128, SEQ).transpose(1, 0, 2))

    def pack_w(w):  # [D, M] -> [128, DT*M] partition-major
        return np.ascontiguousarray(
            w.reshape(DT, 128, M).transpose(1, 0, 2).reshape(128, DT * M))

    nw = norm_w.astype(np.float32)[:, None, None]
    perm = np.concatenate([np.arange(0, HD, 2), np.arange(1, HD, 2)])
    wq_p = (wq * nw)[:, :, perm]
    wk_p = (wk * nw)[:, :, perm]

    inv_freq = 1.0 / (ROPE_BASE ** (np.arange(0, HD, 2, dtype=np.float32) / HD))
    pos = np.arange(SEQ, dtype=np.float32)
    ang = pos[:, None] * inv_freq[None, :]          # [S, 64]
    cos_t = np.cos(ang).T.astype(np.float32)        # [64, S]
    sin_t = np.sin(ang).T.astype(np.float32)
    cosd = np.concatenate([cos_t, cos_t], 0)        # [128, S]
    sind = np.concatenate([sin_t, sin_t], 0)
    cossin = np.ascontiguousarray(np.stack([cosd, sind], axis=1))

    tri = np.triu(np.ones((128, 128), dtype=np.float32)).astype(BF)
    onesm = np.ones((128, 128), dtype=BF)

    common = {
        "ht": ht_pm,
        "cossin": cossin,
        "tri": np.ascontiguousarray(tri),
        "ones": onesm,
    }
    in_maps = []
    for core in range(NCORES):
        sl = slice(core * HPC, (core + 1) * HPC)
        wot = np.transpose(wo[:, sl, :], (2, 1, 0)).reshape(128, HPC * D)
        in_maps.append({
            **common,
            "wq": pack_w(wq_p[:, sl, :].reshape(D, M).astype(BF)),
            "wk": pack_w(wk_p[:, sl, :].reshape(D, M).astype(BF)),
            "wv": pack_w((wv * nw)[:, sl, :].reshape(D, M).astype(BF)),
            "wot": np.ascontiguousarray(wot.astype(BF)),
        })
    return in_maps


def kernel(xs, norm_w, wq, wk, wv, wo):
    trace = bool(int(os.environ.get("KERNEL_TRACE", "0")))
    if trace:
        _inject_ntff_hook()
    from concourse.bass_utils import run_bass_kernel_spmd

    nc = _build_nc()
    in_maps = _host_prep(np.asarray(xs), np.asarray(norm_w), np.asarray(wq),
                         np.asarray(wk), np.asarray(wv), np.asarray(wo))
    try:
        res = run_bass_kernel_spmd(nc, in_maps, core_ids=list(range(NCORES)),
                                   trace=trace)
    except Exception:
        # transient device wedge (NRT_EXEC_UNIT_UNRECOVERABLE) recovers on
        # a fresh attempt; rebuild and retry once
        import time
        time.sleep(15)
        nc = _build_nc()
        res = run_bass_kernel_spmd(nc, in_maps, core_ids=list(range(NCORES)),
                                   trace=trace)
    if trace and res.exec_time_ns is not None:
        print(f"HW exec time: {res.exec_time_ns} ns")
    acc = np.zeros((SEQ, D), dtype=np.float64)
    for r in res.results:
        acc += r["out"].astype(np.float64)
    return acc.astype(np.float32)


if __name__ == "__main__":
    rng = np.random.default_rng(0)
    scale = 1.0 / np.sqrt(D)
    inputs = {
        "xs": rng.standard_normal((SEQ, D), dtype=np.float32),
        "norm_w": np.ones((D,), np.float32),
        "wq": rng.standard_normal((D, NH, HD), dtype=np.float32) * scale,
        "wk": rng.standard_normal((D, NH, HD), dtype=np.float32) * scale,
        "wv": rng.standard_normal((D, NH, HD), dtype=np.float32) * scale,
        "wo": rng.standard_normal((D, NH, HD), dtype=np.float32) * scale,
    }
    out = kernel(**inputs)
    print(out.shape, out.dtype, float(np.abs(out).max()))


# revision 44
# speedup vs baseline: 1.0999x; 1.0003x over previous
"""Trainium2 Bass kernel: RMSNorm + RoPE + causal attention + output projection.

Tensor-parallel over heads: 16 heads / 8 cores = 2 heads per core.
Each core computes a full [S, D] partial output (its heads' contribution to
the 'snh,dnh->sd' projection); the all-reduce is done host-side in the gather.

Fused streaming design (bf16, ~398 us vs 674 us baseline):
  - Host prep uploads the RMSNorm'd activations already transposed and
    partition-major packed (h^T as [128, DT, S] bf16) plus bf16 weights,
    a pre-transposed wo^T, and fp32 RoPE tables.  No PE transposes, no
    DRAM scratch roundtrip, and every DMA is a cheap contiguous pattern
    (strided DIRECT2D issue cost was multi-us per descriptor program).
  - Single pass over 8 q-chunks of 512: QK projections (+RoPE) append to
    per-head K^T in SBUF, V is projected directly into natural [t, hd]
    layout (ht-tile stationary), then causal attention for the chunk runs
    against all K/V tiles so far, with both heads merged into one pair
    pipeline (head 1 produces scores/exps while head 0 drains) and the
    trailing V matmuls streamed one-per-pair-slot into the PE slack; the
    previous chunk's output projection runs at chunk end, and the next
    chunk's h^T/rope-slice DMAs prefetch under the current compute.
  - Scores are computed transposed (S^T[t, s]) at PAIR granularity: two
    j-tiles' scores land in one 2-bank [128, 1024] PSUM tile and a single
    wide exp covers both, halving ACT per-instruction overhead (exp is
    the attention pacer at ~507 ns/j; PE runs ~430 ns/j underneath).
    Softmax denominators: full pairs fold on the DVE as a bf16 pairwise
    tree (~436 ns/add, inside the exp shadow); only the 4 diagonal tiles
    + one final tree reduction hit the PE as ones-matmuls.  PV/Z lag
    scores/exp by 2 pairs so a late exp never stalls the in-order PE
    queue; reciprocal_approx_fast instead of the 3.4us DVE reciprocal.
  - PSUM (16 KiB fully allocated): main 2x2-bank slots (proj pp / score
    pairs / V accum), aux 2 banks (rope ps / outproj op), o 1 bank (PV
    accum), z 1 bank (denominator).  The final chunk's outproj spreads
    over main+aux.
  - Startup: priority-ordered sync-queue DMA FIFO + 48 junk matmuls to
    hold the PE's HAM clock warm while the first loads land.
"""
import os
import sys
import types

import numpy as np

SEQ, D, NH, HD = 4096, 2048, 16, 128
NCORES = 8
HPC = NH // NCORES          # heads per core
M = HPC * HD                # per-core fused head dim (256)
EPS = 1e-6
ROPE_BASE = 10000.0
SM_SCALE = 1.0 / np.sqrt(HD)
CHUNK = 512                 # q-chunk
NCHUNK = SEQ // CHUNK       # 8
NT = SEQ // 128             # 32 s-tiles
DT = D // 128               # 16 d-tiles
LAG = 2


def _inject_ntff_hook():
    """Register the axon NTFF profiling hook (missing antenv.axon_hooks)."""
    if "antenv.axon_hooks" in sys.modules:
        return
    try:
        import antenv
        from trn_agent_boot.trn_boot import _ntff_profile_via_ctypes
    except ImportError:
        return
    holder = [None]
    mod = types.ModuleType("antenv.axon_hooks")
    mod.set_axon_ntff_profile_hook = lambda h: holder.__setitem__(0, h)
    mod.get_axon_ntff_profile_hook = lambda: holder[0]
    sys.modules["antenv.axon_hooks"] = mod
    antenv.axon_hooks = mod
    try:
        mod.set_axon_ntff_profile_hook(
            _ntff_profile_via_ctypes("/opt/axon/libaxon_pjrt.so"))
    except Exception:
        pass


def _build_nc():
    import concourse.bass as bass  # noqa: F401
    import concourse.mybir as mybir
    import concourse.tile as tile
    from concourse import bacc

    FP32 = mybir.dt.float32
    BF16 = mybir.dt.bfloat16
    AF = mybir.ActivationFunctionType
    ALU = mybir.AluOpType

    nc = bacc.Bacc(None, target_bir_lowering=False)

    ht_d = nc.declare_dram_parameter("ht", [128, DT, SEQ], BF16,
                                     isOutput=False)
    wq = nc.declare_dram_parameter("wq", [128, DT * M], BF16, isOutput=False)
    wk = nc.declare_dram_parameter("wk", [128, DT * M], BF16, isOutput=False)
    wv = nc.declare_dram_parameter("wv", [128, DT * M], BF16, isOutput=False)
    wot_d = nc.declare_dram_parameter("wot", [128, HPC * D], BF16,
                                      isOutput=False)
    csd = nc.declare_dram_parameter("cossin", [128, 2, SEQ], FP32,
                                    isOutput=False)
    tri = nc.declare_dram_parameter("tri", [128, 128], BF16, isOutput=False)
    ones = nc.declare_dram_parameter("ones", [128, 128], BF16, isOutput=False)
    out = nc.declare_dram_parameter("out", [SEQ, D], BF16, isOutput=True)

    with tile.TileContext(nc) as tc:
        with tc.tile_pool(name="consts", bufs=1) as consts, \
             tc.tile_pool(name="pht", bufs=2) as pht, \
             tc.tile_pool(name="pqt", bufs=4) as pqt, \
             tc.tile_pool(name="ppc", bufs=2) as ppc, \
             tc.tile_pool(name="ppt", bufs=6) as ppt, \
             tc.tile_pool(name="prz", bufs=2) as prz, \
             tc.tile_pool(name="pat", bufs=4) as pat, \
             tc.tile_pool(name="post", bufs=3) as post, \
             tc.tile_pool(name="pcs", bufs=2) as pcs, \
             tc.tile_pool(name="pzt", bufs=10) as pzt, \
             tc.tile_pool(name="psum", bufs=2, space="PSUM") as psum:
            kt_sb = [consts.tile([128, SEQ], BF16, name=f"kt{h}")
                     for h in range(HPC)]
            # V natural, packed per t-tile: vn[p, jt*M + m] = V[jt*128+p, m]
            vn_sb = consts.tile([128, NT * M], BF16)

            def load_ht(c):
                cs = slice(c * CHUNK, (c + 1) * CHUNK)
                ht = pht.tile([128, DT, CHUNK], BF16, name="ht")
                nc.sync.dma_start(out=ht[:], in_=ht_d[:, :, cs])
                return ht

            def load_cs(c):
                cs = slice(c * CHUNK, (c + 1) * CHUNK)
                cst = pcs.tile([128, 2, CHUNK], FP32, name="cst")
                nc.sync.dma_start(out=cst[:], in_=csd[:, :, cs])
                return cst[:, 0, :], cst[:, 1, :]

            # startup: one sync-queue FIFO in priority order — ring
            # bandwidth serves the first projections' pieces (ht, wq,
            # rope slices) before the bulk loads behind them.
            w_sbs = {k: consts.tile([128, DT, M], BF16, name=f"w{k}_sb")
                     for k in ("q", "k", "v")}
            cs0 = slice(0, CHUNK)
            ht = pht.tile([128, DT, CHUNK], BF16, name="ht")
            for q4 in range(4):
                dts = slice(q4 * 4, (q4 + 1) * 4)
                nc.sync.dma_start(out=ht[:, dts, :], in_=ht_d[:, dts, cs0])
                nc.sync.dma_start(
                    out=w_sbs["q"][:, dts, :].rearrange("p t m -> p (t m)"),
                    in_=wq[:, q4 * 4 * M:(q4 + 1) * 4 * M])
                if q4 == 1:
                    cs_cur = load_cs(0)
            nc.sync.dma_start(
                out=w_sbs["k"][:].rearrange("p t m -> p (t m)"), in_=wk[:])
            nc.sync.dma_start(
                out=w_sbs["v"][:].rearrange("p t m -> p (t m)"), in_=wv[:])
            tri_sb = consts.tile([128, 128], BF16)
            nc.sync.dma_start(out=tri_sb[:], in_=tri[:])
            ones_sb = consts.tile([128, 128], BF16)
            nc.sync.dma_start(out=ones_sb[:], in_=ones[:])
            wot_sb = consts.tile([128, HPC * D], BF16)
            nc.sync.dma_start(out=wot_sb[:], in_=wot_d[:])

            # HAM warm-up: keep the PE busy on junk matmuls while the
            # startup DMAs land, so real matmuls start at the warm clock.
            warm = consts.tile([128, 128], BF16)
            nc.vector.memset(warm[:], 0.0)
            for i in range(48):
                wp = psum.tile([128, CHUNK], FP32, name="wp", tag="main")
                nc.tensor.matmul(wp[:, 0:128], warm[:], warm[:],
                                 start=True, stop=True)

            def outproj(ats, c, spread=False):
                for st in range(4):
                    ost = post.tile([128, D], BF16, name="ost")
                    for dc in range(4):
                        # the final chunk's projection has nothing after it
                        # to hide evac latency: spread over two tag rings so
                        # 4 banks ping-pong instead of 2.
                        tag = "main" if spread and dc % 2 else "aux"
                        op = psum.tile([128, CHUNK], FP32, name="op",
                                       tag=tag)
                        nc.tensor.matmul(
                            op[:],
                            ats[0][:, st * 128:(st + 1) * 128],
                            wot_sb[:, dc * CHUNK:(dc + 1) * CHUNK],
                            start=True, stop=False)
                        nc.tensor.matmul(
                            op[:],
                            ats[1][:, st * 128:(st + 1) * 128],
                            wot_sb[:, D + dc * CHUNK:D + (dc + 1) * CHUNK],
                            start=False, stop=True)
                        if dc % 2 == 0:
                            nc.scalar.activation(
                                ost[:, dc * CHUNK:(dc + 1) * CHUNK],
                                op[:], AF.Copy)
                        else:
                            nc.vector.tensor_copy(
                                ost[:, dc * CHUNK:(dc + 1) * CHUNK], op[:])
                    row = (c * 4 + st) * 128
                    eng = nc.sync if st % 2 == 0 else nc.scalar
                    eng.dma_start(out=out[row:row + 128, :], in_=ost[:])

            ats_prev = None
            for c in range(NCHUNK):
                cs = slice(c * CHUNK, (c + 1) * CHUNK)
                cos_t, sin_t = cs_cur

                # ---- QK projections + RoPE
                qts = []
                for head in range(HPC):
                    for kind in ("q", "k"):
                        w_sb = w_sbs[kind]
                        pp = psum.tile([128, CHUNK], FP32, name="pp",
                                       tag="main")
                        for dt in range(DT):
                            nc.tensor.matmul(
                                pp[:],
                                w_sb[:, dt, head * HD:(head + 1) * HD],
                                ht[:, dt, :],
                                start=(dt == 0), stop=(dt == DT - 1))
                        ps = psum.tile([128, CHUNK], FP32, name="ps",
                                       tag="aux")
                        nc.vector.tensor_tensor(
                            out=ps[:], in0=pp[:], in1=sin_t[:],
                            op=ALU.mult)
                        pc = ppc.tile([128, CHUNK], FP32, name="pc")
                        nc.vector.tensor_tensor(
                            out=pc[:], in0=pp[:], in1=cos_t[:],
                            op=ALU.mult)
                        if kind == "q":
                            dst = pqt.tile([128, CHUNK], BF16,
                                           name=f"qt{head}")
                            qts.append(dst)
                            d0, d1 = dst[0:64, :], dst[64:128, :]
                        else:
                            d0 = kt_sb[head][0:64, cs]
                            d1 = kt_sb[head][64:128, cs]
                        nc.vector.tensor_tensor(
                            out=d0, in0=pc[0:64, :], in1=ps[64:128, :],
                            op=ALU.subtract)
                        nc.vector.tensor_tensor(
                            out=d1, in0=pc[64:128, :], in1=ps[0:64, :],
                            op=ALU.add)

                # ---- V projection directly into natural [t, m] layout.
                # Emitted as thunks: the leading 64-2c matmuls go here, the
                # last 2c stream one-per-pair-slot into the attention
                # pipeline below, filling the PE slack under the exp pacing
                # (the aux ring is idle during attention).
                ht_c, c_ = ht, c
                v_mms = [(g, half, dt)
                         for g in range(2)
                         for half in range(2)
                         for dt in range(DT)]
                pv_tiles = {}

                def emit_v(n):
                    for _ in range(n):
                        if not v_mms:
                            return
                        g, half, dt = v_mms.pop(0)
                        if g not in pv_tiles:
                            pv_tiles[g] = psum.tile([128, CHUNK], FP32,
                                                    name="pv", tag="aux")
                        st = g * 2 + half
                        nc.tensor.matmul(
                            pv_tiles[g][:, half * M:(half + 1) * M],
                            ht_c[:, dt, st * 128:(st + 1) * 128],
                            w_sbs["v"][:, dt, :],
                            start=(dt == 0), stop=(dt == DT - 1))
                        if half == 1 and dt == DT - 1:
                            t0 = (c_ * 4 + g * 2) * M
                            nc.vector.tensor_copy(
                                vn_sb[:, t0:t0 + 2 * M], pv_tiles[g][:])
                emit_v(64 - 2 * c)

                # prefetch next chunk's h^T + rope slices while computing
                if c + 1 < NCHUNK:
                    ht = load_ht(c + 1)
                    cs_cur = load_cs(c + 1)

                # ---- causal attention, both heads merged into ONE pair
                # pipeline: head 1's scores/exps produce while head 0's tail
                # drains, so the exp stream (the attention pacer) never
                # bubbles at the head transition.  Two j-tiles' scores land
                # in one 2-bank [128, 1024] PSUM tile and a single wide exp
                # covers both (halves ACT per-instruction overhead).
                # Softmax denominator: full pairs fold on the DVE as a bf16
                # pairwise tree; only the 4 diagonal tiles + one final tree
                # reduction hit the PE.
                jmax = 4 * c + 3
                npair = (jmax + 1) // 2
                PLAG = 2
                ats = []
                st8 = [{"o": None, "z": None, "zstack": {}}
                       for _ in range(HPC)]
                pairs = {}

                def tree_push(head, t, lvl):
                    zstack = st8[head]["zstack"]
                    while lvl in zstack:
                        prev = zstack.pop(lvl)
                        nt = pzt.tile([128, CHUNK], BF16, name="zt")
                        nc.vector.tensor_tensor(out=nt[:], in0=prev[:],
                                                in1=t[:], op=ALU.add)
                        t = nt
                        lvl += 1
                    zstack[lvl] = t

                def produce(head, pi):
                    spp = psum.tile([128, 2 * CHUNK], FP32,
                                    name="spp", tag="main")
                    ptp = ppt.tile([128, 2 * CHUNK], BF16, name="ptp")
                    for h2 in range(2):
                        j = 2 * pi + h2
                        off = h2 * CHUNK + max(j - 4 * c, 0) * 128
                        nc.tensor.matmul(
                            spp[:, off:(h2 + 1) * CHUNK],
                            kt_sb[head][:, j * 128:(j + 1) * 128],
                            qts[head][:, off - h2 * CHUNK:],
                            start=True, stop=True)
                    lo = max(2 * pi - 4 * c, 0) * 128
                    nc.scalar.activation(ptp[:, lo:], spp[:, lo:], AF.Exp,
                                         scale=float(SM_SCALE))
                    for h2 in range(2):
                        j = 2 * pi + h2
                        r = j - 4 * c
                        if r >= 0:
                            off = h2 * CHUNK + r * 128
                            nc.vector.tensor_tensor(
                                out=ptp[:, off:off + 128],
                                in0=ptp[:, off:off + 128],
                                in1=tri_sb[:], op=ALU.mult)
                    pairs[(head, pi)] = ptp

                def consume(head, i):
                    s = st8[head]
                    if s["o"] is None:
                        s["o"] = psum.tile([128, CHUNK], FP32, name="o_acc",
                                           tag="o", bufs=1)
                    ptp = pairs.pop((head, i))
                    for h2 in range(2):
                        j = 2 * i + h2
                        off = h2 * CHUNK + max(j - 4 * c, 0) * 128
                        nc.tensor.matmul(
                            s["o"][:, off - h2 * CHUNK:],
                            vn_sb[:, j * M + head * HD:
                                  j * M + (head + 1) * HD],
                            ptp[:, off:(h2 + 1) * CHUNK],
                            start=(j == 0), stop=(j == jmax),
                            skip_group_check=True)
                    if 2 * i + 1 < 4 * c:
                        # full pair: fold its two halves, push at level 1
                        # (each tree node covers 2 tiles)
                        nt = pzt.tile([128, CHUNK], BF16, name="zt")
                        nc.vector.tensor_tensor(
                            out=nt[:], in0=ptp[:, 0:CHUNK],
                            in1=ptp[:, CHUNK:], op=ALU.add)
                        tree_push(head, nt, 1)
                    else:
                        if s["z"] is None:
                            s["z"] = psum.tile([128, CHUNK], FP32,
                                               name="z_acc", tag="z",
                                               bufs=1)
                        for h2 in range(2):
                            j = 2 * i + h2
                            off = h2 * CHUNK + max(j - 4 * c, 0) * 128
                            nc.tensor.matmul(
                                s["z"][:, off - h2 * CHUNK:],
                                ones_sb[:],
                                ptp[:, off:(h2 + 1) * CHUNK],
                                start=(j == 4 * c),
                                stop=(j == jmax and c == 0),
                                skip_group_check=True)
                    if c > 0 and i == 2 * c - 1:
                        # last full pair consumed: collapse the tree NOW so
                        # the final ones-matmul in finish() never waits on
                        # the DVE fold chain
                        parts = [s["zstack"][l] for l in sorted(s["zstack"])]
                        zt = parts[0]
                        for nxt in parts[1:]:
                            nt = pzt.tile([128, CHUNK], BF16, name="zt")
                            nc.vector.tensor_tensor(out=nt[:], in0=zt[:],
                                                    in1=nxt[:], op=ALU.add)
                            zt = nt
                        s["ztf"] = zt

                def finish(head):
                    s = st8[head]
                    if c > 0:
                        nc.tensor.matmul(
                            s["z"][:], ones_sb[:], s["ztf"][:],
                            start=False, stop=True, skip_group_check=True)
                    rz = prz.tile([128, CHUNK], FP32, name="rz")
                    nc.vector.reciprocal_approx_fast(out=rz[:],
                                                     in_=s["z"][:])
                    at = pat.tile([128, CHUNK], BF16, name=f"at{head}")
                    nc.vector.tensor_tensor(out=at[:], in0=s["o"][:],
                                            in1=rz[:], op=ALU.mult)
                    ats.append(at)

                for gp in range(2 * npair + PLAG):
                    if gp < npair:
                        produce(0, gp)
                    elif gp < 2 * npair:
                        produce(1, gp - npair)
                    emit_v(1)
                    cg = gp - PLAG
                    if cg >= 0:
                        if cg < npair:
                            consume(0, cg)
                        else:
                            consume(1, cg - npair)
                    if cg == npair - 1:
                        finish(0)
                if ats_prev is not None:
                    outproj(ats_prev, c - 1)
                finish(1)
                ats_prev = ats
            outproj(ats_prev, NCHUNK - 1, spread=True)
    nc.finalize()
    return nc


def _host_prep(xs, norm_w, wq, wk, wv, wo):
    """Fold RMSNorm into h^T upload + weights; build rope tables; bf16."""
    import ml_dtypes
    BF = ml_dtypes.bfloat16

    x64 = xs.astype(np.float64)
    istd = 1.0 / np.sqrt((x64 * x64).mean(axis=1) + EPS)    # [S]
    ht = (x64 * istd[:, None]).T.astype(BF)                 # [D, S]
    # partition-major pack: ht_pm[p, dt, s] = ht[dt*128+p, s]
    ht_pm = np.ascontiguousarray(
        ht.reshape(DT, 128, SEQ).transpose(1, 0, 2))

    def pack_w(w):  # [D, M] -> [128, DT*M] partition-major
        return np.ascontiguousarray(
            w.reshape(DT, 128, M).transpose(1, 0, 2).reshape(128, DT * M))

    nw = norm_w.astype(np.float32)[:, None, None]
    perm = np.concatenate([np.arange(0, HD, 2), np.arange(1, HD, 2)])
    wq_p = (wq * nw)[:, :, perm]
    wk_p = (wk * nw)[:, :, perm]

    inv_freq = 1.0 / (ROPE_BASE ** (np.arange(0, HD, 2, dtype=np.float32) / HD))
    pos = np.arange(SEQ, dtype=np.float32)
    ang = pos[:, None] * inv_freq[None, :]          # [S, 64]
    cos_t = np.cos(ang).T.astype(np.float32)        # [64, S]
    sin_t = np.sin(ang).T.astype(np.float32)
    cosd = np.concatenate([cos_t, cos_t], 0)        # [128, S]
    sind = np.concatenate([sin_t, sin_t], 0)
    cossin = np.ascontiguousarray(np.stack([cosd, sind], axis=1))

    tri = np.triu(np.ones((128, 128), dtype=np.float32)).astype(BF)
    onesm = np.ones((128, 128), dtype=BF)

    common = {
        "ht": ht_pm,
        "cossin": cossin,
        "tri": np.ascontiguousarray(tri),
        "ones": onesm,
    }
    in_maps = []
    for core in range(NCORES):
        sl = slice(core * HPC, (core + 1) * HPC)
        wot = np.transpose(wo[:, sl, :], (2, 1, 0)).reshape(128, HPC * D)
        in_maps.append({
            **common,
            "wq": pack_w(wq_p[:, sl, :].reshape(D, M).astype(BF)),
            "wk": pack_w(wk_p[:, sl, :].reshape(D, M).astype(BF)),
            "wv": pack_w((wv * nw)[:, sl, :].reshape(D, M).astype(BF)),
            "wot": np.ascontiguousarray(wot.astype(BF)),
        })
    return in_maps


def kernel(xs, norm_w, wq, wk, wv, wo):
    trace = bool(int(os.environ.get("KERNEL_TRACE", "0")))
    if trace:
        _inject_ntff_hook()
    from concourse.bass_utils import run_bass_kernel_spmd

    nc = _build_nc()
    in_maps = _host_prep(np.asarray(xs), np.asarray(norm_w), np.asarray(wq),
                         np.asarray(wk), np.asarray(wv), np.asarray(wo))
    try:
        res = run_bass_kernel_spmd(nc, in_maps, core_ids=list(range(NCORES)),
                                   trace=trace)
    except Exception:
        # transient device wedge (NRT_EXEC_UNIT_UNRECOVERABLE) recovers on
        # a fresh attempt; rebuild and retry once
        import time
        time.sleep(15)
        nc = _build_nc()
        res = run_bass_kernel_spmd(nc, in_maps, core_ids=list(range(NCORES)),
                                   trace=trace)
    if trace and res.exec_time_ns is not None:
        print(f"HW exec time: {res.exec_time_ns} ns")
    acc = np.zeros((SEQ, D), dtype=np.float64)
    for r in res.results:
        acc += r["out"].astype(np.float64)
    return acc.astype(np.float32)


if __name__ == "__main__":
    rng = np.random.default_rng(0)
    scale = 1.0 / np.sqrt(D)
    inputs = {
        "xs": rng.standard_normal((SEQ, D), dtype=np.float32),
        "norm_w": np.ones((D,), np.float32),
        "wq": rng.standard_normal((D, NH, HD), dtype=np.float32) * scale,
        "wk": rng.standard_normal((D, NH, HD), dtype=np.float32) * scale,
        "wv": rng.standard_normal((D, NH, HD), dtype=np.float32) * scale,
        "wo": rng.standard_normal((D, NH, HD), dtype=np.float32) * scale,
    }
    out = kernel(**inputs)
    print(out.shape, out.dtype, float(np.abs(out).max()))
